# revision 1
# baseline (speedup 1.0000x reference)
"""Trainium2 Bass kernel: DGCNN forward (4-layer GCN + Conv1d readout) on 8 NeuronCores.

Math restructuring (verified vs reference to 2e-7):
  With A = D^-1/2 (Adj + I) D^-1/2 and Mk / ck derived from the (tiny) weights,
    out = A(x M1 + A(x M2 + A(x M3 + A(x M4)))) + 1 c0 + v1 c1 + v2 c2 + v3 c3
  where vk = A^k 1 (graph-only vectors).  Every aggregation pass is width-16.

Device strategy (graph-parallel over 8 cores):
  - Nodes are permuted: degree-sorted, dealt into 128-row blocks round-robin
    across cores, so each core owns 49 blocks (6272 rows) spanning the degree
    spectrum and same-index blocks across cores have near-equal max degree.
  - Per pass: messages are fetched with dma_gather (256B rows) from a DRAM
    table into an ELL-padded [128, slots, 64] tile (dst j of block b -> SBUF
    partition j), then segment-summed with one strided DVE reduce per block.
  - int16 gather indices limit a window to 32768 rows; the 50176-row table is
    covered by two overlapping windows ([0,32768) and [17408,50176)), and each
    dst's edge list is split between the windows (balanced using the overlap),
    padded with a dedicated all-zero table row.
  - After each pass every core computes its own rows of the next table
    (T = dinv*(x Mk + S), via PE matmuls) and an 8-core AllGather rebuilds the
    full table in DRAM.
  - dinv pre/post scaling is folded into the table rows, so no per-edge
    normalization multiplies are needed.
"""

import dataclasses
import numpy as np

import concourse.bass as bass
import concourse.bacc as bacc
import concourse.tile as tile
from concourse import mybir
from concourse.bass_utils import run_bass_kernel_spmd
from concourse.masks import make_identity

F32 = mybir.dt.float32
I16 = mybir.dt.int16
AF = mybir.ActivationFunctionType


@dataclasses.dataclass(frozen=True)
class Cfg:
    N: int = 50000          # real nodes
    F: int = 64             # features
    NCORES: int = 8
    P: int = 128
    NBLK: int = 49          # dst blocks per core
    NGRP: int = 16          # gather groups per pass
    NQ: int = 4             # SWDGE queues (parallel Q7 desc-gen pairs)

    @property
    def PER(self):
        return self.NBLK * self.P

    @property
    def NPAD(self):
        return self.NCORES * self.PER

    @property
    def WA_LEN(self):
        return min(32768, self.NPAD)

    @property
    def WB_OFF(self):
        return self.NPAD - self.WA_LEN


CFG = Cfg()

# results of the last device run (for test harness profiling)
LAST_RESULTS = None


# --------------------------------------------------------------------------
# host preprocessing
# --------------------------------------------------------------------------

def _host_prep(inputs, cfg: Cfg):
    x = np.asarray(inputs["x"], np.float32)
    ei = np.asarray(inputs["edge_index"]).astype(np.int64)
    W = [np.asarray(inputs[f"W{i}"], np.float64) for i in range(4)]
    b = [np.asarray(inputs[f"b{i}"], np.float64) for i in range(4)]
    conv_w = np.asarray(inputs["conv_w"], np.float64)
    conv_b = np.asarray(inputs["conv_b"], np.float64)

    n = x.shape[0]
    assert n == cfg.N and x.shape[1] == cfg.F
    P, PER, NPAD, NBLK, NC = cfg.P, cfg.PER, cfg.NPAD, cfg.NBLK, cfg.NCORES

    src = np.concatenate([ei[0], np.arange(n, dtype=np.int64)])
    dst = np.concatenate([ei[1], np.arange(n, dtype=np.int64)])
    deg = np.bincount(dst, minlength=n).astype(np.float64)
    dinv = 1.0 / np.sqrt(np.maximum(deg, 1.0))

    # ---- weight-derived small matrices ----
    Cw = [conv_w[:, 0:64], conv_w[:, 64:128], conv_w[:, 128:192], conv_w[:, 192:193]]
    M1 = W[0] @ Cw[0].T
    M2 = W[0] @ W[1] @ Cw[1].T
    M3 = W[0] @ W[1] @ W[2] @ Cw[2].T
    M4 = W[0] @ W[1] @ W[2] @ W[3] @ Cw[3].T
    c0 = b[0] @ Cw[0].T + b[1] @ Cw[1].T + b[2] @ Cw[2].T + b[3] @ Cw[3].T + conv_b
    c1 = (b[0] @ W[1]) @ Cw[1].T + (b[1] @ W[2]) @ Cw[2].T + (b[2] @ W[3]) @ Cw[3].T
    c2 = (b[0] @ W[1] @ W[2]) @ Cw[2].T + (b[1] @ W[2] @ W[3]) @ Cw[3].T
    c3 = (b[0] @ W[1] @ W[2] @ W[3]) @ Cw[3].T

    def aggv(v):
        o = np.zeros(n)
        np.add.at(o, dst, (v * dinv)[src])
        return o * dinv

    v1 = aggv(np.ones(n))
    v2 = aggv(v1)
    v3 = aggv(v2)
    bias = (np.outer(np.ones(n), c0) + np.outer(v1, c1)
            + np.outer(v2, c2) + np.outer(v3, c3))  # [n, 16]

    # ---- permutation: degree sort, deal into blocks; group blocks by size ----
    order = np.argsort(-deg, kind="stable")
    order_p = np.concatenate([order, np.full(NPAD - n, -1, np.int64)])
    assert NPAD - n >= 2
    # force a dummy (all-zero row) into (core 0, class NBLK-1, j=P-1)
    rA = ((NBLK - 1) * NC + 0) * P + (P - 1)
    order_p[rA], order_p[NPAD - 1] = order_p[NPAD - 1], order_p[rA]

    # degree-rank class of each padded rank (class = which 128x8 stripe)
    cls_of_rank = np.arange(NPAD) // P // NC
    # class max total degree (self-loop included) for grouping estimate
    deg_p = np.zeros(NPAD)
    real_rank = order_p >= 0
    deg_p[real_rank] = deg[order_p[real_rank]]
    mTc = np.zeros(NBLK)
    np.maximum.at(mTc, cls_of_rank, deg_p)

    # group classes into NGRP gather groups, balancing estimated slots
    per_grp = NBLK // cfg.NGRP + (1 if NBLK % cfg.NGRP else 0)
    groups_c = [[] for _ in range(cfg.NGRP)]
    gsum = np.zeros(cfg.NGRP)
    for bq in np.argsort(-mTc, kind="stable"):
        cand = sorted(range(cfg.NGRP),
                      key=lambda q: (len(groups_c[q]) >= per_grp, gsum[q], q))
        q = cand[0]
        groups_c[q].append(int(bq))
        gsum[q] += mTc[bq]
    # renumber classes -> local block ids, groups 0..NGRP/2-1 first (contiguous
    # row range for the first-half collective)
    order_cls = [c for q in range(cfg.NGRP) for c in groups_c[q]]
    renum = np.zeros(NBLK, np.int64)
    for newid, c in enumerate(order_cls):
        renum[c] = newid
    nblk_h1 = sum(len(groups_c[q]) for q in range(cfg.NGRP // 2))
    groups = []
    pos = 0
    for q in range(cfg.NGRP):
        groups.append(list(range(pos, pos + len(groups_c[q]))))
        pos += len(groups_c[q])

    g = np.arange(NPAD) // P
    j = np.arange(NPAD) % P
    npos_of_rank = (g % NC) * PER + renum[g // NC] * P + j
    pos2old = np.full(NPAD, -1, np.int64)
    pos2old[npos_of_rank] = order_p
    old2new = np.full(n, -1, np.int64)
    rmask = pos2old >= 0
    old2new[pos2old[rmask]] = np.nonzero(rmask)[0]

    zA = int(renum[NBLK - 1]) * P + (P - 1)          # dummy row, core 0
    zB = (NC - 1) * PER + int(renum[NBLK - 1]) * P + (P - 2)  # dummy row, core NC-1
    assert pos2old[zA] < 0 and pos2old[zB] < 0
    assert zA < cfg.WA_LEN and zB >= cfg.WB_OFF

    # ---- per-edge window split, minimal per-block slot budgets ----
    s_new = old2new[src]
    d_new = old2new[dst]
    eo = np.argsort(d_new, kind="stable")
    s_s = s_new[eo]
    d_s = d_new[eo]
    E = len(s_s)
    starts = np.searchsorted(d_s, np.arange(NPAD + 1))

    isA = s_s < cfg.WB_OFF
    isB = s_s >= cfg.WA_LEN
    isF = ~(isA | isB)
    nAo = np.bincount(d_s, weights=isA, minlength=NPAD).astype(np.int64)
    nBo = np.bincount(d_s, weights=isB, minlength=NPAD).astype(np.int64)
    nf = np.bincount(d_s, weights=isF, minlength=NPAD).astype(np.int64)
    tot = nAo + nBo + nf

    blk_pos = (np.arange(NPAD) % PER) // P
    mT = np.zeros(NBLK, np.int64); np.maximum.at(mT, blk_pos, tot)
    mA = np.zeros(NBLK, np.int64); np.maximum.at(mA, blk_pos, nAo)
    mB = np.zeros(NBLK, np.int64); np.maximum.at(mB, blk_pos, nBo)
    M = np.maximum(mT, mA + mB)
    SA = np.clip((M + 1) // 2, mA, M - mB)
    SA = np.maximum(SA, 1)
    SB = M - SA
    SAp = SA[blk_pos]
    SBp = SB[blk_pos]
    nA = np.clip(tot - SBp, nAo, np.minimum(nAo + nf, SAp))

    cFex = np.concatenate([[0], np.cumsum(isF)])
    frank = cFex[:-1] - cFex[starts[d_s]]
    goA = isA | (isF & (frank < (nA - nAo)[d_s]))
    goB = ~goA
    cAex = np.concatenate([[0], np.cumsum(goA)])
    slotA = cAex[:-1] - cAex[starts[d_s]]
    cBex = np.concatenate([[0], np.cumsum(goB)])
    slotB = cBex[:-1] - cBex[starts[d_s]]
    nB = tot - nA
    assert (nA <= SAp).all() and (nB <= SBp).all()

    oa = np.zeros(NBLK, np.int64)
    ob = np.zeros(NBLK, np.int64)
    grp_of = np.zeros(NBLK, np.int64)
    SAg = np.zeros(cfg.NGRP, np.int64)
    SBg = np.zeros(cfg.NGRP, np.int64)
    for q, bl in enumerate(groups):
        offa = 0
        for bq in bl:
            oa[bq] = offa
            offa += SA[bq]
            grp_of[bq] = q
        offb = 0
        for bq in bl:
            ob[bq] = offb
            offb += SB[bq]
        SAg[q] = offa
        SBg[q] = offb

    colA0 = np.zeros(cfg.NGRP, np.int64)
    colB0 = np.zeros(cfg.NGRP, np.int64)
    cur = 0
    for q in range(cfg.NGRP):
        colA0[q] = cur
        cur += SAg[q] * P // 16
        colB0[q] = cur
        cur += SBg[q] * P // 16
    idxcols = int(cur)

    # ---- build per-core idx tensors ----
    zA_rel = np.int16(zA)
    zB_rel = np.int16(zB - cfg.WB_OFF)
    idx_np = np.empty((NC, 128, idxcols), np.int16)
    # defaults: zero-row padding everywhere
    for q in range(cfg.NGRP):
        idx_np[:, :, colA0[q]:colA0[q] + SAg[q] * P // 16] = zA_rel
        idx_np[:, :, colB0[q]:colB0[q] + SBg[q] * P // 16] = zB_rel

    e_core = d_s // PER
    e_blk = (d_s % PER) // P
    e_j = d_s % P
    e_q = grp_of[e_blk]
    # linear position within the group's gather + column in the idx tensor
    posA = (oa[e_blk] + slotA) * P + e_j
    colA = colA0[e_q] + posA // 16
    rowA = posA % 16
    posB = (ob[e_blk] + slotB) * P + e_j
    colB = colB0[e_q] + posB // 16
    rowB = posB % 16
    valA = s_s.astype(np.int16)                   # window A offset is 0
    valB = (s_s - cfg.WB_OFF).astype(np.int16)
    for k in range(NC):
        mk = e_core == k
        mA = mk & goA
        mB = mk & goB
        for r in range(8):
            idx_np[k, rowA[mA] + 16 * r, colA[mA]] = valA[mA]
            idx_np[k, rowB[mB] + 16 * r, colB[mB]] = valB[mB]

    # ---- dense per-core arrays ----
    x_perm = np.zeros((NPAD, cfg.F), np.float32)
    x_perm[rmask] = x[pos2old[rmask]]
    dinv_perm = np.ones(NPAD, np.float32)
    dinv_perm[rmask] = dinv[pos2old[rmask]].astype(np.float32)
    bias_perm = np.zeros((NPAD, 16), np.float32)
    bias_perm[rmask] = bias[pos2old[rmask]].astype(np.float32)

    dinv_rows = dinv_perm.reshape(P, NPAD // P).copy()
    xT = [np.ascontiguousarray(x_perm[k * PER:(k + 1) * PER].T) for k in range(NC)]
    dinv_blk = [np.ascontiguousarray(dinv_perm[k * PER:(k + 1) * PER].reshape(NBLK, P).T)
                for k in range(NC)]
    dinv2_blk = [d * d for d in dinv_blk]
    bias_blk = [np.ascontiguousarray(
        bias_perm[k * PER:(k + 1) * PER].reshape(NBLK, P, 16).transpose(1, 0, 2))
        for k in range(NC)]
    mmats = np.ascontiguousarray(np.concatenate([M3, M2, M1], axis=1).astype(np.float32))
    m4 = np.ascontiguousarray(M4.astype(np.float32))

    layout = dict(SA=SA, SB=SB, groups=groups, oa=oa, ob=ob, SAg=SAg, SBg=SBg,
                  colA0=colA0, colB0=colB0, idxcols=idxcols, nblk_h1=nblk_h1)
    in_maps = []
    for k in range(NC):
        in_maps.append(dict(
            xraw=x_perm,
            idx=np.ascontiguousarray(idx_np[k]),
            xT=xT[k],
            dinv_rows=dinv_rows,
            dinv_blk=dinv_blk[k],
            dinv2_blk=dinv2_blk[k],
            bias_blk=bias_blk[k],
            mmats=mmats,
            m4=m4,
        ))
    return in_maps, layout, old2new


# --------------------------------------------------------------------------
# device module
# --------------------------------------------------------------------------

def _build_module(cfg: Cfg, layout):
    P, PER, NPAD, NBLK, NC = cfg.P, cfg.PER, cfg.NPAD, cfg.NBLK, cfg.NCORES
    SA, SB = layout["SA"], layout["SB"]
    groups, oa, ob = layout["groups"], layout["oa"], layout["ob"]
    SAg, SBg = layout["SAg"], layout["SBg"]
    colA0, colB0 = layout["colA0"], layout["colB0"]
    idxcols = layout["idxcols"]
    NROW = NPAD // P          # rows per partition in (p c) layouts

    nc = bacc.Bacc("TRN2", target_bir_lowering=False, debug=False, num_devices=NC,
                   num_swdge_queues=cfg.NQ, dynamic_dma_scratch_size=49152)

    xraw = nc.dram_tensor("xraw", [NPAD, cfg.F], F32, kind="ExternalInput").ap()
    idx = nc.dram_tensor("idx", [128, idxcols], I16, kind="ExternalInput").ap()
    xT = nc.dram_tensor("xT", [cfg.F, PER], F32, kind="ExternalInput").ap()
    dinv_rows = nc.dram_tensor("dinv_rows", [P, NROW], F32, kind="ExternalInput").ap()
    dinv_blk = nc.dram_tensor("dinv_blk", [P, NBLK], F32, kind="ExternalInput").ap()
    dinv2_blk = nc.dram_tensor("dinv2_blk", [P, NBLK], F32, kind="ExternalInput").ap()
    bias_blk = nc.dram_tensor("bias_blk", [P, NBLK, 16], F32, kind="ExternalInput").ap()
    mmats = nc.dram_tensor("mmats", [cfg.F, 48], F32, kind="ExternalInput").ap()
    m4 = nc.dram_tensor("m4", [cfg.F, 16], F32, kind="ExternalInput").ap()
    out = nc.dram_tensor("out", [P, NBLK, 16], F32, kind="ExternalOutput").ap()

    with tile.TileContext(nc) as tc:
        with (
            tc.tile_pool(name="const", bufs=1) as cp,
            tc.tile_pool(name="dram", bufs=1, space="DRAM") as dp,
        ):
            idx_sb = cp.tile([128, idxcols], I16)
            nc.sync.dma_start(idx_sb[:], idx)
            xT_sb = cp.tile([cfg.F, PER], F32)
            nc.sync.dma_start(xT_sb[:], xT)
            mm_sb = cp.tile([cfg.F, 48], F32)
            nc.sync.dma_start(mm_sb[:], mmats)
            m4_sb = cp.tile([cfg.F, 16], F32)
            nc.sync.dma_start(m4_sb[:], m4)
            dr_sb = cp.tile([P, NROW], F32)
            nc.sync.dma_start(dr_sb[:], dinv_rows)
            db_sb = cp.tile([P, NBLK], F32)
            nc.sync.dma_start(db_sb[:], dinv_blk)
            d2_sb = cp.tile([P, NBLK], F32)
            nc.sync.dma_start(d2_sb[:], dinv2_blk)
            bias_sb = cp.tile([P, NBLK, 16], F32)
            nc.sync.dma_start(bias_sb[:], bias_blk)
            ident = cp.tile([P, P], F32)
            make_identity(nc, ident[:])

            xtab = dp.tile([NPAD, cfg.F], F32)
            ttabs = [dp.tile([NPAD, cfg.F], F32, name=f"ttab{i}") for i in range(2)]
            nblk_h1 = layout["nblk_h1"]
            nblk_h2 = NBLK - nblk_h1
            rows1, rows2 = nblk_h1 * P, nblk_h2 * P
            ccin1 = [dp.tile([rows1, 16], F32, name=f"ccin1_{i}") for i in range(3)]
            ccin2 = [dp.tile([rows2, 16], F32, name=f"ccin2_{i}") for i in range(3)]
            ccout1 = [dp.tile([NC * rows1, 16], F32, addr_space="Shared",
                              name=f"ccout1_{i}") for i in range(3)]
            ccout2 = [dp.tile([NC * rows2, 16], F32, addr_space="Shared",
                              name=f"ccout2_{i}") for i in range(3)]

            # ---- prologue: xtab = dinv * xraw ----
            xr_r = xraw.rearrange("(p c) f -> p c f", p=P)
            xt_r = xtab[:].rearrange("(p c) f -> p c f", p=P)
            nch = 4
            cc = NROW // nch
            assert cc * nch == NROW
            with tc.tile_pool(name="prol", bufs=2) as pp:
                for i in range(nch):
                    t = pp.tile([P, cc, cfg.F], F32, tag="prol")
                    nc.sync.dma_start(t[:], xr_r[:, i * cc:(i + 1) * cc, :])
                    nc.vector.tensor_tensor(
                        out=t[:], in0=t[:],
                        in1=dr_sb[:, i * cc:(i + 1) * cc].to_broadcast([P, cc, cfg.F]),
                        op=mybir.AluOpType.mult,
                    )
                    nc.sync.dma_start(xt_r[:, i * cc:(i + 1) * cc, :], t[:])

            with (
                tc.tile_pool(name="gath", bufs=3) as gp,
                tc.tile_pool(name="work", bufs=3) as wp,
                tc.tile_pool(name="stage", bufs=2) as sp,
                tc.tile_pool(name="psum", bufs=2, space="PSUM") as psp,
            ):
                qctr = [0]

                def next_q():
                    q = qctr[0] % cfg.NQ
                    qctr[0] += 1
                    return q

                def half_exchange(bidx, half, st_tile, target_tab):
                    ci = (ccin1 if half == 0 else ccin2)[bidx]
                    co = (ccout1 if half == 0 else ccout2)[bidx]
                    n_b = nblk_h1 if half == 0 else nblk_h2
                    base = 0 if half == 0 else nblk_h1
                    nc.sync.dma_start(ci[:].rearrange("(b p) f -> p b f", p=P),
                                      st_tile[:])
                    nc.gpsimd.collective_compute(
                        "AllGather", mybir.AluOpType.bypass,
                        replica_groups=[list(range(NC))],
                        ins=[ci[:]], outs=[co[:]],
                    )
                    for k in range(NC):
                        t = wp.tile([P, n_b, 16], F32, tag="restride")
                        nc.sync.dma_start(
                            t[:],
                            co[k * n_b * P:(k + 1) * n_b * P, :]
                            .rearrange("(b p) f -> p b f", p=P))
                        dst = target_tab[k * PER + base * P:
                                         k * PER + (base + n_b) * P, :]
                        nc.sync.dma_start(
                            dst.rearrange("(b p) f -> p b f", p=P)[:, :, 0:16], t[:])

                def run_pass(tab, width, epi, mid_hook=None, end_hook=None):
                    winA = tab[0:cfg.WA_LEN, :]
                    winB = tab[cfg.WB_OFF:NPAD, :]
                    for q, bl in enumerate(groups):
                        sag, sbg = int(SAg[q]), int(SBg[q])
                        s_all = sag + sbg
                        gt = gp.tile([P, s_all, cfg.F], F32, tag="gt")
                        nc.gpsimd.dma_gather(
                            out_ap=gt[:, 0:sag, :],
                            in_ap=winA,
                            idxs_ap=idx_sb[:, int(colA0[q]):int(colA0[q]) + sag * P // 16],
                            num_idxs=sag * P,
                            num_idxs_reg=sag * P,
                            elem_size=cfg.F,
                            single_packet=False,
                            queue_num=next_q(),
                        )
                        if sbg:
                            nc.gpsimd.dma_gather(
                                out_ap=gt[:, sag:s_all, :],
                                in_ap=winB,
                                idxs_ap=idx_sb[:, int(colB0[q]):int(colB0[q]) + sbg * P // 16],
                                num_idxs=sbg * P,
                                num_idxs_reg=sbg * P,
                                elem_size=cfg.F,
                                single_packet=False,
                                queue_num=next_q(),
                            )
                        for bq in bl:
                            a0, a1 = int(oa[bq]), int(oa[bq] + SA[bq])
                            acc = wp.tile([P, cfg.F], F32, tag="acc")
                            nc.vector.reduce_sum(
                                out=acc[:, 0:width],
                                in_=gt[:, a0:a1, 0:width].rearrange("p s f -> p f s"),
                                axis=mybir.AxisListType.X,
                            )
                            if SB[bq]:
                                b0_, b1_ = sag + int(ob[bq]), sag + int(ob[bq] + SB[bq])
                                acc2 = wp.tile([P, cfg.F], F32, tag="acc2")
                                nc.vector.reduce_sum(
                                    out=acc2[:, 0:width],
                                    in_=gt[:, b0_:b1_, 0:width].rearrange("p s f -> p f s"),
                                    axis=mybir.AxisListType.X,
                                )
                                nc.vector.tensor_add(
                                    out=acc[:, 0:width], in0=acc[:, 0:width],
                                    in1=acc2[:, 0:width])
                            epi(bq, acc)
                        if mid_hook is not None and q == cfg.NGRP // 2 - 1:
                            mid_hook()
                    if end_hook is not None:
                        end_hook()

                def make_stage():
                    st1 = sp.tile([P, nblk_h1, 16], F32, tag="stage1")
                    st2 = sp.tile([P, nblk_h2, 16], F32, tag="stage2")
                    return st1, st2

                def st_slot(st1, st2, b):
                    return st1[:, b, :] if b < nblk_h1 else st2[:, b - nblk_h1, :]

                # ---- pass 4: gather x-table, build T3 into ttabs[0] ----
                st1, st2 = make_stage()

                def epi4(bq, R):
                    rs = wp.tile([P, cfg.F], F32, tag="rs")
                    nc.scalar.activation(rs[:], R[:], AF.Copy, scale=db_sb[:, bq:bq + 1])
                    pT = psp.tile([cfg.F, P], F32, tag="pT")
                    nc.tensor.transpose(pT[:], rs[:], ident[:])
                    rsT = wp.tile([cfg.F, P], F32, tag="rsT")
                    nc.vector.tensor_copy(rsT[:], pT[:])
                    ps = psp.tile([P, 16], F32, tag="ps")
                    nc.tensor.matmul(out=ps[:], lhsT=xT_sb[:, bq * P:(bq + 1) * P],
                                     rhs=mm_sb[:, 0:16], start=True, stop=False)
                    nc.tensor.matmul(out=ps[:], lhsT=rsT[:], rhs=m4_sb[:],
                                     start=False, stop=True)
                    nc.scalar.activation(st_slot(st1, st2, bq), ps[:], AF.Copy,
                                         scale=db_sb[:, bq:bq + 1])

                run_pass(xtab[:], cfg.F, epi4,
                         mid_hook=lambda: half_exchange(0, 0, st1, ttabs[0]),
                         end_hook=lambda: half_exchange(0, 1, st2, ttabs[0]))

                # ---- passes 3 and 2 ----
                def mk_epi(mcol, st1, st2):
                    def epi(bq, R):
                        ps = psp.tile([P, 16], F32, tag="ps")
                        nc.tensor.matmul(out=ps[:],
                                         lhsT=xT_sb[:, bq * P:(bq + 1) * P],
                                         rhs=mm_sb[:, mcol:mcol + 16],
                                         start=True, stop=True)
                        ta = wp.tile([P, 16], F32, tag="ta")
                        nc.scalar.activation(ta[:], ps[:], AF.Copy,
                                             scale=db_sb[:, bq:bq + 1])
                        tb = wp.tile([P, 16], F32, tag="tb")
                        nc.scalar.activation(tb[:], R[:, 0:16], AF.Copy,
                                             scale=d2_sb[:, bq:bq + 1])
                        nc.vector.tensor_add(out=st_slot(st1, st2, bq),
                                             in0=ta[:], in1=tb[:])
                    return epi

                st1, st2 = make_stage()
                run_pass(ttabs[0][:], 16, mk_epi(16, st1, st2),
                         mid_hook=lambda: half_exchange(1, 0, st1, ttabs[1]),
                         end_hook=lambda: half_exchange(1, 1, st2, ttabs[1]))

                st1, st2 = make_stage()
                run_pass(ttabs[1][:], 16, mk_epi(32, st1, st2),
                         mid_hook=lambda: half_exchange(2, 0, st1, ttabs[0]),
                         end_hook=lambda: half_exchange(2, 1, st2, ttabs[0]))

                # ---- pass 1: final output ----
                sto1, sto2 = make_stage()

                def epi1(bq, R):
                    t1 = wp.tile([P, 16], F32, tag="ta")
                    nc.scalar.activation(t1[:], R[:, 0:16], AF.Copy,
                                         scale=db_sb[:, bq:bq + 1])
                    nc.vector.tensor_add(out=st_slot(sto1, sto2, bq), in0=t1[:],
                                         in1=bias_sb[:, bq, :])

                run_pass(ttabs[0][:], 16, epi1)
                nc.sync.dma_start(out[:, 0:nblk_h1, :], sto1[:])
                nc.sync.dma_start(out[:, nblk_h1:NBLK, :], sto2[:])

    return nc


# --------------------------------------------------------------------------
# entry point
# --------------------------------------------------------------------------

def _run(inputs, cfg: Cfg, runner=None, **run_kwargs):
    """runner(nc, in_maps) -> list[dict] allows sim injection for testing."""
    global LAST_RESULTS
    in_maps, layout, old2new = _host_prep(inputs, cfg)
    nc = _build_module(cfg, layout)
    nc.compile()
    if runner is None:
        res = run_bass_kernel_spmd(nc, in_maps, core_ids=list(range(cfg.NCORES)),
                                   **run_kwargs)
        LAST_RESULTS = res
        outs = res.results
    else:
        outs = runner(nc, in_maps)
    full = np.empty((cfg.NPAD, 16), np.float32)
    for k in range(cfg.NCORES):
        o = np.asarray(outs[k]["out"])  # [P, NBLK, 16]
        full[k * cfg.PER:(k + 1) * cfg.PER] = o.transpose(1, 0, 2).reshape(cfg.PER, 16)
    return full[old2new]


def kernel(**inputs) -> np.ndarray:
    return _run(inputs, CFG)



# revision 11
# speedup vs baseline: 1.4127x; 1.4127x over previous
"""Trainium2 Bass kernel: DGCNN forward (4-layer GCN + Conv1d readout) on 8 NeuronCores.

Math restructuring (validated vs reference to 2e-7):
  With A = D^-1/2 (Adj + I) D^-1/2 and Mk / ck derived from the (tiny) weights,
    out = A(x M1 + A(x M2 + A(x M3 + A(x M4)))) + 1 c0 + v1 c1 + v2 c2 + v3 c3
  Passes aggregate tables T; self-loop contributions are added in the epilogue
  from SBUF-resident data (previous pass's stage), so gathers cover only real
  edges.  Pass 4 gathers the 64-wide dinv*x table (host pre-scaled); its
  epilogue projects through M4.  Passes 3/2/1 gather 16-wide tables.

Device strategy (graph-parallel over 8 cores):
  - dma_gather (SWDGE) is descriptor-rate-bound (~8.1ns/desc per queue, 4
    queues scale linearly), so the kernel minimizes descriptors and keeps all
    4 queues fed:
    * nodes are placed into 128-row blocks clustered by (degree, #window-A
      sources, #window-B sources) signature, cutting ELL padding to ~5%
    * blocks are dealt into 8-wide "classes" (one block per core) so the SPMD
      module has uniform shapes; class slot budgets are cross-core maxes
    * gather tile pool is 6 deep and ~36 gather calls/pass rotate over the 4
      SWDGE queues so descriptor generation runs ~4-way concurrent
  - int16 gather indices limit a window to 32768 rows; the 50176-row table is
    covered by two overlapping windows ([0,32768) and [17408,50176)); each
    dst's edges are split between windows inside its class budgets SA/SB.
  - The table is laid out in 4 exchange chunks ([17,15,13,4] blocks/core,
    region-aligned) so each AllGather output is a contiguous table range; a
    single DRAM->DRAM DMA restrides [rows,16] into the 256B-row table.  The
    first 3 chunk exchanges overlap the current pass's remaining gathers; only
    the small 4-block tail exchange sits on the pass boundary.
"""

import dataclasses
import numpy as np

import concourse.bass as bass
import concourse.bacc as bacc
import concourse.tile as tile
from concourse import mybir
from concourse.bass_utils import run_bass_kernel_spmd
from concourse.masks import make_identity

F32 = mybir.dt.float32
I16 = mybir.dt.int16
AF = mybir.ActivationFunctionType


@dataclasses.dataclass(frozen=True)
class Cfg:
    N: int = 50000          # real nodes
    F: int = 64             # features
    NCORES: int = 8
    P: int = 128
    NBLK: int = 49          # blocks (classes) per core
    NQ: int = 4             # SWDGE queues
    GT_BUFS: int = 4        # gather tile pool depth
    GRP_TARGET: int = 3     # classes per gather group (approx)

    # exchange chunks: (region, blocks-per-core); regions are the int16
    # window-split areas: R0=[0,17408) R1=[17408,32768) R2=[32768,50176)
    CHUNKS = ((0, 17), (1, 15), (2, 13), (2, 4))

    @property
    def PER(self):
        return self.NBLK * self.P

    @property
    def NPAD(self):
        return self.NCORES * self.PER

    @property
    def WA_LEN(self):
        return 32768

    @property
    def WB_OFF(self):
        return self.NPAD - 32768


CFG = Cfg()

LAST_RESULTS = None


# --------------------------------------------------------------------------
# host preprocessing
# --------------------------------------------------------------------------

def _host_prep(inputs, cfg: Cfg):
    x = np.asarray(inputs["x"], np.float32)
    ei = np.asarray(inputs["edge_index"]).astype(np.int64)
    W = [np.asarray(inputs[f"W{i}"], np.float64) for i in range(4)]
    b = [np.asarray(inputs[f"b{i}"], np.float64) for i in range(4)]
    conv_w = np.asarray(inputs["conv_w"], np.float64)
    conv_b = np.asarray(inputs["conv_b"], np.float64)

    n = x.shape[0]
    assert n == cfg.N and x.shape[1] == cfg.F
    P, PER, NPAD, NBLK, NC = cfg.P, cfg.PER, cfg.NPAD, cfg.NBLK, cfg.NCORES

    src0, dst0 = ei[0], ei[1]           # real edges only; self-loops in epilogue
    E0 = len(src0)
    deg = np.bincount(dst0, minlength=n).astype(np.float64) + 1.0  # incl self
    dinv = 1.0 / np.sqrt(deg)

    # ---- weight-derived small matrices ----
    Cw = [conv_w[:, 0:64], conv_w[:, 64:128], conv_w[:, 128:192], conv_w[:, 192:193]]
    M1 = W[0] @ Cw[0].T
    M2 = W[0] @ W[1] @ Cw[1].T
    M3 = W[0] @ W[1] @ W[2] @ Cw[2].T
    M4 = W[0] @ W[1] @ W[2] @ W[3] @ Cw[3].T
    c0 = b[0] @ Cw[0].T + b[1] @ Cw[1].T + b[2] @ Cw[2].T + b[3] @ Cw[3].T + conv_b
    c1 = (b[0] @ W[1]) @ Cw[1].T + (b[1] @ W[2]) @ Cw[2].T + (b[2] @ W[3]) @ Cw[3].T
    c2 = (b[0] @ W[1] @ W[2]) @ Cw[2].T + (b[1] @ W[2] @ W[3]) @ Cw[3].T
    c3 = (b[0] @ W[1] @ W[2] @ W[3]) @ Cw[3].T

    def aggv(v):
        o = np.zeros(n)
        np.add.at(o, dst0, (v * dinv)[src0])
        o += v * dinv
        return o * dinv

    v1 = aggv(np.ones(n))
    v2 = aggv(v1)
    v3 = aggv(v2)
    bias = (np.outer(np.ones(n), c0) + np.outer(v1, c1)
            + np.outer(v2, c2) + np.outer(v3, c3))  # [n, 16]

    # ---- region assignment + signatures ----
    # region position ranges (block aligned): R0 [0,17408) R1 [17408,32768)
    # R2 [32768,50176); pads: 2 at end of R0, 2 at end of R1, 172 end of R2
    RSTART = np.array([0, 17408, 32768, NPAD])
    RCAP = np.array([17408 - 2, 15360 - 2, 17408 - 172])
    assert RCAP.sum() == n
    region_of_node = np.repeat(np.arange(3), RCAP)  # node id order
    sreg = region_of_node[src0]
    gdeg = np.bincount(dst0, minlength=n).astype(np.int64)
    nA_n = np.bincount(dst0, weights=(sreg == 0), minlength=n).astype(np.int64)
    nB_n = np.bincount(dst0, weights=(sreg == 2), minlength=n).astype(np.int64)

    # chunk layout: per core, chunks of blocks; classes indexed 0..NBLK-1
    chunks = cfg.CHUNKS
    nb_of_chunk = [c[1] for c in chunks]
    assert sum(nb_of_chunk) == NBLK
    chunk_start_pos = np.concatenate([[0], np.cumsum([NC * nb * P for nb in nb_of_chunk])])
    chunk_cls0 = np.concatenate([[0], np.cumsum(nb_of_chunk)])

    # per region: sort real nodes by signature, form blocks, rank into classes
    pos_of_node = np.full(n, -1, np.int64)
    cls_cost = np.zeros(NBLK, np.int64)
    for r in range(3):
        nodes = np.nonzero(region_of_node == r)[0]
        k = np.lexsort((nB_n[nodes], nA_n[nodes], gdeg[nodes] // 4))
        nodes = nodes[k]
        nblocks_r = (RSTART[r + 1] - RSTART[r]) // P
        ncls_r = nblocks_r // NC
        # blocks of 128 consecutive sorted nodes (pads fill the tail)
        nfull = len(nodes)
        # block cost = max(gdeg, nA+nB) over its nodes (pads contribute 0)
        bcost = np.zeros(nblocks_r, np.int64)
        bi = np.arange(nfull) // P
        np.maximum.at(bcost, bi, np.maximum(gdeg[nodes], nA_n[nodes] + nB_n[nodes]))
        # class i = NC CONSECUTIVE blocks in signature order (keeps budgets
        # tight); rank classes by cost only to route small ones to the tail
        # chunk; per-core load is Sum(class budgets) regardless (1 blk/core).
        ccost = bcost.reshape(ncls_r, NC).max(axis=1)
        cls_rank = np.argsort(-ccost, kind="stable")  # region-class in cost order
        rchunks = [ci for ci, c in enumerate(chunks) if c[0] == r]
        cls_slots = []  # (chunk_idx, slot_in_chunk); tail chunk listed last
        for ci in rchunks:
            cls_slots += [(ci, s) for s in range(nb_of_chunk[ci])]
        assert len(cls_slots) == ncls_r
        for i in range(ncls_r):
            ci, slot = cls_slots[i]
            cls_id = chunk_cls0[ci] + slot
            rc = cls_rank[i]                      # region-class index
            for kc in range(NC):
                blk = rc * NC + kc
                base = (chunk_start_pos[ci] + kc * nb_of_chunk[ci] * P + slot * P)
                lo, hi = blk * P, min((blk + 1) * P, nfull)
                if hi > lo:
                    pos_of_node[nodes[lo:hi]] = base + np.arange(hi - lo)
                cls_cost[cls_id] = max(cls_cost[cls_id], bcost[blk])

    assert (pos_of_node >= 0).all()
    pos2old = np.full(NPAD, -1, np.int64)
    pos2old[pos_of_node] = np.arange(n)

    # position -> (core, local block b, j)
    pos = np.arange(NPAD)
    core_of_pos = np.zeros(NPAD, np.int64)
    cls_of_pos = np.zeros(NPAD, np.int64)
    for ci, (r, nb) in enumerate(chunks):
        s, e = chunk_start_pos[ci], chunk_start_pos[ci + 1]
        rel = pos[s:e] - s
        core_of_pos[s:e] = rel // (nb * P)
        cls_of_pos[s:e] = chunk_cls0[ci] + (rel % (nb * P)) // P
    j_of_pos = pos % P

    # dummy zero rows: a pad position in window A and one in window B
    pad_pos = np.nonzero(pos2old < 0)[0]
    zA = int(pad_pos[pad_pos < cfg.WA_LEN][-1])
    zB = int(pad_pos[pad_pos >= cfg.WB_OFF][-1])
    assert zA != zB

    # ---- per-edge window split with per-class budgets ----
    s_pos = pos_of_node[src0]
    d_pos = pos_of_node[dst0]
    eo = np.argsort(d_pos, kind="stable")
    s_s = s_pos[eo]
    d_s = d_pos[eo]
    starts = np.searchsorted(d_s, np.arange(NPAD + 1))

    isA = s_s < cfg.WB_OFF
    isB = s_s >= cfg.WA_LEN
    isF = ~(isA | isB)
    nAo = np.bincount(d_s, weights=isA, minlength=NPAD).astype(np.int64)
    nBo = np.bincount(d_s, weights=isB, minlength=NPAD).astype(np.int64)
    nf = np.bincount(d_s, weights=isF, minlength=NPAD).astype(np.int64)
    tot = nAo + nBo + nf

    cp = cls_of_pos
    mA = np.zeros(NBLK, np.int64); np.maximum.at(mA, cp, nAo)
    mB = np.zeros(NBLK, np.int64); np.maximum.at(mB, cp, nBo)
    mT = np.zeros(NBLK, np.int64); np.maximum.at(mT, cp, tot)
    M = np.maximum(mT, mA + mB)
    SA = np.clip((M + 1) // 2, mA, M - mB)
    SB = M - SA
    SAp = SA[cp]
    SBp = SB[cp]
    nA = np.clip(tot - SBp, nAo, np.minimum(nAo + nf, SAp))

    cFex = np.concatenate([[0], np.cumsum(isF)])
    frank = cFex[:-1] - cFex[starts[d_s]]
    goA = isA | (isF & (frank < (nA - nAo)[d_s]))
    goB = ~goA
    cAex = np.concatenate([[0], np.cumsum(goA)])
    slotA = cAex[:-1] - cAex[starts[d_s]]
    cBex = np.concatenate([[0], np.cumsum(goB)])
    slotB = cBex[:-1] - cBex[starts[d_s]]
    nB_ = tot - nA
    assert (nA <= SAp).all() and (nB_ <= SBp).all()
    assert (SA + SB > 0).all()

    # ---- groups: classes within each chunk, balanced by slots ----
    groups = []          # list of list of class ids
    grp_chunk = []       # chunk index of each group
    for ci, (r, nb) in enumerate(chunks):
        cls_list = list(range(chunk_cls0[ci], chunk_cls0[ci] + nb))
        ng = -(-nb // cfg.GRP_TARGET)
        # greedy balance by SA+SB
        order_d = sorted(cls_list, key=lambda c: -(SA[c] + SB[c]))
        gsets = [[] for _ in range(ng)]
        gsum = [0] * ng
        for c in order_d:
            q = min(range(ng), key=lambda i: (gsum[i], i))
            gsets[q].append(c)
            gsum[q] += SA[c] + SB[c]
        for g in gsets:
            groups.append(sorted(g))
            grp_chunk.append(ci)
    NGRP = len(groups)

    # slot offsets per class within its group's A/B gathers
    oa = np.zeros(NBLK, np.int64)
    ob = np.zeros(NBLK, np.int64)
    grp_of = np.zeros(NBLK, np.int64)
    SAg = np.zeros(NGRP, np.int64)
    SBg = np.zeros(NGRP, np.int64)
    for q, bl in enumerate(groups):
        offa = 0
        for bq in bl:
            oa[bq] = offa
            offa += SA[bq]
            grp_of[bq] = q
        offb = 0
        for bq in bl:
            ob[bq] = offb
            offb += SB[bq]
        SAg[q] = offa
        SBg[q] = offb

    colA0 = np.zeros(NGRP, np.int64)
    colB0 = np.zeros(NGRP, np.int64)
    cur = 0
    for q in range(NGRP):
        colA0[q] = cur
        cur += int(SAg[q]) * P // 16
        colB0[q] = cur
        cur += int(SBg[q]) * P // 16
    idxcols = int(cur)

    # ---- build per-core idx tensors ----
    zA_rel = np.int16(zA)
    zB_rel = np.int16(zB - cfg.WB_OFF)
    idx_np = np.empty((NC, 128, idxcols), np.int16)
    for q in range(NGRP):
        idx_np[:, :, colA0[q]:colA0[q] + int(SAg[q]) * P // 16] = zA_rel
        idx_np[:, :, colB0[q]:colB0[q] + int(SBg[q]) * P // 16] = zB_rel

    e_core = core_of_pos[d_s]
    e_cls = cls_of_pos[d_s]
    e_j = j_of_pos[d_s]
    e_q = grp_of[e_cls]
    posA = (oa[e_cls] + slotA) * P + e_j
    colA = colA0[e_q] + posA // 16
    rowA = posA % 16
    posB = (ob[e_cls] + slotB) * P + e_j
    colB = colB0[e_q] + posB // 16
    rowB = posB % 16
    valA = s_s.astype(np.int16)
    valB = (s_s - cfg.WB_OFF).astype(np.int16)
    for k in range(NC):
        mk = e_core == k
        mAk = mk & goA
        mBk = mk & goB
        for r in range(8):
            idx_np[k, rowA[mAk] + 16 * r, colA[mAk]] = valA[mAk]
            idx_np[k, rowB[mBk] + 16 * r, colB[mBk]] = valB[mBk]

    # ---- dense per-core arrays ----
    rmask = pos2old >= 0
    dinv_pos = np.ones(NPAD, np.float32)
    dinv_pos[rmask] = dinv[pos2old[rmask]].astype(np.float32)
    x_pos = np.zeros((NPAD, cfg.F), np.float32)
    x_pos[rmask] = x[pos2old[rmask]]
    bias_pos = np.zeros((NPAD, 16), np.float32)
    bias_pos[rmask] = bias[pos2old[rmask]].astype(np.float32)

    xraw_t = x_pos * dinv_pos[:, None]            # pass-4 table: dinv*x
    # per-core [j, b] layouts: position of (core k, cls b, j)
    pos_kbj = np.zeros((NC, NBLK, P), np.int64)
    for ci, (r, nb) in enumerate(chunks):
        for kc in range(NC):
            for s in range(nb):
                base = chunk_start_pos[ci] + kc * nb * P + s * P
                pos_kbj[kc, chunk_cls0[ci] + s] = base + np.arange(P)

    in_maps = []
    mmats = np.ascontiguousarray(np.concatenate([M3, M2, M1], axis=1).astype(np.float32))
    m4 = np.ascontiguousarray(M4.astype(np.float32))
    for k in range(NC):
        pk = pos_kbj[k]                            # [NBLK, P] positions
        db = dinv_pos[pk].T.astype(np.float32)     # [P, NBLK]
        d2 = (db * db).astype(np.float32)
        xTloc = np.ascontiguousarray(
            x_pos[pk.reshape(-1)].T)               # [F, PER] raw x
        xd2 = np.ascontiguousarray(x_pos[pk].transpose(1, 0, 2)
                                   * (dinv_pos[pk] ** 2).T[:, :, None]).astype(np.float32)
        bias_blk = np.ascontiguousarray(bias_pos[pk].transpose(1, 0, 2)).astype(np.float32)
        in_maps.append(dict(
            xraw=xraw_t,
            idx=np.ascontiguousarray(idx_np[k]),
            xT=xTloc,
            db=np.ascontiguousarray(db),
            d2=np.ascontiguousarray(d2),
            xd2=xd2,
            bias_blk=bias_blk,
            mmats=mmats,
            m4=m4,
        ))

    layout = dict(SA=SA, SB=SB, groups=groups, grp_chunk=grp_chunk, oa=oa, ob=ob,
                  SAg=SAg, SBg=SBg, colA0=colA0, colB0=colB0, idxcols=idxcols,
                  chunks=chunks, chunk_start_pos=chunk_start_pos,
                  chunk_cls0=chunk_cls0)
    return in_maps, layout, pos_kbj, pos2old


# --------------------------------------------------------------------------
# device module
# --------------------------------------------------------------------------

def _build_module(cfg: Cfg, layout):
    P, PER, NPAD, NBLK, NC = cfg.P, cfg.PER, cfg.NPAD, cfg.NBLK, cfg.NCORES
    SA, SB = layout["SA"], layout["SB"]
    groups, grp_chunk = layout["groups"], layout["grp_chunk"]
    oa, ob = layout["oa"], layout["ob"]
    SAg, SBg = layout["SAg"], layout["SBg"]
    colA0, colB0 = layout["colA0"], layout["colB0"]
    idxcols = layout["idxcols"]
    chunks = layout["chunks"]
    chunk_start_pos = layout["chunk_start_pos"]
    chunk_cls0 = layout["chunk_cls0"]
    NGRP = len(groups)
    NCH = len(chunks)

    nc = bacc.Bacc("TRN2", target_bir_lowering=False, debug=False, num_devices=NC,
                   num_swdge_queues=cfg.NQ, dynamic_dma_scratch_size=32768)

    xraw = nc.dram_tensor("xraw", [NPAD, cfg.F], F32, kind="ExternalInput").ap()
    idx = nc.dram_tensor("idx", [128, idxcols], I16, kind="ExternalInput").ap()
    xT = nc.dram_tensor("xT", [cfg.F, PER], F32, kind="ExternalInput").ap()
    db_in = nc.dram_tensor("db", [P, NBLK], F32, kind="ExternalInput").ap()
    d2_in = nc.dram_tensor("d2", [P, NBLK], F32, kind="ExternalInput").ap()
    xd2_in = nc.dram_tensor("xd2", [P, NBLK, cfg.F], F32, kind="ExternalInput").ap()
    bias_in = nc.dram_tensor("bias_blk", [P, NBLK, 16], F32, kind="ExternalInput").ap()
    mmats = nc.dram_tensor("mmats", [cfg.F, 48], F32, kind="ExternalInput").ap()
    m4 = nc.dram_tensor("m4", [cfg.F, 16], F32, kind="ExternalInput").ap()
    out = nc.dram_tensor("out", [P, NBLK, 16], F32, kind="ExternalOutput").ap()

    with tile.TileContext(nc) as tc:
        with (
            tc.tile_pool(name="const", bufs=1) as cp,
            tc.tile_pool(name="dram", bufs=1, space="DRAM") as dp,
        ):
            idx_sb = cp.tile([128, idxcols], I16)
            nc.sync.dma_start(idx_sb[:], idx)
            xT_sb = cp.tile([cfg.F, PER], F32)
            nc.sync.dma_start(xT_sb[:], xT)
            mm_sb = cp.tile([cfg.F, 48], F32)
            nc.sync.dma_start(mm_sb[:], mmats)
            m4_sb = cp.tile([cfg.F, 16], F32)
            nc.sync.dma_start(m4_sb[:], m4)
            db_sb = cp.tile([P, NBLK], F32)
            nc.sync.dma_start(db_sb[:], db_in)
            d2_sb = cp.tile([P, NBLK], F32)
            nc.sync.dma_start(d2_sb[:], d2_in)
            xd2_sb = cp.tile([P, NBLK, cfg.F], F32)
            nc.sync.dma_start(xd2_sb[:], xd2_in)
            bias_sb = cp.tile([P, NBLK, 16], F32)
            nc.sync.dma_start(bias_sb[:], bias_in)
            ident = cp.tile([P, P], F32)
            make_identity(nc, ident[:])

            ttabs = [dp.tile([NPAD, cfg.F], F32, name=f"ttab{i}") for i in range(2)]
            ccin = [dp.tile([nb * P, 16], F32, name=f"ccin{c}")
                    for c, (r, nb) in enumerate(chunks)]
            ccout = [[dp.tile([NC * nb * P, 16], F32, addr_space="Shared",
                              name=f"ccout{p}_{c}")
                      for c, (r, nb) in enumerate(chunks)] for p in range(3)]

            with (
                tc.tile_pool(name="gath", bufs=cfg.GT_BUFS) as gp,
                tc.tile_pool(name="work", bufs=4) as wp,
                tc.tile_pool(name="stage", bufs=2) as sp,
                tc.tile_pool(name="psum", bufs=4, space="PSUM") as psp,
            ):
                # greedy queue schedule: call (in issue order) -> least-loaded
                qload = [0] * cfg.NQ
                qsched = {}
                for q in range(NGRP):
                    for part, sz in (("A", int(SAg[q])), ("B", int(SBg[q]))):
                        if sz:
                            qq = min(range(cfg.NQ), key=lambda i: (qload[i], i))
                            qload[qq] += sz * P
                            qsched[(q, part)] = qq

                def make_stages(tag):
                    return [sp.tile([P, nb, 16], F32, tag=f"{tag}{c}",
                                    name=f"st_{tag}{c}")
                            for c, (r, nb) in enumerate(chunks)]

                def st_slot(sts, bq):
                    for c in range(NCH):
                        if bq < chunk_cls0[c + 1]:
                            return sts[c][:, bq - chunk_cls0[c], :]
                    raise AssertionError

                def exchange(p, c, st_tile, target_tab):
                    r, nb = chunks[c]
                    nc.sync.dma_start(
                        ccin[c][:].rearrange("(b p) f -> p b f", p=P), st_tile[:])
                    nc.gpsimd.collective_compute(
                        "AllGather", mybir.AluOpType.bypass,
                        replica_groups=[list(range(NC))],
                        ins=[ccin[c][:]], outs=[ccout[p][c][:]],
                    )
                    s = int(chunk_start_pos[c])
                    rows = NC * nb * P
                    nc.sync.dma_start(
                        target_tab[s:s + rows, 0:16], ccout[p][c][:])

                def run_pass(tab, width, epi, chunk_hook=None):
                    winA = tab[0:cfg.WA_LEN, :]
                    winB = tab[cfg.WB_OFF:NPAD, :]
                    prev_chunk = grp_chunk[0]
                    for q in range(NGRP):
                        if chunk_hook is not None and grp_chunk[q] != prev_chunk:
                            chunk_hook(prev_chunk)
                            prev_chunk = grp_chunk[q]
                        bl = groups[q]
                        sag, sbg = int(SAg[q]), int(SBg[q])
                        s_all = sag + sbg
                        gt = gp.tile([P, s_all, cfg.F], F32, tag="gt")
                        if sag:
                            nc.gpsimd.dma_gather(
                                out_ap=gt[:, 0:sag, :],
                                in_ap=winA,
                                idxs_ap=idx_sb[:, int(colA0[q]):int(colA0[q]) + sag * P // 16],
                                num_idxs=sag * P,
                                num_idxs_reg=sag * P,
                                elem_size=cfg.F,
                                single_packet=False,
                                queue_num=qsched[(q, "A")],
                            )
                        if sbg:
                            nc.gpsimd.dma_gather(
                                out_ap=gt[:, sag:s_all, :],
                                in_ap=winB,
                                idxs_ap=idx_sb[:, int(colB0[q]):int(colB0[q]) + sbg * P // 16],
                                num_idxs=sbg * P,
                                num_idxs_reg=sbg * P,
                                elem_size=cfg.F,
                                single_packet=False,
                                queue_num=qsched[(q, "B")],
                            )
                        for bq in bl:
                            a0, a1 = int(oa[bq]), int(oa[bq] + SA[bq])
                            acc = wp.tile([P, cfg.F], F32, tag="acc")
                            if SA[bq]:
                                nc.vector.reduce_sum(
                                    out=acc[:, 0:width],
                                    in_=gt[:, a0:a1, 0:width].rearrange("p s f -> p f s"),
                                    axis=mybir.AxisListType.X,
                                )
                            if SB[bq]:
                                b0_, b1_ = sag + int(ob[bq]), sag + int(ob[bq] + SB[bq])
                                if SA[bq]:
                                    acc2 = wp.tile([P, cfg.F], F32, tag="acc2")
                                    nc.vector.reduce_sum(
                                        out=acc2[:, 0:width],
                                        in_=gt[:, b0_:b1_, 0:width].rearrange("p s f -> p f s"),
                                        axis=mybir.AxisListType.X,
                                    )
                                    nc.vector.tensor_add(
                                        out=acc[:, 0:width], in0=acc[:, 0:width],
                                        in1=acc2[:, 0:width])
                                else:
                                    nc.vector.reduce_sum(
                                        out=acc[:, 0:width],
                                        in_=gt[:, b0_:b1_, 0:width].rearrange("p s f -> p f s"),
                                        axis=mybir.AxisListType.X,
                                    )
                            epi(bq, acc)
                    if chunk_hook is not None:
                        chunk_hook(prev_chunk)

                # ---- pass 4: gather dinv*x (64-wide), project via M4 ----
                st4 = make_stages("s")

                def epi4(bq, R):
                    rs = wp.tile([P, cfg.F], F32, tag="rs")
                    # rs = db*R + d2*x_own   (u such that st = db*(x@M3) + db*u@M4)
                    nc.scalar.activation(rs[:], R[:], AF.Copy, scale=db_sb[:, bq:bq + 1])
                    nc.vector.tensor_add(out=rs[:], in0=rs[:], in1=xd2_sb[:, bq, :])
                    pT = psp.tile([cfg.F, P], F32, tag="pT")
                    nc.tensor.transpose(pT[:], rs[:], ident[:])
                    rsT = wp.tile([cfg.F, P], F32, tag="rsT")
                    nc.vector.tensor_copy(rsT[:], pT[:])
                    ps = psp.tile([P, 16], F32, tag="ps")
                    nc.tensor.matmul(out=ps[:], lhsT=xT_sb[:, bq * P:(bq + 1) * P],
                                     rhs=mm_sb[:, 0:16], start=True, stop=False)
                    nc.tensor.matmul(out=ps[:], lhsT=rsT[:], rhs=m4_sb[:],
                                     start=False, stop=True)
                    nc.scalar.activation(st_slot(st4, bq), ps[:], AF.Copy,
                                         scale=db_sb[:, bq:bq + 1])

                run_pass(xraw, cfg.F, epi4,
                         chunk_hook=lambda c: exchange(0, c, st4[c], ttabs[0]))

                # ---- passes 3 and 2 ----
                def mk_epi(mcol, sts_prev, sts_new):
                    def epi(bq, R):
                        ps = psp.tile([P, 16], F32, tag="ps")
                        nc.tensor.matmul(out=ps[:],
                                         lhsT=xT_sb[:, bq * P:(bq + 1) * P],
                                         rhs=mm_sb[:, mcol:mcol + 16],
                                         start=True, stop=True)
                        # acc_full = R + prev_stage (self-loop)
                        accf = wp.tile([P, 16], F32, tag="accf")
                        nc.vector.tensor_add(out=accf[:], in0=R[:, 0:16],
                                             in1=st_slot(sts_prev, bq))
                        ta = wp.tile([P, 16], F32, tag="ta")
                        nc.scalar.activation(ta[:], ps[:], AF.Copy,
                                             scale=db_sb[:, bq:bq + 1])
                        tb = wp.tile([P, 16], F32, tag="tb")
                        nc.scalar.activation(tb[:], accf[:], AF.Copy,
                                             scale=d2_sb[:, bq:bq + 1])
                        nc.vector.tensor_add(out=st_slot(sts_new, bq),
                                             in0=ta[:], in1=tb[:])
                    return epi

                st3 = make_stages("s")
                run_pass(ttabs[0][:], 16, mk_epi(16, st4, st3),
                         chunk_hook=lambda c: exchange(1, c, st3[c], ttabs[1]))

                st2 = make_stages("s")
                run_pass(ttabs[1][:], 16, mk_epi(32, st3, st2),
                         chunk_hook=lambda c: exchange(2, c, st2[c], ttabs[0]))

                # ---- pass 1: final output ----
                st1 = make_stages("s")

                def epi1(bq, R):
                    accf = wp.tile([P, 16], F32, tag="accf")
                    nc.vector.tensor_add(out=accf[:], in0=R[:, 0:16],
                                         in1=st_slot(st2, bq))
                    t1 = wp.tile([P, 16], F32, tag="ta")
                    nc.scalar.activation(t1[:], accf[:], AF.Copy,
                                         scale=db_sb[:, bq:bq + 1])
                    nc.vector.tensor_add(out=st_slot(st1, bq), in0=t1[:],
                                         in1=bias_sb[:, bq, :])

                run_pass(ttabs[0][:], 16, epi1)
                for c in range(NCH):
                    lo, hi = int(chunk_cls0[c]), int(chunk_cls0[c + 1])
                    nc.sync.dma_start(out[:, lo:hi, :], st1[c][:])

    return nc


# --------------------------------------------------------------------------
# entry point
# --------------------------------------------------------------------------

def _run(inputs, cfg: Cfg, runner=None, **run_kwargs):
    global LAST_RESULTS
    in_maps, layout, pos_kbj, pos2old = _host_prep(inputs, cfg)
    nc = _build_module(cfg, layout)
    nc.compile()
    if runner is None:
        res = run_bass_kernel_spmd(nc, in_maps, core_ids=list(range(cfg.NCORES)),
                                   **run_kwargs)
        LAST_RESULTS = res
        outs = res.results
    else:
        outs = runner(nc, in_maps)
    full = np.empty((cfg.NPAD, 16), np.float32)
    for k in range(cfg.NCORES):
        o = np.asarray(outs[k]["out"])  # [P, NBLK, 16]
        full[pos_kbj[k].reshape(-1)] = o.transpose(1, 0, 2).reshape(cfg.PER, 16)
    old2new = np.empty(cfg.N, np.int64)
    rmask = pos2old >= 0
    old2new[pos2old[rmask]] = np.nonzero(rmask)[0]
    return full[old2new]


def kernel(**inputs) -> np.ndarray:
    return _run(inputs, CFG)


# revision 14
# speedup vs baseline: 1.4826x; 1.0495x over previous
"""Trainium2 Bass kernel: DGCNN forward (4-layer GCN + Conv1d readout) on 8 NeuronCores.

Math restructuring (validated vs reference to 2e-7):
  With A = D^-1/2 (Adj + I) D^-1/2 and Mk / ck derived from the (tiny) weights,
    out = A(x M1 + A(x M2 + A(x M3 + A(x M4)))) + 1 c0 + v1 c1 + v2 c2 + v3 c3
  Passes aggregate tables T; self-loop contributions are added in the epilogue
  from SBUF-resident data (previous pass's stage), so gathers cover only real
  edges.  Pass 4 gathers the 64-wide dinv*x table (host pre-scaled); its
  epilogue projects through M4.  Passes 3/2/1 gather 16-wide tables.

Device strategy (graph-parallel over 8 cores):
  - dma_gather (SWDGE) is descriptor-rate-bound (~8.1ns/desc per queue, 4
    queues scale linearly), so the kernel minimizes descriptors and keeps all
    4 queues fed:
    * nodes are placed into 128-row blocks clustered by (degree, #window-A
      sources, #window-B sources) signature, cutting ELL padding to ~5%
    * blocks are dealt into 8-wide "classes" (one block per core) so the SPMD
      module has uniform shapes; class slot budgets are cross-core maxes
    * gather tile pool is 6 deep and ~36 gather calls/pass rotate over the 4
      SWDGE queues so descriptor generation runs ~4-way concurrent
  - int16 gather indices limit a window to 32768 rows; the 50176-row table is
    covered by two overlapping windows ([0,32768) and [17408,50176)); each
    dst's edges are split between windows inside its class budgets SA/SB.
  - The table is laid out in 4 exchange chunks ([17,15,13,4] blocks/core,
    region-aligned) so each AllGather output is a contiguous table range; a
    single DRAM->DRAM DMA restrides [rows,16] into the 256B-row table.  The
    first 3 chunk exchanges overlap the current pass's remaining gathers; only
    the small 4-block tail exchange sits on the pass boundary.
"""

import dataclasses
import numpy as np

import concourse.bass as bass
import concourse.bacc as bacc
import concourse.tile as tile
from concourse import mybir
from concourse.bass_utils import run_bass_kernel_spmd
from concourse.masks import make_identity

F32 = mybir.dt.float32
I16 = mybir.dt.int16
AF = mybir.ActivationFunctionType


@dataclasses.dataclass(frozen=True)
class Cfg:
    N: int = 50000          # real nodes
    F: int = 64             # features
    NCORES: int = 8
    P: int = 128
    NBLK: int = 49          # blocks (classes) per core
    NQ: int = 4             # SWDGE queues
    GT_BUFS: int = 4        # gather tile pool depth
    GRP_TARGET: float = 2.5  # classes per gather group (approx)

    # exchange chunks: (region, blocks-per-core) in PROCESSING order; regions
    # are the int16 window areas R0=[0,17408) R1=[17408,32768) R2=[32768,50176).
    # The overlap region (1) is processed/exchanged first since both gather
    # windows need it; the 4-block tail is the only boundary-critical piece.
    CHUNKS = ((1, 15), (0, 17), (2, 13), (2, 4))
    LAG: int = 3            # A-call emission lead over B-call+reduce

    @property
    def PER(self):
        return self.NBLK * self.P

    @property
    def NPAD(self):
        return self.NCORES * self.PER

    @property
    def WA_LEN(self):
        return 32768

    @property
    def WB_OFF(self):
        return self.NPAD - 32768


CFG = Cfg()

LAST_RESULTS = None


# --------------------------------------------------------------------------
# host preprocessing
# --------------------------------------------------------------------------

def _host_prep(inputs, cfg: Cfg):
    x = np.asarray(inputs["x"], np.float32)
    ei = np.asarray(inputs["edge_index"]).astype(np.int64)
    W = [np.asarray(inputs[f"W{i}"], np.float64) for i in range(4)]
    b = [np.asarray(inputs[f"b{i}"], np.float64) for i in range(4)]
    conv_w = np.asarray(inputs["conv_w"], np.float64)
    conv_b = np.asarray(inputs["conv_b"], np.float64)

    n = x.shape[0]
    assert n == cfg.N and x.shape[1] == cfg.F
    P, PER, NPAD, NBLK, NC = cfg.P, cfg.PER, cfg.NPAD, cfg.NBLK, cfg.NCORES

    src0, dst0 = ei[0], ei[1]           # real edges only; self-loops in epilogue
    E0 = len(src0)
    deg = np.bincount(dst0, minlength=n).astype(np.float64) + 1.0  # incl self
    dinv = 1.0 / np.sqrt(deg)

    # ---- weight-derived small matrices ----
    Cw = [conv_w[:, 0:64], conv_w[:, 64:128], conv_w[:, 128:192], conv_w[:, 192:193]]
    M1 = W[0] @ Cw[0].T
    M2 = W[0] @ W[1] @ Cw[1].T
    M3 = W[0] @ W[1] @ W[2] @ Cw[2].T
    M4 = W[0] @ W[1] @ W[2] @ W[3] @ Cw[3].T
    c0 = b[0] @ Cw[0].T + b[1] @ Cw[1].T + b[2] @ Cw[2].T + b[3] @ Cw[3].T + conv_b
    c1 = (b[0] @ W[1]) @ Cw[1].T + (b[1] @ W[2]) @ Cw[2].T + (b[2] @ W[3]) @ Cw[3].T
    c2 = (b[0] @ W[1] @ W[2]) @ Cw[2].T + (b[1] @ W[2] @ W[3]) @ Cw[3].T
    c3 = (b[0] @ W[1] @ W[2] @ W[3]) @ Cw[3].T

    def aggv(v):
        o = np.zeros(n)
        np.add.at(o, dst0, (v * dinv)[src0])
        o += v * dinv
        return o * dinv

    v1 = aggv(np.ones(n))
    v2 = aggv(v1)
    v3 = aggv(v2)
    bias = (np.outer(np.ones(n), c0) + np.outer(v1, c1)
            + np.outer(v2, c2) + np.outer(v3, c3))  # [n, 16]

    # ---- region assignment + signatures ----
    # region position ranges (block aligned): R0 [0,17408) R1 [17408,32768)
    # R2 [32768,50176); pads: 2 at end of R0, 2 at end of R1, 172 end of R2
    RSTART = np.array([0, 17408, 32768, NPAD])
    RCAP = np.array([17408 - 2, 15360 - 2, 17408 - 172])
    assert RCAP.sum() == n
    region_of_node = np.repeat(np.arange(3), RCAP)  # node id order
    sreg = region_of_node[src0]
    gdeg = np.bincount(dst0, minlength=n).astype(np.int64)
    nA_n = np.bincount(dst0, weights=(sreg == 0), minlength=n).astype(np.int64)
    nB_n = np.bincount(dst0, weights=(sreg == 2), minlength=n).astype(np.int64)

    # chunk layout: per core, chunks of blocks; classes indexed 0..NBLK-1
    chunks = cfg.CHUNKS
    nb_of_chunk = [c[1] for c in chunks]
    assert sum(nb_of_chunk) == NBLK
    rcursor = {0: 0, 1: 17408, 2: 32768}
    chunk_start_pos = []
    for r, nb in chunks:
        chunk_start_pos.append(rcursor[r])
        rcursor[r] += NC * nb * P
    chunk_start_pos = np.array(chunk_start_pos + [NPAD])  # last entry unused
    chunk_cls0 = np.concatenate([[0], np.cumsum(nb_of_chunk)])

    # per region: sort real nodes by signature, form blocks, rank into classes
    pos_of_node = np.full(n, -1, np.int64)
    cls_cost = np.zeros(NBLK, np.int64)
    for r in range(3):
        nodes = np.nonzero(region_of_node == r)[0]
        k = np.lexsort((nB_n[nodes], nA_n[nodes], gdeg[nodes] // 4))
        nodes = nodes[k]
        nblocks_r = (RSTART[r + 1] - RSTART[r]) // P
        ncls_r = nblocks_r // NC
        # blocks of 128 consecutive sorted nodes (pads fill the tail)
        nfull = len(nodes)
        # block cost = max(gdeg, nA+nB) over its nodes (pads contribute 0)
        bcost = np.zeros(nblocks_r, np.int64)
        bi = np.arange(nfull) // P
        np.maximum.at(bcost, bi, np.maximum(gdeg[nodes], nA_n[nodes] + nB_n[nodes]))
        # class i = NC CONSECUTIVE blocks in signature order (keeps budgets
        # tight); rank classes by cost only to route small ones to the tail
        # chunk; per-core load is Sum(class budgets) regardless (1 blk/core).
        ccost = bcost.reshape(ncls_r, NC).max(axis=1)
        cls_rank = np.argsort(-ccost, kind="stable")  # region-class in cost order
        rchunks = [ci for ci, c in enumerate(chunks) if c[0] == r]
        cls_slots = []  # (chunk_idx, slot_in_chunk); tail chunk listed last
        for ci in rchunks:
            cls_slots += [(ci, s) for s in range(nb_of_chunk[ci])]
        assert len(cls_slots) == ncls_r
        for i in range(ncls_r):
            ci, slot = cls_slots[i]
            cls_id = chunk_cls0[ci] + slot
            rc = cls_rank[i]                      # region-class index
            for kc in range(NC):
                blk = rc * NC + kc
                base = (chunk_start_pos[ci] + kc * nb_of_chunk[ci] * P + slot * P)
                lo, hi = blk * P, min((blk + 1) * P, nfull)
                if hi > lo:
                    pos_of_node[nodes[lo:hi]] = base + np.arange(hi - lo)
                cls_cost[cls_id] = max(cls_cost[cls_id], bcost[blk])

    assert (pos_of_node >= 0).all()
    pos2old = np.full(NPAD, -1, np.int64)
    pos2old[pos_of_node] = np.arange(n)

    # position -> (core, local block b, j)
    pos = np.arange(NPAD)
    core_of_pos = np.zeros(NPAD, np.int64)
    cls_of_pos = np.zeros(NPAD, np.int64)
    for ci, (r, nb) in enumerate(chunks):
        s = chunk_start_pos[ci]
        e = s + NC * nb_of_chunk[ci] * P
        rel = pos[s:e] - s
        core_of_pos[s:e] = rel // (nb * P)
        cls_of_pos[s:e] = chunk_cls0[ci] + (rel % (nb * P)) // P
    j_of_pos = pos % P

    # dummy zero rows: a pad position in window A and one in window B
    pad_pos = np.nonzero(pos2old < 0)[0]
    zA = int(pad_pos[pad_pos < cfg.WA_LEN][-1])
    zB = int(pad_pos[pad_pos >= cfg.WB_OFF][-1])
    assert zA != zB

    # ---- per-edge window split with per-class budgets ----
    s_pos = pos_of_node[src0]
    d_pos = pos_of_node[dst0]
    eo = np.argsort(d_pos, kind="stable")
    s_s = s_pos[eo]
    d_s = d_pos[eo]
    starts = np.searchsorted(d_s, np.arange(NPAD + 1))

    isA = s_s < cfg.WB_OFF
    isB = s_s >= cfg.WA_LEN
    isF = ~(isA | isB)
    nAo = np.bincount(d_s, weights=isA, minlength=NPAD).astype(np.int64)
    nBo = np.bincount(d_s, weights=isB, minlength=NPAD).astype(np.int64)
    nf = np.bincount(d_s, weights=isF, minlength=NPAD).astype(np.int64)
    tot = nAo + nBo + nf

    cp = cls_of_pos
    mA = np.zeros(NBLK, np.int64); np.maximum.at(mA, cp, nAo)
    mB = np.zeros(NBLK, np.int64); np.maximum.at(mB, cp, nBo)
    mT = np.zeros(NBLK, np.int64); np.maximum.at(mT, cp, tot)
    M = np.maximum(mT, mA + mB)
    SA = np.clip((M + 1) // 2, mA, M - mB)
    SB = M - SA
    SAp = SA[cp]
    SBp = SB[cp]
    nA = np.clip(tot - SBp, nAo, np.minimum(nAo + nf, SAp))

    cFex = np.concatenate([[0], np.cumsum(isF)])
    frank = cFex[:-1] - cFex[starts[d_s]]
    goA = isA | (isF & (frank < (nA - nAo)[d_s]))
    goB = ~goA
    cAex = np.concatenate([[0], np.cumsum(goA)])
    slotA = cAex[:-1] - cAex[starts[d_s]]
    cBex = np.concatenate([[0], np.cumsum(goB)])
    slotB = cBex[:-1] - cBex[starts[d_s]]
    nB_ = tot - nA
    assert (nA <= SAp).all() and (nB_ <= SBp).all()
    assert (SA + SB > 0).all()

    # ---- groups: classes within each chunk, balanced by slots ----
    groups = []          # list of list of class ids
    grp_chunk = []       # chunk index of each group
    for ci, (r, nb) in enumerate(chunks):
        cls_list = list(range(chunk_cls0[ci], chunk_cls0[ci] + nb))
        ng = int(np.ceil(nb / cfg.GRP_TARGET))
        # greedy balance by SA+SB
        order_d = sorted(cls_list, key=lambda c: -(SA[c] + SB[c]))
        gsets = [[] for _ in range(ng)]
        gsum = [0] * ng
        for c in order_d:
            q = min(range(ng), key=lambda i: (gsum[i], i))
            gsets[q].append(c)
            gsum[q] += SA[c] + SB[c]
        for g in gsets:
            groups.append(sorted(g))
            grp_chunk.append(ci)
    NGRP = len(groups)

    # slot offsets per class within its group's A/B gathers
    oa = np.zeros(NBLK, np.int64)
    ob = np.zeros(NBLK, np.int64)
    grp_of = np.zeros(NBLK, np.int64)
    SAg = np.zeros(NGRP, np.int64)
    SBg = np.zeros(NGRP, np.int64)
    for q, bl in enumerate(groups):
        offa = 0
        for bq in bl:
            oa[bq] = offa
            offa += SA[bq]
            grp_of[bq] = q
        offb = 0
        for bq in bl:
            ob[bq] = offb
            offb += SB[bq]
        SAg[q] = offa
        SBg[q] = offb

    colA0 = np.zeros(NGRP, np.int64)
    colB0 = np.zeros(NGRP, np.int64)
    cur = 0
    for q in range(NGRP):
        colA0[q] = cur
        cur += int(SAg[q]) * P // 16
        colB0[q] = cur
        cur += int(SBg[q]) * P // 16
    idxcols = int(cur)

    # ---- build per-core idx tensors ----
    zA_rel = np.int16(zA)
    zB_rel = np.int16(zB - cfg.WB_OFF)
    idx_np = np.empty((NC, 128, idxcols), np.int16)
    for q in range(NGRP):
        idx_np[:, :, colA0[q]:colA0[q] + int(SAg[q]) * P // 16] = zA_rel
        idx_np[:, :, colB0[q]:colB0[q] + int(SBg[q]) * P // 16] = zB_rel

    e_core = core_of_pos[d_s]
    e_cls = cls_of_pos[d_s]
    e_j = j_of_pos[d_s]
    e_q = grp_of[e_cls]
    posA = (oa[e_cls] + slotA) * P + e_j
    colA = colA0[e_q] + posA // 16
    rowA = posA % 16
    posB = (ob[e_cls] + slotB) * P + e_j
    colB = colB0[e_q] + posB // 16
    rowB = posB % 16
    valA = s_s.astype(np.int16)
    valB = (s_s - cfg.WB_OFF).astype(np.int16)
    for k in range(NC):
        mk = e_core == k
        mAk = mk & goA
        mBk = mk & goB
        for r in range(8):
            idx_np[k, rowA[mAk] + 16 * r, colA[mAk]] = valA[mAk]
            idx_np[k, rowB[mBk] + 16 * r, colB[mBk]] = valB[mBk]

    # ---- dense per-core arrays ----
    rmask = pos2old >= 0
    dinv_pos = np.ones(NPAD, np.float32)
    dinv_pos[rmask] = dinv[pos2old[rmask]].astype(np.float32)
    x_pos = np.zeros((NPAD, cfg.F), np.float32)
    x_pos[rmask] = x[pos2old[rmask]]
    bias_pos = np.zeros((NPAD, 16), np.float32)
    bias_pos[rmask] = bias[pos2old[rmask]].astype(np.float32)

    xraw_t = x_pos * dinv_pos[:, None]            # pass-4 table: dinv*x
    # per-core [j, b] layouts: position of (core k, cls b, j)
    pos_kbj = np.zeros((NC, NBLK, P), np.int64)
    for ci, (r, nb) in enumerate(chunks):
        for kc in range(NC):
            for s in range(nb):
                base = chunk_start_pos[ci] + kc * nb * P + s * P
                pos_kbj[kc, chunk_cls0[ci] + s] = base + np.arange(P)

    in_maps = []
    mmats = np.ascontiguousarray(np.concatenate([M3, M2, M1], axis=1).astype(np.float32))
    m4 = np.ascontiguousarray(M4.astype(np.float32))
    for k in range(NC):
        pk = pos_kbj[k]                            # [NBLK, P] positions
        db = dinv_pos[pk].T.astype(np.float32)     # [P, NBLK]
        d2 = (db * db).astype(np.float32)
        xTloc = np.ascontiguousarray(
            x_pos[pk.reshape(-1)].T)               # [F, PER] raw x
        xd2 = np.ascontiguousarray(x_pos[pk].transpose(1, 0, 2)
                                   * (dinv_pos[pk] ** 2).T[:, :, None]).astype(np.float32)
        bias_blk = np.ascontiguousarray(bias_pos[pk].transpose(1, 0, 2)).astype(np.float32)
        in_maps.append(dict(
            xraw=xraw_t,
            idx=np.ascontiguousarray(idx_np[k]),
            xT=xTloc,
            db=np.ascontiguousarray(db),
            d2=np.ascontiguousarray(d2),
            xd2=xd2,
            bias_blk=bias_blk,
            mmats=mmats,
            m4=m4,
        ))

    layout = dict(SA=SA, SB=SB, groups=groups, grp_chunk=grp_chunk, oa=oa, ob=ob,
                  SAg=SAg, SBg=SBg, colA0=colA0, colB0=colB0, idxcols=idxcols,
                  chunks=chunks, chunk_start_pos=chunk_start_pos,
                  chunk_cls0=chunk_cls0)
    return in_maps, layout, pos_kbj, pos2old


# --------------------------------------------------------------------------
# device module
# --------------------------------------------------------------------------

def _build_module(cfg: Cfg, layout):
    P, PER, NPAD, NBLK, NC = cfg.P, cfg.PER, cfg.NPAD, cfg.NBLK, cfg.NCORES
    SA, SB = layout["SA"], layout["SB"]
    groups, grp_chunk = layout["groups"], layout["grp_chunk"]
    oa, ob = layout["oa"], layout["ob"]
    SAg, SBg = layout["SAg"], layout["SBg"]
    colA0, colB0 = layout["colA0"], layout["colB0"]
    idxcols = layout["idxcols"]
    chunks = layout["chunks"]
    chunk_start_pos = layout["chunk_start_pos"]
    chunk_cls0 = layout["chunk_cls0"]
    NGRP = len(groups)
    NCH = len(chunks)

    nc = bacc.Bacc("TRN2", target_bir_lowering=False, debug=False, num_devices=NC,
                   num_swdge_queues=cfg.NQ, dynamic_dma_scratch_size=24576)

    xraw = nc.dram_tensor("xraw", [NPAD, cfg.F], F32, kind="ExternalInput").ap()
    idx = nc.dram_tensor("idx", [128, idxcols], I16, kind="ExternalInput").ap()
    xT = nc.dram_tensor("xT", [cfg.F, PER], F32, kind="ExternalInput").ap()
    db_in = nc.dram_tensor("db", [P, NBLK], F32, kind="ExternalInput").ap()
    d2_in = nc.dram_tensor("d2", [P, NBLK], F32, kind="ExternalInput").ap()
    xd2_in = nc.dram_tensor("xd2", [P, NBLK, cfg.F], F32, kind="ExternalInput").ap()
    bias_in = nc.dram_tensor("bias_blk", [P, NBLK, 16], F32, kind="ExternalInput").ap()
    mmats = nc.dram_tensor("mmats", [cfg.F, 48], F32, kind="ExternalInput").ap()
    m4 = nc.dram_tensor("m4", [cfg.F, 16], F32, kind="ExternalInput").ap()
    out = nc.dram_tensor("out", [P, NBLK, 16], F32, kind="ExternalOutput").ap()

    with tile.TileContext(nc) as tc:
        with (
            tc.tile_pool(name="const", bufs=1) as cp,
            tc.tile_pool(name="dram", bufs=1, space="DRAM") as dp,
        ):
            idx_sb = cp.tile([128, idxcols], I16)
            nc.sync.dma_start(idx_sb[:], idx)
            xT_sb = cp.tile([cfg.F, PER], F32)
            nc.sync.dma_start(xT_sb[:], xT)
            mm_sb = cp.tile([cfg.F, 48], F32)
            nc.sync.dma_start(mm_sb[:], mmats)
            m4_sb = cp.tile([cfg.F, 16], F32)
            nc.sync.dma_start(m4_sb[:], m4)
            db_sb = cp.tile([P, NBLK], F32)
            nc.sync.dma_start(db_sb[:], db_in)
            d2_sb = cp.tile([P, NBLK], F32)
            nc.sync.dma_start(d2_sb[:], d2_in)
            xd2_sb = cp.tile([P, NBLK, cfg.F], F32)
            nc.sync.dma_start(xd2_sb[:], xd2_in)
            bias_sb = cp.tile([P, NBLK, 16], F32)
            nc.sync.dma_start(bias_sb[:], bias_in)
            ident = cp.tile([P, P], F32)
            make_identity(nc, ident[:])

            # each generation is a (w1, w2) pair: w1 = table rows [0,32768),
            # w2 = rows [17408, 50176); chunk exchanges write into one or both
            tw = [(dp.tile([cfg.WA_LEN, cfg.F], F32, name=f"tw{i}_1"),
                   dp.tile([cfg.WA_LEN, cfg.F], F32, name=f"tw{i}_2"))
                  for i in range(2)]
            ccin = [dp.tile([nb * P, 16], F32, name=f"ccin{c}")
                    for c, (r, nb) in enumerate(chunks)]
            ccout = [[dp.tile([NC * nb * P, 16], F32, addr_space="Shared",
                              name=f"ccout{p}_{c}")
                      for c, (r, nb) in enumerate(chunks)] for p in range(3)]

            with (
                tc.tile_pool(name="gatha", bufs=cfg.LAG + 3) as gpa,
                tc.tile_pool(name="gathb", bufs=4) as gpb,
                tc.tile_pool(name="work", bufs=4) as wp,
                tc.tile_pool(name="stage", bufs=2) as sp,
                tc.tile_pool(name="psum", bufs=4, space="PSUM") as psp,
            ):
                # greedy queue schedule: call (in EMISSION order) -> least-loaded
                emit_order = []
                for step in range(NGRP + cfg.LAG):
                    if step < NGRP:
                        emit_order.append((step, "A", int(SAg[step])))
                    h = step - cfg.LAG
                    if h >= 0:
                        emit_order.append((h, "B", int(SBg[h])))
                qload = [0] * cfg.NQ
                qsched = {}
                for q, part, sz in emit_order:
                    if sz:
                        qq = min(range(cfg.NQ), key=lambda i: (qload[i], i))
                        qload[qq] += sz * P
                        qsched[(q, part)] = qq

                def make_stages(tag):
                    return [sp.tile([P, nb, 16], F32, tag=f"{tag}{c}",
                                    name=f"st_{tag}{c}")
                            for c, (r, nb) in enumerate(chunks)]

                def st_slot(sts, bq):
                    for c in range(NCH):
                        if bq < chunk_cls0[c + 1]:
                            return sts[c][:, bq - chunk_cls0[c], :]
                    raise AssertionError

                def exchange(p, c, st_tile, target):
                    r, nb = chunks[c]
                    w1, w2 = target
                    nc.sync.dma_start(
                        ccin[c][:].rearrange("(b p) f -> p b f", p=P), st_tile[:])
                    nc.gpsimd.collective_compute(
                        "AllGather", mybir.AluOpType.bypass,
                        replica_groups=[list(range(NC))],
                        ins=[ccin[c][:]], outs=[ccout[p][c][:]],
                    )
                    s = int(chunk_start_pos[c])
                    rows = NC * nb * P
                    if s < cfg.WA_LEN:                    # overlaps window 1
                        hi = min(s + rows, cfg.WA_LEN)
                        nc.sync.dma_start(
                            w1[s:hi, 0:16], ccout[p][c][0:hi - s, :])
                    if s + rows > cfg.WB_OFF:             # overlaps window 2
                        lo = max(s, cfg.WB_OFF)
                        nc.sync.dma_start(
                            w2[lo - cfg.WB_OFF:s + rows - cfg.WB_OFF, 0:16],
                            ccout[p][c][lo - s:rows, :])

                last_of_chunk = {}
                for q in range(NGRP):
                    last_of_chunk[grp_chunk[q]] = q

                def run_pass(winA, winB, width, epi, chunk_hook=None):
                    # A-gathers issue LAG groups ahead of B-gathers+reduces so
                    # queue FIFOs stay busy across the pass boundary (A only
                    # depends on the first two chunk exchanges of the prior
                    # pass, B on all four).
                    gtA = {}
                    gtB = {}

                    def emit_A(q):
                        sag = int(SAg[q])
                        if not sag:
                            return
                        t = gpa.tile([P, sag, cfg.F], F32, tag="gtA", name="gtA")
                        gtA[q] = t
                        nc.gpsimd.dma_gather(
                            out_ap=t[:],
                            in_ap=winA,
                            idxs_ap=idx_sb[:, int(colA0[q]):int(colA0[q]) + sag * P // 16],
                            num_idxs=sag * P,
                            num_idxs_reg=sag * P,
                            elem_size=cfg.F,
                            single_packet=False,
                            queue_num=qsched[(q, "A")],
                        )

                    def emit_B(q):
                        sbg = int(SBg[q])
                        if not sbg:
                            return
                        t = gpb.tile([P, sbg, cfg.F], F32, tag="gtB", name="gtB")
                        gtB[q] = t
                        nc.gpsimd.dma_gather(
                            out_ap=t[:],
                            in_ap=winB,
                            idxs_ap=idx_sb[:, int(colB0[q]):int(colB0[q]) + sbg * P // 16],
                            num_idxs=sbg * P,
                            num_idxs_reg=sbg * P,
                            elem_size=cfg.F,
                            single_packet=False,
                            queue_num=qsched[(q, "B")],
                        )

                    def emit_reduces(q):
                        for bq in groups[q]:
                            acc = wp.tile([P, cfg.F], F32, tag="acc")
                            wrote = False
                            if SA[bq]:
                                a0, a1 = int(oa[bq]), int(oa[bq] + SA[bq])
                                nc.vector.reduce_sum(
                                    out=acc[:, 0:width],
                                    in_=gtA[q][:, a0:a1, 0:width].rearrange("p s f -> p f s"),
                                    axis=mybir.AxisListType.X,
                                )
                                wrote = True
                            if SB[bq]:
                                b0_, b1_ = int(ob[bq]), int(ob[bq] + SB[bq])
                                if wrote:
                                    acc2 = wp.tile([P, cfg.F], F32, tag="acc2")
                                    nc.vector.reduce_sum(
                                        out=acc2[:, 0:width],
                                        in_=gtB[q][:, b0_:b1_, 0:width].rearrange("p s f -> p f s"),
                                        axis=mybir.AxisListType.X,
                                    )
                                    nc.vector.tensor_add(
                                        out=acc[:, 0:width], in0=acc[:, 0:width],
                                        in1=acc2[:, 0:width])
                                else:
                                    nc.vector.reduce_sum(
                                        out=acc[:, 0:width],
                                        in_=gtB[q][:, b0_:b1_, 0:width].rearrange("p s f -> p f s"),
                                        axis=mybir.AxisListType.X,
                                    )
                            epi(bq, acc)

                    for step in range(NGRP + cfg.LAG):
                        if step < NGRP:
                            emit_A(step)
                        h = step - cfg.LAG
                        if h >= 0:
                            emit_B(h)
                            emit_reduces(h)
                            if chunk_hook is not None and h == last_of_chunk[grp_chunk[h]]:
                                chunk_hook(grp_chunk[h])

                # ---- pass 4: gather dinv*x (64-wide), project via M4 ----
                st4 = make_stages("s")

                def epi4(bq, R):
                    rs = wp.tile([P, cfg.F], F32, tag="rs")
                    # rs = db*R + d2*x_own   (u such that st = db*(x@M3) + db*u@M4)
                    nc.scalar.activation(rs[:], R[:], AF.Copy, scale=db_sb[:, bq:bq + 1])
                    nc.vector.tensor_add(out=rs[:], in0=rs[:], in1=xd2_sb[:, bq, :])
                    pT = psp.tile([cfg.F, P], F32, tag="pT")
                    nc.tensor.transpose(pT[:], rs[:], ident[:])
                    rsT = wp.tile([cfg.F, P], F32, tag="rsT")
                    nc.vector.tensor_copy(rsT[:], pT[:])
                    ps = psp.tile([P, 16], F32, tag="ps")
                    nc.tensor.matmul(out=ps[:], lhsT=xT_sb[:, bq * P:(bq + 1) * P],
                                     rhs=mm_sb[:, 0:16], start=True, stop=False)
                    nc.tensor.matmul(out=ps[:], lhsT=rsT[:], rhs=m4_sb[:],
                                     start=False, stop=True)
                    nc.scalar.activation(st_slot(st4, bq), ps[:], AF.Copy,
                                         scale=db_sb[:, bq:bq + 1])

                run_pass(xraw[0:cfg.WA_LEN, :], xraw[cfg.WB_OFF:NPAD, :], cfg.F,
                         epi4, chunk_hook=lambda c: exchange(0, c, st4[c], tw[0]))

                # ---- passes 3 and 2 ----
                def mk_epi(mcol, sts_prev, sts_new):
                    def epi(bq, R):
                        ps = psp.tile([P, 16], F32, tag="ps")
                        nc.tensor.matmul(out=ps[:],
                                         lhsT=xT_sb[:, bq * P:(bq + 1) * P],
                                         rhs=mm_sb[:, mcol:mcol + 16],
                                         start=True, stop=True)
                        # acc_full = R + prev_stage (self-loop)
                        accf = wp.tile([P, 16], F32, tag="accf")
                        nc.vector.tensor_add(out=accf[:], in0=R[:, 0:16],
                                             in1=st_slot(sts_prev, bq))
                        ta = wp.tile([P, 16], F32, tag="ta")
                        nc.scalar.activation(ta[:], ps[:], AF.Copy,
                                             scale=db_sb[:, bq:bq + 1])
                        tb = wp.tile([P, 16], F32, tag="tb")
                        nc.scalar.activation(tb[:], accf[:], AF.Copy,
                                             scale=d2_sb[:, bq:bq + 1])
                        nc.vector.tensor_add(out=st_slot(sts_new, bq),
                                             in0=ta[:], in1=tb[:])
                    return epi

                st3 = make_stages("s")
                run_pass(tw[0][0][:], tw[0][1][:], 16, mk_epi(16, st4, st3),
                         chunk_hook=lambda c: exchange(1, c, st3[c], tw[1]))

                st2 = make_stages("s")
                run_pass(tw[1][0][:], tw[1][1][:], 16, mk_epi(32, st3, st2),
                         chunk_hook=lambda c: exchange(2, c, st2[c], tw[0]))

                # ---- pass 1: final output ----
                st1 = make_stages("s")

                def epi1(bq, R):
                    accf = wp.tile([P, 16], F32, tag="accf")
                    nc.vector.tensor_add(out=accf[:], in0=R[:, 0:16],
                                         in1=st_slot(st2, bq))
                    t1 = wp.tile([P, 16], F32, tag="ta")
                    nc.scalar.activation(t1[:], accf[:], AF.Copy,
                                         scale=db_sb[:, bq:bq + 1])
                    nc.vector.tensor_add(out=st_slot(st1, bq), in0=t1[:],
                                         in1=bias_sb[:, bq, :])

                run_pass(tw[0][0][:], tw[0][1][:], 16, epi1)
                for c in range(NCH):
                    lo, hi = int(chunk_cls0[c]), int(chunk_cls0[c + 1])
                    nc.sync.dma_start(out[:, lo:hi, :], st1[c][:])

    return nc


# --------------------------------------------------------------------------
# entry point
# --------------------------------------------------------------------------

def _run(inputs, cfg: Cfg, runner=None, **run_kwargs):
    global LAST_RESULTS
    in_maps, layout, pos_kbj, pos2old = _host_prep(inputs, cfg)
    nc = _build_module(cfg, layout)
    nc.compile()
    if runner is None:
        res = run_bass_kernel_spmd(nc, in_maps, core_ids=list(range(cfg.NCORES)),
                                   **run_kwargs)
        LAST_RESULTS = res
        outs = res.results
    else:
        outs = runner(nc, in_maps)
    full = np.empty((cfg.NPAD, 16), np.float32)
    for k in range(cfg.NCORES):
        o = np.asarray(outs[k]["out"])  # [P, NBLK, 16]
        full[pos_kbj[k].reshape(-1)] = o.transpose(1, 0, 2).reshape(cfg.PER, 16)
    old2new = np.empty(cfg.N, np.int64)
    rmask = pos2old >= 0
    old2new[pos2old[rmask]] = np.nonzero(rmask)[0]
    return full[old2new]


def kernel(**inputs) -> np.ndarray:
    return _run(inputs, CFG)


# revision 15
# speedup vs baseline: 1.5316x; 1.0331x over previous
"""Trainium2 Bass kernel: DGCNN forward (4-layer GCN + Conv1d readout) on 8 NeuronCores.

Math restructuring (validated vs reference to 2e-7):
  With A = D^-1/2 (Adj + I) D^-1/2 and Mk / ck derived from the (tiny) weights,
    out = A(x M1 + A(x M2 + A(x M3 + A(x M4)))) + 1 c0 + v1 c1 + v2 c2 + v3 c3
  Passes aggregate tables T; self-loop contributions are added in the epilogue
  from SBUF-resident data (previous pass's stage), so gathers cover only real
  edges.  Pass 4 gathers the 64-wide dinv*x table (host pre-scaled); its
  epilogue projects through M4.  Passes 3/2/1 gather 16-wide tables.

Device strategy (graph-parallel over 8 cores):
  - dma_gather (SWDGE) is descriptor-rate-bound (~8.1ns/desc per queue, 4
    queues scale linearly), so the kernel minimizes descriptors and keeps all
    4 queues fed:
    * nodes are placed into 128-row blocks clustered by (degree, #window-A
      sources, #window-B sources) signature, cutting ELL padding to ~5%
    * blocks are dealt into 8-wide "classes" (one block per core) so the SPMD
      module has uniform shapes; class slot budgets are cross-core maxes
    * gather tile pool is 6 deep and ~36 gather calls/pass rotate over the 4
      SWDGE queues so descriptor generation runs ~4-way concurrent
  - int16 gather indices limit a window to 32768 rows; the 50176-row table is
    covered by two overlapping windows ([0,32768) and [17408,50176)); each
    dst's edges are split between windows inside its class budgets SA/SB.
  - The table is laid out in 4 exchange chunks ([17,15,13,4] blocks/core,
    region-aligned) so each AllGather output is a contiguous table range; a
    single DRAM->DRAM DMA restrides [rows,16] into the 256B-row table.  The
    first 3 chunk exchanges overlap the current pass's remaining gathers; only
    the small 4-block tail exchange sits on the pass boundary.
"""

import dataclasses
import numpy as np

import concourse.bass as bass
import concourse.bacc as bacc
import concourse.tile as tile
from concourse import mybir
from concourse.bass_utils import run_bass_kernel_spmd
from concourse.masks import make_identity

F32 = mybir.dt.float32
I16 = mybir.dt.int16
AF = mybir.ActivationFunctionType


@dataclasses.dataclass(frozen=True)
class Cfg:
    N: int = 50000          # real nodes
    F: int = 64             # features
    NCORES: int = 8
    P: int = 128
    NBLK: int = 49          # blocks (classes) per core
    NQ: int = 4             # SWDGE queues
    GT_BUFS: int = 4        # gather tile pool depth
    GRP_TARGET: float = 2.5  # classes per gather group (approx)

    # exchange chunks: (region, blocks-per-core) in PROCESSING order; regions
    # are the int16 window areas R0=[0,17408) R1=[17408,32768) R2=[32768,50176).
    # The overlap region (1) is processed/exchanged first since both gather
    # windows need it; the 4-block tail is the only boundary-critical piece.
    CHUNKS = ((1, 15), (0, 17), (2, 13), (2, 4))
    LAG: int = 4            # A-call emission lead over B-call+reduce
    CC_DELAY: int = 2       # groups between ccin DMA and collective trigger

    @property
    def PER(self):
        return self.NBLK * self.P

    @property
    def NPAD(self):
        return self.NCORES * self.PER

    @property
    def WA_LEN(self):
        return 32768

    @property
    def WB_OFF(self):
        return self.NPAD - 32768


CFG = Cfg()

LAST_RESULTS = None


# --------------------------------------------------------------------------
# host preprocessing
# --------------------------------------------------------------------------

def _host_prep(inputs, cfg: Cfg):
    x = np.asarray(inputs["x"], np.float32)
    ei = np.asarray(inputs["edge_index"]).astype(np.int64)
    W = [np.asarray(inputs[f"W{i}"], np.float64) for i in range(4)]
    b = [np.asarray(inputs[f"b{i}"], np.float64) for i in range(4)]
    conv_w = np.asarray(inputs["conv_w"], np.float64)
    conv_b = np.asarray(inputs["conv_b"], np.float64)

    n = x.shape[0]
    assert n == cfg.N and x.shape[1] == cfg.F
    P, PER, NPAD, NBLK, NC = cfg.P, cfg.PER, cfg.NPAD, cfg.NBLK, cfg.NCORES

    src0, dst0 = ei[0], ei[1]           # real edges only; self-loops in epilogue
    E0 = len(src0)
    deg = np.bincount(dst0, minlength=n).astype(np.float64) + 1.0  # incl self
    dinv = 1.0 / np.sqrt(deg)

    # ---- weight-derived small matrices ----
    Cw = [conv_w[:, 0:64], conv_w[:, 64:128], conv_w[:, 128:192], conv_w[:, 192:193]]
    M1 = W[0] @ Cw[0].T
    M2 = W[0] @ W[1] @ Cw[1].T
    M3 = W[0] @ W[1] @ W[2] @ Cw[2].T
    M4 = W[0] @ W[1] @ W[2] @ W[3] @ Cw[3].T
    c0 = b[0] @ Cw[0].T + b[1] @ Cw[1].T + b[2] @ Cw[2].T + b[3] @ Cw[3].T + conv_b
    c1 = (b[0] @ W[1]) @ Cw[1].T + (b[1] @ W[2]) @ Cw[2].T + (b[2] @ W[3]) @ Cw[3].T
    c2 = (b[0] @ W[1] @ W[2]) @ Cw[2].T + (b[1] @ W[2] @ W[3]) @ Cw[3].T
    c3 = (b[0] @ W[1] @ W[2] @ W[3]) @ Cw[3].T

    def aggv(v):
        o = np.zeros(n)
        np.add.at(o, dst0, (v * dinv)[src0])
        o += v * dinv
        return o * dinv

    v1 = aggv(np.ones(n))
    v2 = aggv(v1)
    v3 = aggv(v2)
    bias = (np.outer(np.ones(n), c0) + np.outer(v1, c1)
            + np.outer(v2, c2) + np.outer(v3, c3))  # [n, 16]

    # ---- region assignment + signatures ----
    # region position ranges (block aligned): R0 [0,17408) R1 [17408,32768)
    # R2 [32768,50176); pads: 2 at end of R0, 2 at end of R1, 172 end of R2
    RSTART = np.array([0, 17408, 32768, NPAD])
    RCAP = np.array([17408 - 2, 15360 - 2, 17408 - 172])
    assert RCAP.sum() == n
    region_of_node = np.repeat(np.arange(3), RCAP)  # node id order
    sreg = region_of_node[src0]
    gdeg = np.bincount(dst0, minlength=n).astype(np.int64)
    nA_n = np.bincount(dst0, weights=(sreg == 0), minlength=n).astype(np.int64)
    nB_n = np.bincount(dst0, weights=(sreg == 2), minlength=n).astype(np.int64)

    # chunk layout: per core, chunks of blocks; classes indexed 0..NBLK-1
    chunks = cfg.CHUNKS
    nb_of_chunk = [c[1] for c in chunks]
    assert sum(nb_of_chunk) == NBLK
    rcursor = {0: 0, 1: 17408, 2: 32768}
    chunk_start_pos = []
    for r, nb in chunks:
        chunk_start_pos.append(rcursor[r])
        rcursor[r] += NC * nb * P
    chunk_start_pos = np.array(chunk_start_pos + [NPAD])  # last entry unused
    chunk_cls0 = np.concatenate([[0], np.cumsum(nb_of_chunk)])

    # per region: sort real nodes by signature, form blocks, rank into classes
    pos_of_node = np.full(n, -1, np.int64)
    cls_cost = np.zeros(NBLK, np.int64)
    for r in range(3):
        nodes = np.nonzero(region_of_node == r)[0]
        k = np.lexsort((nB_n[nodes], nA_n[nodes], gdeg[nodes] // 4))
        nodes = nodes[k]
        nblocks_r = (RSTART[r + 1] - RSTART[r]) // P
        ncls_r = nblocks_r // NC
        # blocks of 128 consecutive sorted nodes (pads fill the tail)
        nfull = len(nodes)
        # block cost = max(gdeg, nA+nB) over its nodes (pads contribute 0)
        bcost = np.zeros(nblocks_r, np.int64)
        bi = np.arange(nfull) // P
        np.maximum.at(bcost, bi, np.maximum(gdeg[nodes], nA_n[nodes] + nB_n[nodes]))
        # class i = NC CONSECUTIVE blocks in signature order (keeps budgets
        # tight); rank classes by cost only to route small ones to the tail
        # chunk; per-core load is Sum(class budgets) regardless (1 blk/core).
        ccost = bcost.reshape(ncls_r, NC).max(axis=1)
        cls_rank = np.argsort(-ccost, kind="stable")  # region-class in cost order
        rchunks = [ci for ci, c in enumerate(chunks) if c[0] == r]
        cls_slots = []  # (chunk_idx, slot_in_chunk); tail chunk listed last
        for ci in rchunks:
            cls_slots += [(ci, s) for s in range(nb_of_chunk[ci])]
        assert len(cls_slots) == ncls_r
        for i in range(ncls_r):
            ci, slot = cls_slots[i]
            cls_id = chunk_cls0[ci] + slot
            rc = cls_rank[i]                      # region-class index
            for kc in range(NC):
                blk = rc * NC + kc
                base = (chunk_start_pos[ci] + kc * nb_of_chunk[ci] * P + slot * P)
                lo, hi = blk * P, min((blk + 1) * P, nfull)
                if hi > lo:
                    pos_of_node[nodes[lo:hi]] = base + np.arange(hi - lo)
                cls_cost[cls_id] = max(cls_cost[cls_id], bcost[blk])

    assert (pos_of_node >= 0).all()
    pos2old = np.full(NPAD, -1, np.int64)
    pos2old[pos_of_node] = np.arange(n)

    # position -> (core, local block b, j)
    pos = np.arange(NPAD)
    core_of_pos = np.zeros(NPAD, np.int64)
    cls_of_pos = np.zeros(NPAD, np.int64)
    for ci, (r, nb) in enumerate(chunks):
        s = chunk_start_pos[ci]
        e = s + NC * nb_of_chunk[ci] * P
        rel = pos[s:e] - s
        core_of_pos[s:e] = rel // (nb * P)
        cls_of_pos[s:e] = chunk_cls0[ci] + (rel % (nb * P)) // P
    j_of_pos = pos % P

    # dummy zero rows: a pad position in window A and one in window B
    pad_pos = np.nonzero(pos2old < 0)[0]
    zA = int(pad_pos[pad_pos < cfg.WA_LEN][-1])
    zB = int(pad_pos[pad_pos >= cfg.WB_OFF][-1])
    assert zA != zB

    # ---- per-edge window split with per-class budgets ----
    s_pos = pos_of_node[src0]
    d_pos = pos_of_node[dst0]
    eo = np.argsort(d_pos, kind="stable")
    s_s = s_pos[eo]
    d_s = d_pos[eo]
    starts = np.searchsorted(d_s, np.arange(NPAD + 1))

    isA = s_s < cfg.WB_OFF
    isB = s_s >= cfg.WA_LEN
    isF = ~(isA | isB)
    nAo = np.bincount(d_s, weights=isA, minlength=NPAD).astype(np.int64)
    nBo = np.bincount(d_s, weights=isB, minlength=NPAD).astype(np.int64)
    nf = np.bincount(d_s, weights=isF, minlength=NPAD).astype(np.int64)
    tot = nAo + nBo + nf

    cp = cls_of_pos
    mA = np.zeros(NBLK, np.int64); np.maximum.at(mA, cp, nAo)
    mB = np.zeros(NBLK, np.int64); np.maximum.at(mB, cp, nBo)
    mT = np.zeros(NBLK, np.int64); np.maximum.at(mT, cp, tot)
    M = np.maximum(mT, mA + mB)
    SA = np.clip((M + 1) // 2, mA, M - mB)
    SB = M - SA
    SAp = SA[cp]
    SBp = SB[cp]
    nA = np.clip(tot - SBp, nAo, np.minimum(nAo + nf, SAp))

    cFex = np.concatenate([[0], np.cumsum(isF)])
    frank = cFex[:-1] - cFex[starts[d_s]]
    goA = isA | (isF & (frank < (nA - nAo)[d_s]))
    goB = ~goA
    cAex = np.concatenate([[0], np.cumsum(goA)])
    slotA = cAex[:-1] - cAex[starts[d_s]]
    cBex = np.concatenate([[0], np.cumsum(goB)])
    slotB = cBex[:-1] - cBex[starts[d_s]]
    nB_ = tot - nA
    assert (nA <= SAp).all() and (nB_ <= SBp).all()
    assert (SA + SB > 0).all()

    # ---- groups: classes within each chunk, balanced by slots ----
    groups = []          # list of list of class ids
    grp_chunk = []       # chunk index of each group
    for ci, (r, nb) in enumerate(chunks):
        cls_list = list(range(chunk_cls0[ci], chunk_cls0[ci] + nb))
        ng = int(np.ceil(nb / cfg.GRP_TARGET))
        # greedy balance by SA+SB
        order_d = sorted(cls_list, key=lambda c: -(SA[c] + SB[c]))
        gsets = [[] for _ in range(ng)]
        gsum = [0] * ng
        for c in order_d:
            q = min(range(ng), key=lambda i: (gsum[i], i))
            gsets[q].append(c)
            gsum[q] += SA[c] + SB[c]
        for g in gsets:
            groups.append(sorted(g))
            grp_chunk.append(ci)
    NGRP = len(groups)

    # slot offsets per class within its group's A/B gathers
    oa = np.zeros(NBLK, np.int64)
    ob = np.zeros(NBLK, np.int64)
    grp_of = np.zeros(NBLK, np.int64)
    SAg = np.zeros(NGRP, np.int64)
    SBg = np.zeros(NGRP, np.int64)
    for q, bl in enumerate(groups):
        offa = 0
        for bq in bl:
            oa[bq] = offa
            offa += SA[bq]
            grp_of[bq] = q
        offb = 0
        for bq in bl:
            ob[bq] = offb
            offb += SB[bq]
        SAg[q] = offa
        SBg[q] = offb

    colA0 = np.zeros(NGRP, np.int64)
    colB0 = np.zeros(NGRP, np.int64)
    cur = 0
    for q in range(NGRP):
        colA0[q] = cur
        cur += int(SAg[q]) * P // 16
        colB0[q] = cur
        cur += int(SBg[q]) * P // 16
    idxcols = int(cur)

    # ---- build per-core idx tensors ----
    zA_rel = np.int16(zA)
    zB_rel = np.int16(zB - cfg.WB_OFF)
    idx_np = np.empty((NC, 128, idxcols), np.int16)
    for q in range(NGRP):
        idx_np[:, :, colA0[q]:colA0[q] + int(SAg[q]) * P // 16] = zA_rel
        idx_np[:, :, colB0[q]:colB0[q] + int(SBg[q]) * P // 16] = zB_rel

    e_core = core_of_pos[d_s]
    e_cls = cls_of_pos[d_s]
    e_j = j_of_pos[d_s]
    e_q = grp_of[e_cls]
    posA = (oa[e_cls] + slotA) * P + e_j
    colA = colA0[e_q] + posA // 16
    rowA = posA % 16
    posB = (ob[e_cls] + slotB) * P + e_j
    colB = colB0[e_q] + posB // 16
    rowB = posB % 16
    valA = s_s.astype(np.int16)
    valB = (s_s - cfg.WB_OFF).astype(np.int16)
    for k in range(NC):
        mk = e_core == k
        mAk = mk & goA
        mBk = mk & goB
        for r in range(8):
            idx_np[k, rowA[mAk] + 16 * r, colA[mAk]] = valA[mAk]
            idx_np[k, rowB[mBk] + 16 * r, colB[mBk]] = valB[mBk]

    # ---- dense per-core arrays ----
    rmask = pos2old >= 0
    dinv_pos = np.ones(NPAD, np.float32)
    dinv_pos[rmask] = dinv[pos2old[rmask]].astype(np.float32)
    x_pos = np.zeros((NPAD, cfg.F), np.float32)
    x_pos[rmask] = x[pos2old[rmask]]
    bias_pos = np.zeros((NPAD, 16), np.float32)
    bias_pos[rmask] = bias[pos2old[rmask]].astype(np.float32)

    xraw_t = x_pos * dinv_pos[:, None]            # pass-4 table: dinv*x
    # per-core [j, b] layouts: position of (core k, cls b, j)
    pos_kbj = np.zeros((NC, NBLK, P), np.int64)
    for ci, (r, nb) in enumerate(chunks):
        for kc in range(NC):
            for s in range(nb):
                base = chunk_start_pos[ci] + kc * nb * P + s * P
                pos_kbj[kc, chunk_cls0[ci] + s] = base + np.arange(P)

    in_maps = []
    mmats = np.ascontiguousarray(np.concatenate([M3, M2, M1], axis=1).astype(np.float32))
    m4 = np.ascontiguousarray(M4.astype(np.float32))
    for k in range(NC):
        pk = pos_kbj[k]                            # [NBLK, P] positions
        db = dinv_pos[pk].T.astype(np.float32)     # [P, NBLK]
        d2 = (db * db).astype(np.float32)
        xTloc = np.ascontiguousarray(
            x_pos[pk.reshape(-1)].T)               # [F, PER] raw x
        xd2 = np.ascontiguousarray(x_pos[pk].transpose(1, 0, 2)
                                   * (dinv_pos[pk] ** 2).T[:, :, None]).astype(np.float32)
        bias_blk = np.ascontiguousarray(bias_pos[pk].transpose(1, 0, 2)).astype(np.float32)
        in_maps.append(dict(
            xraw=xraw_t,
            idx=np.ascontiguousarray(idx_np[k]),
            xT=xTloc,
            db=np.ascontiguousarray(db),
            d2=np.ascontiguousarray(d2),
            xd2=xd2,
            bias_blk=bias_blk,
            mmats=mmats,
            m4=m4,
        ))

    layout = dict(SA=SA, SB=SB, groups=groups, grp_chunk=grp_chunk, oa=oa, ob=ob,
                  SAg=SAg, SBg=SBg, colA0=colA0, colB0=colB0, idxcols=idxcols,
                  chunks=chunks, chunk_start_pos=chunk_start_pos,
                  chunk_cls0=chunk_cls0)
    return in_maps, layout, pos_kbj, pos2old


# --------------------------------------------------------------------------
# device module
# --------------------------------------------------------------------------

def _build_module(cfg: Cfg, layout):
    P, PER, NPAD, NBLK, NC = cfg.P, cfg.PER, cfg.NPAD, cfg.NBLK, cfg.NCORES
    SA, SB = layout["SA"], layout["SB"]
    groups, grp_chunk = layout["groups"], layout["grp_chunk"]
    oa, ob = layout["oa"], layout["ob"]
    SAg, SBg = layout["SAg"], layout["SBg"]
    colA0, colB0 = layout["colA0"], layout["colB0"]
    idxcols = layout["idxcols"]
    chunks = layout["chunks"]
    chunk_start_pos = layout["chunk_start_pos"]
    chunk_cls0 = layout["chunk_cls0"]
    NGRP = len(groups)
    NCH = len(chunks)

    nc = bacc.Bacc("TRN2", target_bir_lowering=False, debug=False, num_devices=NC,
                   num_swdge_queues=cfg.NQ, dynamic_dma_scratch_size=16384)

    xraw = nc.dram_tensor("xraw", [NPAD, cfg.F], F32, kind="ExternalInput").ap()
    idx = nc.dram_tensor("idx", [128, idxcols], I16, kind="ExternalInput").ap()
    xT = nc.dram_tensor("xT", [cfg.F, PER], F32, kind="ExternalInput").ap()
    db_in = nc.dram_tensor("db", [P, NBLK], F32, kind="ExternalInput").ap()
    d2_in = nc.dram_tensor("d2", [P, NBLK], F32, kind="ExternalInput").ap()
    xd2_in = nc.dram_tensor("xd2", [P, NBLK, cfg.F], F32, kind="ExternalInput").ap()
    bias_in = nc.dram_tensor("bias_blk", [P, NBLK, 16], F32, kind="ExternalInput").ap()
    mmats = nc.dram_tensor("mmats", [cfg.F, 48], F32, kind="ExternalInput").ap()
    m4 = nc.dram_tensor("m4", [cfg.F, 16], F32, kind="ExternalInput").ap()
    out = nc.dram_tensor("out", [P, NBLK, 16], F32, kind="ExternalOutput").ap()

    with tile.TileContext(nc) as tc:
        with (
            tc.tile_pool(name="const", bufs=1) as cp,
            tc.tile_pool(name="dram", bufs=1, space="DRAM") as dp,
        ):
            idx_sb = cp.tile([128, idxcols], I16)
            nc.sync.dma_start(idx_sb[:], idx)
            xT_sb = cp.tile([cfg.F, PER], F32)
            nc.sync.dma_start(xT_sb[:], xT)
            mm_sb = cp.tile([cfg.F, 48], F32)
            nc.sync.dma_start(mm_sb[:], mmats)
            m4_sb = cp.tile([cfg.F, 16], F32)
            nc.sync.dma_start(m4_sb[:], m4)
            db_sb = cp.tile([P, NBLK], F32)
            nc.sync.dma_start(db_sb[:], db_in)
            d2_sb = cp.tile([P, NBLK], F32)
            nc.sync.dma_start(d2_sb[:], d2_in)
            xd2_sb = cp.tile([P, NBLK, cfg.F], F32)
            nc.sync.dma_start(xd2_sb[:], xd2_in)
            bias_sb = cp.tile([P, NBLK, 16], F32)
            nc.sync.dma_start(bias_sb[:], bias_in)
            ident = cp.tile([P, P], F32)
            make_identity(nc, ident[:])

            # each generation is a (w1, w2) pair: w1 = table rows [0,32768),
            # w2 = rows [17408, 50176); chunk exchanges write into one or both
            tw = [(dp.tile([cfg.WA_LEN, cfg.F], F32, name=f"tw{i}_1"),
                   dp.tile([cfg.WA_LEN, cfg.F], F32, name=f"tw{i}_2"))
                  for i in range(2)]
            ccin = [dp.tile([nb * P, 16], F32, name=f"ccin{c}")
                    for c, (r, nb) in enumerate(chunks)]
            ccout = [[dp.tile([NC * nb * P, 16], F32, addr_space="Shared",
                              name=f"ccout{p}_{c}")
                      for c, (r, nb) in enumerate(chunks)] for p in range(3)]

            with (
                tc.tile_pool(name="gatha", bufs=cfg.LAG + 3) as gpa,
                tc.tile_pool(name="gathb", bufs=6) as gpb,
                tc.tile_pool(name="work", bufs=4) as wp,
                tc.tile_pool(name="stage", bufs=2) as sp,
                tc.tile_pool(name="psum", bufs=4, space="PSUM") as psp,
            ):
                # greedy queue schedule: call (in EMISSION order) -> least-loaded
                emit_order = []
                for step in range(NGRP + cfg.LAG):
                    if step < NGRP:
                        emit_order.append((step, "A", int(SAg[step])))
                    h = step - cfg.LAG
                    if h >= 0:
                        emit_order.append((h, "B", int(SBg[h])))
                qload = [0] * cfg.NQ
                qsched = {}
                for q, part, sz in emit_order:
                    if sz:
                        qq = min(range(cfg.NQ), key=lambda i: (qload[i], i))
                        qload[qq] += sz * P
                        qsched[(q, part)] = qq

                def make_stages(tag):
                    return [sp.tile([P, nb, 16], F32, tag=f"{tag}{c}",
                                    name=f"st_{tag}{c}")
                            for c, (r, nb) in enumerate(chunks)]

                def st_slot(sts, bq):
                    for c in range(NCH):
                        if bq < chunk_cls0[c + 1]:
                            return sts[c][:, bq - chunk_cls0[c], :]
                    raise AssertionError

                def exchange_start(c, st_tile):
                    nc.sync.dma_start(
                        ccin[c][:].rearrange("(b p) f -> p b f", p=P), st_tile[:])

                def exchange_finish(p, c, target):
                    # deferred so the gpsimd-resident collective trigger never
                    # blocks gather dispatch waiting on the ccin DMA
                    r, nb = chunks[c]
                    w1, w2 = target
                    nc.gpsimd.collective_compute(
                        "AllGather", mybir.AluOpType.bypass,
                        replica_groups=[list(range(NC))],
                        ins=[ccin[c][:]], outs=[ccout[p][c][:]],
                    )
                    s = int(chunk_start_pos[c])
                    rows = NC * nb * P
                    if s < cfg.WA_LEN:                    # overlaps window 1
                        hi = min(s + rows, cfg.WA_LEN)
                        nc.scalar.dma_start(
                            w1[s:hi, 0:16], ccout[p][c][0:hi - s, :])
                    if s + rows > cfg.WB_OFF:             # overlaps window 2
                        lo = max(s, cfg.WB_OFF)
                        nc.scalar.dma_start(
                            w2[lo - cfg.WB_OFF:s + rows - cfg.WB_OFF, 0:16],
                            ccout[p][c][lo - s:rows, :])

                last_of_chunk = {}
                for q in range(NGRP):
                    last_of_chunk[grp_chunk[q]] = q

                def run_pass(winA, winB, width, epi, chunk_hook=None):
                    # A-gathers issue LAG groups ahead of B-gathers+reduces so
                    # queue FIFOs stay busy across the pass boundary (A only
                    # depends on the first two chunk exchanges of the prior
                    # pass, B on all four).
                    gtA = {}
                    gtB = {}

                    def emit_A(q):
                        sag = int(SAg[q])
                        if not sag:
                            return
                        t = gpa.tile([P, sag, cfg.F], F32, tag="gtA", name="gtA")
                        gtA[q] = t
                        nc.gpsimd.dma_gather(
                            out_ap=t[:],
                            in_ap=winA,
                            idxs_ap=idx_sb[:, int(colA0[q]):int(colA0[q]) + sag * P // 16],
                            num_idxs=sag * P,
                            num_idxs_reg=sag * P,
                            elem_size=cfg.F,
                            single_packet=False,
                            queue_num=qsched[(q, "A")],
                        )

                    def emit_B(q):
                        sbg = int(SBg[q])
                        if not sbg:
                            return
                        t = gpb.tile([P, sbg, cfg.F], F32, tag="gtB", name="gtB")
                        gtB[q] = t
                        nc.gpsimd.dma_gather(
                            out_ap=t[:],
                            in_ap=winB,
                            idxs_ap=idx_sb[:, int(colB0[q]):int(colB0[q]) + sbg * P // 16],
                            num_idxs=sbg * P,
                            num_idxs_reg=sbg * P,
                            elem_size=cfg.F,
                            single_packet=False,
                            queue_num=qsched[(q, "B")],
                        )

                    def emit_reduces(q):
                        for bq in groups[q]:
                            acc = wp.tile([P, cfg.F], F32, tag="acc")
                            wrote = False
                            if SA[bq]:
                                a0, a1 = int(oa[bq]), int(oa[bq] + SA[bq])
                                nc.vector.reduce_sum(
                                    out=acc[:, 0:width],
                                    in_=gtA[q][:, a0:a1, 0:width].rearrange("p s f -> p f s"),
                                    axis=mybir.AxisListType.X,
                                )
                                wrote = True
                            if SB[bq]:
                                b0_, b1_ = int(ob[bq]), int(ob[bq] + SB[bq])
                                if wrote:
                                    acc2 = wp.tile([P, cfg.F], F32, tag="acc2")
                                    nc.vector.reduce_sum(
                                        out=acc2[:, 0:width],
                                        in_=gtB[q][:, b0_:b1_, 0:width].rearrange("p s f -> p f s"),
                                        axis=mybir.AxisListType.X,
                                    )
                                    nc.vector.tensor_add(
                                        out=acc[:, 0:width], in0=acc[:, 0:width],
                                        in1=acc2[:, 0:width])
                                else:
                                    nc.vector.reduce_sum(
                                        out=acc[:, 0:width],
                                        in_=gtB[q][:, b0_:b1_, 0:width].rearrange("p s f -> p f s"),
                                        axis=mybir.AxisListType.X,
                                    )
                            epi(bq, acc)

                    pending_finish = []
                    for step in range(NGRP + cfg.LAG + cfg.CC_DELAY):
                        if step < NGRP:
                            emit_A(step)
                        h = step - cfg.LAG
                        if chunk_hook is not None:
                            for dc, due in list(pending_finish):
                                if step - due >= cfg.CC_DELAY or h >= NGRP - 1:
                                    chunk_hook[1](dc)
                                    pending_finish.remove((dc, due))
                        if 0 <= h < NGRP:
                            emit_B(h)
                            emit_reduces(h)
                            if chunk_hook is not None and h == last_of_chunk[grp_chunk[h]]:
                                chunk_hook[0](grp_chunk[h])
                                pending_finish.append((grp_chunk[h], step))

                # ---- pass 4: gather dinv*x (64-wide), project via M4 ----
                st4 = make_stages("s")

                def epi4(bq, R):
                    rs = wp.tile([P, cfg.F], F32, tag="rs")
                    # rs = db*R + d2*x_own   (u such that st = db*(x@M3) + db*u@M4)
                    nc.scalar.activation(rs[:], R[:], AF.Copy, scale=db_sb[:, bq:bq + 1])
                    nc.vector.tensor_add(out=rs[:], in0=rs[:], in1=xd2_sb[:, bq, :])
                    pT = psp.tile([cfg.F, P], F32, tag="pT")
                    nc.tensor.transpose(pT[:], rs[:], ident[:])
                    rsT = wp.tile([cfg.F, P], F32, tag="rsT")
                    nc.vector.tensor_copy(rsT[:], pT[:])
                    ps = psp.tile([P, 16], F32, tag="ps")
                    nc.tensor.matmul(out=ps[:], lhsT=xT_sb[:, bq * P:(bq + 1) * P],
                                     rhs=mm_sb[:, 0:16], start=True, stop=False)
                    nc.tensor.matmul(out=ps[:], lhsT=rsT[:], rhs=m4_sb[:],
                                     start=False, stop=True)
                    nc.scalar.activation(st_slot(st4, bq), ps[:], AF.Copy,
                                         scale=db_sb[:, bq:bq + 1])

                run_pass(xraw[0:cfg.WA_LEN, :], xraw[cfg.WB_OFF:NPAD, :], cfg.F,
                         epi4, chunk_hook=(lambda c: exchange_start(c, st4[c]),
                                           lambda c: exchange_finish(0, c, tw[0])))

                # ---- passes 3 and 2 ----
                def mk_epi(mcol, sts_prev, sts_new):
                    def epi(bq, R):
                        ps = psp.tile([P, 16], F32, tag="ps")
                        nc.tensor.matmul(out=ps[:],
                                         lhsT=xT_sb[:, bq * P:(bq + 1) * P],
                                         rhs=mm_sb[:, mcol:mcol + 16],
                                         start=True, stop=True)
                        # acc_full = R + prev_stage (self-loop)
                        accf = wp.tile([P, 16], F32, tag="accf")
                        nc.vector.tensor_add(out=accf[:], in0=R[:, 0:16],
                                             in1=st_slot(sts_prev, bq))
                        ta = wp.tile([P, 16], F32, tag="ta")
                        nc.scalar.activation(ta[:], ps[:], AF.Copy,
                                             scale=db_sb[:, bq:bq + 1])
                        tb = wp.tile([P, 16], F32, tag="tb")
                        nc.scalar.activation(tb[:], accf[:], AF.Copy,
                                             scale=d2_sb[:, bq:bq + 1])
                        nc.vector.tensor_add(out=st_slot(sts_new, bq),
                                             in0=ta[:], in1=tb[:])
                    return epi

                st3 = make_stages("s")
                run_pass(tw[0][0][:], tw[0][1][:], 16, mk_epi(16, st4, st3),
                         chunk_hook=(lambda c: exchange_start(c, st3[c]),
                                     lambda c: exchange_finish(1, c, tw[1])))

                st2 = make_stages("s")
                run_pass(tw[1][0][:], tw[1][1][:], 16, mk_epi(32, st3, st2),
                         chunk_hook=(lambda c: exchange_start(c, st2[c]),
                                     lambda c: exchange_finish(2, c, tw[0])))

                # ---- pass 1: final output ----
                st1 = make_stages("s")

                def epi1(bq, R):
                    accf = wp.tile([P, 16], F32, tag="accf")
                    nc.vector.tensor_add(out=accf[:], in0=R[:, 0:16],
                                         in1=st_slot(st2, bq))
                    t1 = wp.tile([P, 16], F32, tag="ta")
                    nc.scalar.activation(t1[:], accf[:], AF.Copy,
                                         scale=db_sb[:, bq:bq + 1])
                    nc.vector.tensor_add(out=st_slot(st1, bq), in0=t1[:],
                                         in1=bias_sb[:, bq, :])

                run_pass(tw[0][0][:], tw[0][1][:], 16, epi1)
                for c in range(NCH):
                    lo, hi = int(chunk_cls0[c]), int(chunk_cls0[c + 1])
                    nc.sync.dma_start(out[:, lo:hi, :], st1[c][:])

    return nc


# --------------------------------------------------------------------------
# entry point
# --------------------------------------------------------------------------

def _run(inputs, cfg: Cfg, runner=None, **run_kwargs):
    global LAST_RESULTS
    in_maps, layout, pos_kbj, pos2old = _host_prep(inputs, cfg)
    nc = _build_module(cfg, layout)
    nc.compile()
    if runner is None:
        res = run_bass_kernel_spmd(nc, in_maps, core_ids=list(range(cfg.NCORES)),
                                   **run_kwargs)
        LAST_RESULTS = res
        outs = res.results
    else:
        outs = runner(nc, in_maps)
    full = np.empty((cfg.NPAD, 16), np.float32)
    for k in range(cfg.NCORES):
        o = np.asarray(outs[k]["out"])  # [P, NBLK, 16]
        full[pos_kbj[k].reshape(-1)] = o.transpose(1, 0, 2).reshape(cfg.PER, 16)
    old2new = np.empty(cfg.N, np.int64)
    rmask = pos2old >= 0
    old2new[pos2old[rmask]] = np.nonzero(rmask)[0]
    return full[old2new]


def kernel(**inputs) -> np.ndarray:
    return _run(inputs, CFG)


# revision 16
# speedup vs baseline: 1.6219x; 1.0589x over previous
"""Trainium2 Bass kernel: DGCNN forward (4-layer GCN + Conv1d readout) on 8 NeuronCores.

Math restructuring (validated vs reference to 2e-7):
  With A = D^-1/2 (Adj + I) D^-1/2 and Mk / ck derived from the (tiny) weights,
    out = A(x M1 + A(x M2 + A(x M3 + A(x M4)))) + 1 c0 + v1 c1 + v2 c2 + v3 c3
  Passes aggregate tables T; self-loop contributions are added in the epilogue
  from SBUF-resident data (previous pass's stage), so gathers cover only real
  edges.  Pass 4 gathers the 64-wide dinv*x table (host pre-scaled); its
  epilogue projects through M4.  Passes 3/2/1 gather 16-wide tables.

Device strategy (graph-parallel over 8 cores):
  - dma_gather (SWDGE) is descriptor-rate-bound (~8.1ns/desc per queue, 4
    queues scale linearly), so the kernel minimizes descriptors and keeps all
    4 queues fed:
    * nodes are placed into 128-row blocks clustered by (degree, #window-A
      sources, #window-B sources) signature, cutting ELL padding to ~5%
    * blocks are dealt into 8-wide "classes" (one block per core) so the SPMD
      module has uniform shapes; class slot budgets are cross-core maxes
    * gather tile pool is 6 deep and ~36 gather calls/pass rotate over the 4
      SWDGE queues so descriptor generation runs ~4-way concurrent
  - int16 gather indices limit a window to 32768 rows; the 50176-row table is
    covered by two overlapping windows ([0,32768) and [17408,50176)); each
    dst's edges are split between windows inside its class budgets SA/SB.
  - The table is laid out in 4 exchange chunks ([17,15,13,4] blocks/core,
    region-aligned) so each AllGather output is a contiguous table range; a
    single DRAM->DRAM DMA restrides [rows,16] into the 256B-row table.  The
    first 3 chunk exchanges overlap the current pass's remaining gathers; only
    the small 4-block tail exchange sits on the pass boundary.
"""

import dataclasses
import numpy as np

import concourse.bass as bass
import concourse.bacc as bacc
import concourse.tile as tile
from concourse import mybir
from concourse.bass_utils import run_bass_kernel_spmd
from concourse.masks import make_identity

F32 = mybir.dt.float32
I16 = mybir.dt.int16
AF = mybir.ActivationFunctionType


@dataclasses.dataclass(frozen=True)
class Cfg:
    N: int = 50000          # real nodes
    F: int = 64             # features
    NCORES: int = 8
    P: int = 128
    NBLK: int = 49          # blocks (classes) per core
    NQ: int = 4             # SWDGE queues
    GT_BUFS: int = 4        # gather tile pool depth
    GRP_TARGET: float = 2.5  # classes per gather group (approx)

    # exchange chunks: (region, blocks-per-core) in PROCESSING order; regions
    # are the int16 window areas R0=[0,17408) R1=[17408,32768) R2=[32768,50176).
    # The overlap region (1) is processed/exchanged first since both gather
    # windows need it; the 4-block tail is the only boundary-critical piece.
    CHUNKS = ((1, 15), (0, 17), (2, 13), (2, 4))
    LAG: int = 4            # A-call emission lead over B-call+reduce
    CC_DELAY: int = 2       # groups between ccin DMA and collective trigger

    @property
    def PER(self):
        return self.NBLK * self.P

    @property
    def NPAD(self):
        return self.NCORES * self.PER

    @property
    def WA_LEN(self):
        return 32768

    @property
    def WB_OFF(self):
        return self.NPAD - 32768


CFG = Cfg()

LAST_RESULTS = None


# --------------------------------------------------------------------------
# host preprocessing
# --------------------------------------------------------------------------

def _host_prep(inputs, cfg: Cfg):
    x = np.asarray(inputs["x"], np.float32)
    ei = np.asarray(inputs["edge_index"]).astype(np.int64)
    W = [np.asarray(inputs[f"W{i}"], np.float64) for i in range(4)]
    b = [np.asarray(inputs[f"b{i}"], np.float64) for i in range(4)]
    conv_w = np.asarray(inputs["conv_w"], np.float64)
    conv_b = np.asarray(inputs["conv_b"], np.float64)

    n = x.shape[0]
    assert n == cfg.N and x.shape[1] == cfg.F
    P, PER, NPAD, NBLK, NC = cfg.P, cfg.PER, cfg.NPAD, cfg.NBLK, cfg.NCORES

    src0, dst0 = ei[0], ei[1]           # real edges only; self-loops in epilogue
    E0 = len(src0)
    deg = np.bincount(dst0, minlength=n).astype(np.float64) + 1.0  # incl self
    dinv = 1.0 / np.sqrt(deg)

    # ---- weight-derived small matrices ----
    Cw = [conv_w[:, 0:64], conv_w[:, 64:128], conv_w[:, 128:192], conv_w[:, 192:193]]
    M1 = W[0] @ Cw[0].T
    M2 = W[0] @ W[1] @ Cw[1].T
    M3 = W[0] @ W[1] @ W[2] @ Cw[2].T
    M4 = W[0] @ W[1] @ W[2] @ W[3] @ Cw[3].T
    c0 = b[0] @ Cw[0].T + b[1] @ Cw[1].T + b[2] @ Cw[2].T + b[3] @ Cw[3].T + conv_b
    c1 = (b[0] @ W[1]) @ Cw[1].T + (b[1] @ W[2]) @ Cw[2].T + (b[2] @ W[3]) @ Cw[3].T
    c2 = (b[0] @ W[1] @ W[2]) @ Cw[2].T + (b[1] @ W[2] @ W[3]) @ Cw[3].T
    c3 = (b[0] @ W[1] @ W[2] @ W[3]) @ Cw[3].T

    def aggv(v):
        o = np.zeros(n)
        np.add.at(o, dst0, (v * dinv)[src0])
        o += v * dinv
        return o * dinv

    v1 = aggv(np.ones(n))
    v2 = aggv(v1)
    v3 = aggv(v2)
    bias = (np.outer(np.ones(n), c0) + np.outer(v1, c1)
            + np.outer(v2, c2) + np.outer(v3, c3))  # [n, 16]

    # ---- region assignment + signatures ----
    # region position ranges (block aligned): R0 [0,17408) R1 [17408,32768)
    # R2 [32768,50176); pads: 2 at end of R0, 2 at end of R1, 172 end of R2
    RSTART = np.array([0, 17408, 32768, NPAD])
    RCAP = np.array([17408 - 2, 15360 - 2, 17408 - 172])
    assert RCAP.sum() == n
    region_of_node = np.repeat(np.arange(3), RCAP)  # node id order
    sreg = region_of_node[src0]
    gdeg = np.bincount(dst0, minlength=n).astype(np.int64)
    nA_n = np.bincount(dst0, weights=(sreg == 0), minlength=n).astype(np.int64)
    nB_n = np.bincount(dst0, weights=(sreg == 2), minlength=n).astype(np.int64)

    # chunk layout: per core, chunks of blocks; classes indexed 0..NBLK-1
    chunks = cfg.CHUNKS
    nb_of_chunk = [c[1] for c in chunks]
    assert sum(nb_of_chunk) == NBLK
    rcursor = {0: 0, 1: 17408, 2: 32768}
    chunk_start_pos = []
    for r, nb in chunks:
        chunk_start_pos.append(rcursor[r])
        rcursor[r] += NC * nb * P
    chunk_start_pos = np.array(chunk_start_pos + [NPAD])  # last entry unused
    chunk_cls0 = np.concatenate([[0], np.cumsum(nb_of_chunk)])

    # per region: sort real nodes by signature, form blocks, rank into classes
    pos_of_node = np.full(n, -1, np.int64)
    cls_cost = np.zeros(NBLK, np.int64)
    for r in range(3):
        nodes = np.nonzero(region_of_node == r)[0]
        k = np.lexsort((nB_n[nodes], nA_n[nodes], gdeg[nodes] // 4))
        nodes = nodes[k]
        nblocks_r = (RSTART[r + 1] - RSTART[r]) // P
        ncls_r = nblocks_r // NC
        # blocks of 128 consecutive sorted nodes (pads fill the tail)
        nfull = len(nodes)
        # block cost = max(gdeg, nA+nB) over its nodes (pads contribute 0)
        bcost = np.zeros(nblocks_r, np.int64)
        bi = np.arange(nfull) // P
        np.maximum.at(bcost, bi, np.maximum(gdeg[nodes], nA_n[nodes] + nB_n[nodes]))
        # class i = NC CONSECUTIVE blocks in signature order (keeps budgets
        # tight); rank classes by cost only to route small ones to the tail
        # chunk; per-core load is Sum(class budgets) regardless (1 blk/core).
        ccost = bcost.reshape(ncls_r, NC).max(axis=1)
        cls_rank = np.argsort(-ccost, kind="stable")  # region-class in cost order
        rchunks = [ci for ci, c in enumerate(chunks) if c[0] == r]
        cls_slots = []  # (chunk_idx, slot_in_chunk); tail chunk listed last
        for ci in rchunks:
            cls_slots += [(ci, s) for s in range(nb_of_chunk[ci])]
        assert len(cls_slots) == ncls_r
        for i in range(ncls_r):
            ci, slot = cls_slots[i]
            cls_id = chunk_cls0[ci] + slot
            rc = cls_rank[i]                      # region-class index
            for kc in range(NC):
                blk = rc * NC + kc
                base = (chunk_start_pos[ci] + kc * nb_of_chunk[ci] * P + slot * P)
                lo, hi = blk * P, min((blk + 1) * P, nfull)
                if hi > lo:
                    pos_of_node[nodes[lo:hi]] = base + np.arange(hi - lo)
                cls_cost[cls_id] = max(cls_cost[cls_id], bcost[blk])

    assert (pos_of_node >= 0).all()
    pos2old = np.full(NPAD, -1, np.int64)
    pos2old[pos_of_node] = np.arange(n)

    # position -> (core, local block b, j)
    pos = np.arange(NPAD)
    core_of_pos = np.zeros(NPAD, np.int64)
    cls_of_pos = np.zeros(NPAD, np.int64)
    for ci, (r, nb) in enumerate(chunks):
        s = chunk_start_pos[ci]
        e = s + NC * nb_of_chunk[ci] * P
        rel = pos[s:e] - s
        core_of_pos[s:e] = rel // (nb * P)
        cls_of_pos[s:e] = chunk_cls0[ci] + (rel % (nb * P)) // P
    j_of_pos = pos % P

    # dummy zero rows: a pad position in window A and one in window B
    pad_pos = np.nonzero(pos2old < 0)[0]
    zA = int(pad_pos[pad_pos < cfg.WA_LEN][-1])
    zB = int(pad_pos[pad_pos >= cfg.WB_OFF][-1])
    assert zA != zB

    # ---- per-edge window split with per-class budgets ----
    s_pos = pos_of_node[src0]
    d_pos = pos_of_node[dst0]
    eo = np.argsort(d_pos, kind="stable")
    s_s = s_pos[eo]
    d_s = d_pos[eo]
    starts = np.searchsorted(d_s, np.arange(NPAD + 1))

    isA = s_s < cfg.WB_OFF
    isB = s_s >= cfg.WA_LEN
    isF = ~(isA | isB)
    nAo = np.bincount(d_s, weights=isA, minlength=NPAD).astype(np.int64)
    nBo = np.bincount(d_s, weights=isB, minlength=NPAD).astype(np.int64)
    nf = np.bincount(d_s, weights=isF, minlength=NPAD).astype(np.int64)
    tot = nAo + nBo + nf

    cp = cls_of_pos
    mA = np.zeros(NBLK, np.int64); np.maximum.at(mA, cp, nAo)
    mB = np.zeros(NBLK, np.int64); np.maximum.at(mB, cp, nBo)
    mT = np.zeros(NBLK, np.int64); np.maximum.at(mT, cp, tot)
    M = np.maximum(mT, mA + mB)
    SA = np.clip((M + 1) // 2, mA, M - mB)
    SB = M - SA
    SAp = SA[cp]
    SBp = SB[cp]
    nA = np.clip(tot - SBp, nAo, np.minimum(nAo + nf, SAp))

    cFex = np.concatenate([[0], np.cumsum(isF)])
    frank = cFex[:-1] - cFex[starts[d_s]]
    goA = isA | (isF & (frank < (nA - nAo)[d_s]))
    goB = ~goA
    cAex = np.concatenate([[0], np.cumsum(goA)])
    slotA = cAex[:-1] - cAex[starts[d_s]]
    cBex = np.concatenate([[0], np.cumsum(goB)])
    slotB = cBex[:-1] - cBex[starts[d_s]]
    nB_ = tot - nA
    assert (nA <= SAp).all() and (nB_ <= SBp).all()
    assert (SA + SB > 0).all()

    # ---- groups: classes within each chunk, balanced by slots ----
    groups = []          # list of list of class ids
    grp_chunk = []       # chunk index of each group
    for ci, (r, nb) in enumerate(chunks):
        cls_list = list(range(chunk_cls0[ci], chunk_cls0[ci] + nb))
        ng = int(np.ceil(nb / cfg.GRP_TARGET))
        # greedy balance by SA+SB
        order_d = sorted(cls_list, key=lambda c: -(SA[c] + SB[c]))
        gsets = [[] for _ in range(ng)]
        gsum = [0] * ng
        for c in order_d:
            q = min(range(ng), key=lambda i: (gsum[i], i))
            gsets[q].append(c)
            gsum[q] += SA[c] + SB[c]
        for g in gsets:
            groups.append(sorted(g))
            grp_chunk.append(ci)
    NGRP = len(groups)

    # slot offsets per class within its group's A/B gathers
    oa = np.zeros(NBLK, np.int64)
    ob = np.zeros(NBLK, np.int64)
    grp_of = np.zeros(NBLK, np.int64)
    SAg = np.zeros(NGRP, np.int64)
    SBg = np.zeros(NGRP, np.int64)
    for q, bl in enumerate(groups):
        offa = 0
        for bq in bl:
            oa[bq] = offa
            offa += SA[bq]
            grp_of[bq] = q
        offb = 0
        for bq in bl:
            ob[bq] = offb
            offb += SB[bq]
        SAg[q] = offa
        SBg[q] = offb

    colA0 = np.zeros(NGRP, np.int64)
    colB0 = np.zeros(NGRP, np.int64)
    cur = 0
    for q in range(NGRP):
        colA0[q] = cur
        cur += int(SAg[q]) * P // 16
        colB0[q] = cur
        cur += int(SBg[q]) * P // 16
    idxcols = int(cur)

    # ---- build per-core idx tensors ----
    zA_rel = np.int16(zA)
    zB_rel = np.int16(zB - cfg.WB_OFF)
    idx_np = np.empty((NC, 128, idxcols), np.int16)
    for q in range(NGRP):
        idx_np[:, :, colA0[q]:colA0[q] + int(SAg[q]) * P // 16] = zA_rel
        idx_np[:, :, colB0[q]:colB0[q] + int(SBg[q]) * P // 16] = zB_rel

    e_core = core_of_pos[d_s]
    e_cls = cls_of_pos[d_s]
    e_j = j_of_pos[d_s]
    e_q = grp_of[e_cls]
    posA = (oa[e_cls] + slotA) * P + e_j
    colA = colA0[e_q] + posA // 16
    rowA = posA % 16
    posB = (ob[e_cls] + slotB) * P + e_j
    colB = colB0[e_q] + posB // 16
    rowB = posB % 16
    valA = s_s.astype(np.int16)
    valB = (s_s - cfg.WB_OFF).astype(np.int16)
    for k in range(NC):
        mk = e_core == k
        mAk = mk & goA
        mBk = mk & goB
        for r in range(8):
            idx_np[k, rowA[mAk] + 16 * r, colA[mAk]] = valA[mAk]
            idx_np[k, rowB[mBk] + 16 * r, colB[mBk]] = valB[mBk]

    # ---- dense per-core arrays ----
    rmask = pos2old >= 0
    dinv_pos = np.ones(NPAD, np.float32)
    dinv_pos[rmask] = dinv[pos2old[rmask]].astype(np.float32)
    x_pos = np.zeros((NPAD, cfg.F), np.float32)
    x_pos[rmask] = x[pos2old[rmask]]
    bias_pos = np.zeros((NPAD, 16), np.float32)
    bias_pos[rmask] = bias[pos2old[rmask]].astype(np.float32)

    xraw_t = x_pos * dinv_pos[:, None]            # pass-4 table: dinv*x
    # per-core [j, b] layouts: position of (core k, cls b, j)
    pos_kbj = np.zeros((NC, NBLK, P), np.int64)
    for ci, (r, nb) in enumerate(chunks):
        for kc in range(NC):
            for s in range(nb):
                base = chunk_start_pos[ci] + kc * nb * P + s * P
                pos_kbj[kc, chunk_cls0[ci] + s] = base + np.arange(P)

    in_maps = []
    mmats = np.ascontiguousarray(np.concatenate([M3, M2, M1], axis=1).astype(np.float32))
    m4 = np.ascontiguousarray(M4.astype(np.float32))
    for k in range(NC):
        pk = pos_kbj[k]                            # [NBLK, P] positions
        db = dinv_pos[pk].T.astype(np.float32)     # [P, NBLK]
        d2 = (db * db).astype(np.float32)
        xTloc = np.ascontiguousarray(
            x_pos[pk.reshape(-1)].T)               # [F, PER] raw x
        xd2 = np.ascontiguousarray(x_pos[pk].transpose(1, 0, 2)
                                   * (dinv_pos[pk] ** 2).T[:, :, None]).astype(np.float32)
        bias_blk = np.ascontiguousarray(bias_pos[pk].transpose(1, 0, 2)).astype(np.float32)
        in_maps.append(dict(
            xraw=xraw_t,
            idx=np.ascontiguousarray(idx_np[k]),
            xT=xTloc,
            db=np.ascontiguousarray(db),
            d2=np.ascontiguousarray(d2),
            xd2=xd2,
            bias_blk=bias_blk,
            mmats=mmats,
            m4=m4,
        ))

    layout = dict(SA=SA, SB=SB, groups=groups, grp_chunk=grp_chunk, oa=oa, ob=ob,
                  SAg=SAg, SBg=SBg, colA0=colA0, colB0=colB0, idxcols=idxcols,
                  chunks=chunks, chunk_start_pos=chunk_start_pos,
                  chunk_cls0=chunk_cls0)
    return in_maps, layout, pos_kbj, pos2old


# --------------------------------------------------------------------------
# device module
# --------------------------------------------------------------------------

def _build_module(cfg: Cfg, layout):
    P, PER, NPAD, NBLK, NC = cfg.P, cfg.PER, cfg.NPAD, cfg.NBLK, cfg.NCORES
    SA, SB = layout["SA"], layout["SB"]
    groups, grp_chunk = layout["groups"], layout["grp_chunk"]
    oa, ob = layout["oa"], layout["ob"]
    SAg, SBg = layout["SAg"], layout["SBg"]
    colA0, colB0 = layout["colA0"], layout["colB0"]
    idxcols = layout["idxcols"]
    chunks = layout["chunks"]
    chunk_start_pos = layout["chunk_start_pos"]
    chunk_cls0 = layout["chunk_cls0"]
    NGRP = len(groups)
    NCH = len(chunks)

    nc = bacc.Bacc("TRN2", target_bir_lowering=False, debug=False, num_devices=NC,
                   num_swdge_queues=cfg.NQ, dynamic_dma_scratch_size=16384)

    xraw = nc.dram_tensor("xraw", [NPAD, cfg.F], F32, kind="ExternalInput").ap()
    idx = nc.dram_tensor("idx", [128, idxcols], I16, kind="ExternalInput").ap()
    xT = nc.dram_tensor("xT", [cfg.F, PER], F32, kind="ExternalInput").ap()
    db_in = nc.dram_tensor("db", [P, NBLK], F32, kind="ExternalInput").ap()
    d2_in = nc.dram_tensor("d2", [P, NBLK], F32, kind="ExternalInput").ap()
    xd2_in = nc.dram_tensor("xd2", [P, NBLK, cfg.F], F32, kind="ExternalInput").ap()
    bias_in = nc.dram_tensor("bias_blk", [P, NBLK, 16], F32, kind="ExternalInput").ap()
    mmats = nc.dram_tensor("mmats", [cfg.F, 48], F32, kind="ExternalInput").ap()
    m4 = nc.dram_tensor("m4", [cfg.F, 16], F32, kind="ExternalInput").ap()
    out = nc.dram_tensor("out", [P, NBLK, 16], F32, kind="ExternalOutput").ap()

    with tile.TileContext(nc) as tc:
        with (
            tc.tile_pool(name="const", bufs=1) as cp,
            tc.tile_pool(name="dram", bufs=1, space="DRAM") as dp,
        ):
            idx_sb = cp.tile([128, idxcols], I16)
            nc.sync.dma_start(idx_sb[:], idx)
            xT_sb = cp.tile([cfg.F, PER], F32)
            nc.sync.dma_start(xT_sb[:], xT)
            mm_sb = cp.tile([cfg.F, 48], F32)
            nc.sync.dma_start(mm_sb[:], mmats)
            m4_sb = cp.tile([cfg.F, 16], F32)
            nc.sync.dma_start(m4_sb[:], m4)
            db_sb = cp.tile([P, NBLK], F32)
            nc.sync.dma_start(db_sb[:], db_in)
            d2_sb = cp.tile([P, NBLK], F32)
            nc.sync.dma_start(d2_sb[:], d2_in)
            xd2_sb = cp.tile([P, NBLK, cfg.F], F32)
            nc.sync.dma_start(xd2_sb[:], xd2_in)
            bias_sb = cp.tile([P, NBLK, 16], F32)
            nc.sync.dma_start(bias_sb[:], bias_in)
            ident = cp.tile([P, P], F32)
            make_identity(nc, ident[:])

            # each generation is a (w1, w2) pair: w1 = table rows [0,32768),
            # w2 = rows [17408, 50176); chunk exchanges write into one or both
            tw = [(dp.tile([cfg.WA_LEN, cfg.F], F32, name=f"tw{i}_1"),
                   dp.tile([cfg.WA_LEN, cfg.F], F32, name=f"tw{i}_2"))
                  for i in range(2)]
            ccin = [dp.tile([nb * P, 16], F32, name=f"ccin{c}")
                    for c, (r, nb) in enumerate(chunks)]
            ccout = [[dp.tile([NC * nb * P, 16], F32, addr_space="Shared",
                              name=f"ccout{p}_{c}")
                      for c, (r, nb) in enumerate(chunks)] for p in range(3)]

            with (
                tc.tile_pool(name="gatha", bufs=cfg.LAG + 3) as gpa,
                tc.tile_pool(name="gathb", bufs=6) as gpb,
                tc.tile_pool(name="work", bufs=4) as wp,
                tc.tile_pool(name="stage", bufs=2) as sp,
                tc.tile_pool(name="psum", bufs=4, space="PSUM") as psp,
            ):
                # greedy queue schedule: call (in EMISSION order) -> least-loaded
                emit_order = []
                for step in range(NGRP + cfg.LAG):
                    if step < NGRP:
                        emit_order.append((step, "A", int(SAg[step])))
                    h = step - cfg.LAG
                    if h >= 0:
                        emit_order.append((h, "B", int(SBg[h])))
                # STRICT round-robin in emission order: each queue runs one
                # descgen at a time and the gpsimd engine dispatches in order,
                # so consecutive same-queue calls would head-of-line block.
                qsched = {}
                rr = [0]
                for q, part, sz in emit_order:
                    if sz:
                        qsched[(q, part)] = rr[0] % cfg.NQ
                        rr[0] += 1

                def make_stages(tag):
                    return [sp.tile([P, nb, 16], F32, tag=f"{tag}{c}",
                                    name=f"st_{tag}{c}")
                            for c, (r, nb) in enumerate(chunks)]

                def st_slot(sts, bq):
                    for c in range(NCH):
                        if bq < chunk_cls0[c + 1]:
                            return sts[c][:, bq - chunk_cls0[c], :]
                    raise AssertionError

                def exchange_start(c, st_tile):
                    nc.sync.dma_start(
                        ccin[c][:].rearrange("(b p) f -> p b f", p=P), st_tile[:])

                def exchange_finish(p, c, target):
                    # deferred so the gpsimd-resident collective trigger never
                    # blocks gather dispatch waiting on the ccin DMA
                    r, nb = chunks[c]
                    w1, w2 = target
                    nc.gpsimd.collective_compute(
                        "AllGather", mybir.AluOpType.bypass,
                        replica_groups=[list(range(NC))],
                        ins=[ccin[c][:]], outs=[ccout[p][c][:]],
                    )
                    s = int(chunk_start_pos[c])
                    rows = NC * nb * P
                    if s < cfg.WA_LEN:                    # overlaps window 1
                        hi = min(s + rows, cfg.WA_LEN)
                        nc.scalar.dma_start(
                            w1[s:hi, 0:16], ccout[p][c][0:hi - s, :])
                    if s + rows > cfg.WB_OFF:             # overlaps window 2
                        lo = max(s, cfg.WB_OFF)
                        nc.scalar.dma_start(
                            w2[lo - cfg.WB_OFF:s + rows - cfg.WB_OFF, 0:16],
                            ccout[p][c][lo - s:rows, :])

                last_of_chunk = {}
                for q in range(NGRP):
                    last_of_chunk[grp_chunk[q]] = q

                def run_pass(winA, winB, width, epi, chunk_hook=None):
                    # A-gathers issue LAG groups ahead of B-gathers+reduces so
                    # queue FIFOs stay busy across the pass boundary (A only
                    # depends on the first two chunk exchanges of the prior
                    # pass, B on all four).
                    gtA = {}
                    gtB = {}

                    def emit_A(q):
                        sag = int(SAg[q])
                        if not sag:
                            return
                        t = gpa.tile([P, sag, cfg.F], F32, tag="gtA", name="gtA")
                        gtA[q] = t
                        nc.gpsimd.dma_gather(
                            out_ap=t[:],
                            in_ap=winA,
                            idxs_ap=idx_sb[:, int(colA0[q]):int(colA0[q]) + sag * P // 16],
                            num_idxs=sag * P,
                            num_idxs_reg=sag * P,
                            elem_size=cfg.F,
                            single_packet=False,
                            queue_num=qsched[(q, "A")],
                        )

                    def emit_B(q):
                        sbg = int(SBg[q])
                        if not sbg:
                            return
                        t = gpb.tile([P, sbg, cfg.F], F32, tag="gtB", name="gtB")
                        gtB[q] = t
                        nc.gpsimd.dma_gather(
                            out_ap=t[:],
                            in_ap=winB,
                            idxs_ap=idx_sb[:, int(colB0[q]):int(colB0[q]) + sbg * P // 16],
                            num_idxs=sbg * P,
                            num_idxs_reg=sbg * P,
                            elem_size=cfg.F,
                            single_packet=False,
                            queue_num=qsched[(q, "B")],
                        )

                    def emit_reduces(q):
                        for bq in groups[q]:
                            acc = wp.tile([P, cfg.F], F32, tag="acc")
                            wrote = False
                            if SA[bq]:
                                a0, a1 = int(oa[bq]), int(oa[bq] + SA[bq])
                                nc.vector.reduce_sum(
                                    out=acc[:, 0:width],
                                    in_=gtA[q][:, a0:a1, 0:width].rearrange("p s f -> p f s"),
                                    axis=mybir.AxisListType.X,
                                )
                                wrote = True
                            if SB[bq]:
                                b0_, b1_ = int(ob[bq]), int(ob[bq] + SB[bq])
                                if wrote:
                                    acc2 = wp.tile([P, cfg.F], F32, tag="acc2")
                                    nc.vector.reduce_sum(
                                        out=acc2[:, 0:width],
                                        in_=gtB[q][:, b0_:b1_, 0:width].rearrange("p s f -> p f s"),
                                        axis=mybir.AxisListType.X,
                                    )
                                    nc.vector.tensor_add(
                                        out=acc[:, 0:width], in0=acc[:, 0:width],
                                        in1=acc2[:, 0:width])
                                else:
                                    nc.vector.reduce_sum(
                                        out=acc[:, 0:width],
                                        in_=gtB[q][:, b0_:b1_, 0:width].rearrange("p s f -> p f s"),
                                        axis=mybir.AxisListType.X,
                                    )
                            epi(bq, acc)

                    pending_finish = []
                    for step in range(NGRP + cfg.LAG + cfg.CC_DELAY):
                        if step < NGRP:
                            emit_A(step)
                        h = step - cfg.LAG
                        if chunk_hook is not None:
                            for dc, due in list(pending_finish):
                                if step - due >= cfg.CC_DELAY or h >= NGRP - 1:
                                    chunk_hook[1](dc)
                                    pending_finish.remove((dc, due))
                        if 0 <= h < NGRP:
                            emit_B(h)
                            emit_reduces(h)
                            if chunk_hook is not None and h == last_of_chunk[grp_chunk[h]]:
                                chunk_hook[0](grp_chunk[h])
                                pending_finish.append((grp_chunk[h], step))

                # ---- pass 4: gather dinv*x (64-wide), project via M4 ----
                st4 = make_stages("s")

                def epi4(bq, R):
                    rs = wp.tile([P, cfg.F], F32, tag="rs")
                    # rs = db*R + d2*x_own   (u such that st = db*(x@M3) + db*u@M4)
                    nc.scalar.activation(rs[:], R[:], AF.Copy, scale=db_sb[:, bq:bq + 1])
                    nc.vector.tensor_add(out=rs[:], in0=rs[:], in1=xd2_sb[:, bq, :])
                    pT = psp.tile([cfg.F, P], F32, tag="pT")
                    nc.tensor.transpose(pT[:], rs[:], ident[:])
                    rsT = wp.tile([cfg.F, P], F32, tag="rsT")
                    nc.vector.tensor_copy(rsT[:], pT[:])
                    ps = psp.tile([P, 16], F32, tag="ps")
                    nc.tensor.matmul(out=ps[:], lhsT=xT_sb[:, bq * P:(bq + 1) * P],
                                     rhs=mm_sb[:, 0:16], start=True, stop=False)
                    nc.tensor.matmul(out=ps[:], lhsT=rsT[:], rhs=m4_sb[:],
                                     start=False, stop=True)
                    nc.scalar.activation(st_slot(st4, bq), ps[:], AF.Copy,
                                         scale=db_sb[:, bq:bq + 1])

                run_pass(xraw[0:cfg.WA_LEN, :], xraw[cfg.WB_OFF:NPAD, :], cfg.F,
                         epi4, chunk_hook=(lambda c: exchange_start(c, st4[c]),
                                           lambda c: exchange_finish(0, c, tw[0])))

                # ---- passes 3 and 2 ----
                def mk_epi(mcol, sts_prev, sts_new):
                    def epi(bq, R):
                        ps = psp.tile([P, 16], F32, tag="ps")
                        nc.tensor.matmul(out=ps[:],
                                         lhsT=xT_sb[:, bq * P:(bq + 1) * P],
                                         rhs=mm_sb[:, mcol:mcol + 16],
                                         start=True, stop=True)
                        # acc_full = R + prev_stage (self-loop)
                        accf = wp.tile([P, 16], F32, tag="accf")
                        nc.vector.tensor_add(out=accf[:], in0=R[:, 0:16],
                                             in1=st_slot(sts_prev, bq))
                        ta = wp.tile([P, 16], F32, tag="ta")
                        nc.scalar.activation(ta[:], ps[:], AF.Copy,
                                             scale=db_sb[:, bq:bq + 1])
                        tb = wp.tile([P, 16], F32, tag="tb")
                        nc.scalar.activation(tb[:], accf[:], AF.Copy,
                                             scale=d2_sb[:, bq:bq + 1])
                        nc.vector.tensor_add(out=st_slot(sts_new, bq),
                                             in0=ta[:], in1=tb[:])
                    return epi

                st3 = make_stages("s")
                run_pass(tw[0][0][:], tw[0][1][:], 16, mk_epi(16, st4, st3),
                         chunk_hook=(lambda c: exchange_start(c, st3[c]),
                                     lambda c: exchange_finish(1, c, tw[1])))

                st2 = make_stages("s")
                run_pass(tw[1][0][:], tw[1][1][:], 16, mk_epi(32, st3, st2),
                         chunk_hook=(lambda c: exchange_start(c, st2[c]),
                                     lambda c: exchange_finish(2, c, tw[0])))

                # ---- pass 1: final output ----
                st1 = make_stages("s")

                def epi1(bq, R):
                    accf = wp.tile([P, 16], F32, tag="accf")
                    nc.vector.tensor_add(out=accf[:], in0=R[:, 0:16],
                                         in1=st_slot(st2, bq))
                    t1 = wp.tile([P, 16], F32, tag="ta")
                    nc.scalar.activation(t1[:], accf[:], AF.Copy,
                                         scale=db_sb[:, bq:bq + 1])
                    nc.vector.tensor_add(out=st_slot(st1, bq), in0=t1[:],
                                         in1=bias_sb[:, bq, :])

                run_pass(tw[0][0][:], tw[0][1][:], 16, epi1)
                for c in range(NCH):
                    lo, hi = int(chunk_cls0[c]), int(chunk_cls0[c + 1])
                    nc.sync.dma_start(out[:, lo:hi, :], st1[c][:])

    return nc


# --------------------------------------------------------------------------
# entry point
# --------------------------------------------------------------------------

def _run(inputs, cfg: Cfg, runner=None, **run_kwargs):
    global LAST_RESULTS
    in_maps, layout, pos_kbj, pos2old = _host_prep(inputs, cfg)
    nc = _build_module(cfg, layout)
    nc.compile()
    if runner is None:
        res = run_bass_kernel_spmd(nc, in_maps, core_ids=list(range(cfg.NCORES)),
                                   **run_kwargs)
        LAST_RESULTS = res
        outs = res.results
    else:
        outs = runner(nc, in_maps)
    full = np.empty((cfg.NPAD, 16), np.float32)
    for k in range(cfg.NCORES):
        o = np.asarray(outs[k]["out"])  # [P, NBLK, 16]
        full[pos_kbj[k].reshape(-1)] = o.transpose(1, 0, 2).reshape(cfg.PER, 16)
    old2new = np.empty(cfg.N, np.int64)
    rmask = pos2old >= 0
    old2new[pos2old[rmask]] = np.nonzero(rmask)[0]
    return full[old2new]


def kernel(**inputs) -> np.ndarray:
    return _run(inputs, CFG)


# revision 17
# speedup vs baseline: 1.6491x; 1.0168x over previous
"""Trainium2 Bass kernel: DGCNN forward (4-layer GCN + Conv1d readout) on 8 NeuronCores.

Math restructuring (validated vs reference to 2e-7):
  With A = D^-1/2 (Adj + I) D^-1/2 and Mk / ck derived from the (tiny) weights,
    out = A(x M1 + A(x M2 + A(x M3 + A(x M4)))) + 1 c0 + v1 c1 + v2 c2 + v3 c3
  Passes aggregate tables T; self-loop contributions are added in the epilogue
  from SBUF-resident data (previous pass's stage), so gathers cover only real
  edges.  Pass 4 gathers the 64-wide dinv*x table (host pre-scaled); its
  epilogue projects through M4.  Passes 3/2/1 gather 16-wide tables.

Device strategy (graph-parallel over 8 cores):
  - dma_gather (SWDGE) is descriptor-rate-bound (~8.1ns/desc per queue, 4
    queues scale linearly), so the kernel minimizes descriptors and keeps all
    4 queues fed:
    * nodes are placed into 128-row blocks clustered by (degree, #window-A
      sources, #window-B sources) signature, cutting ELL padding to ~5%
    * blocks are dealt into 8-wide "classes" (one block per core) so the SPMD
      module has uniform shapes; class slot budgets are cross-core maxes
    * gather tile pool is 6 deep and ~36 gather calls/pass rotate over the 4
      SWDGE queues so descriptor generation runs ~4-way concurrent
  - int16 gather indices limit a window to 32768 rows; the 50176-row table is
    covered by two overlapping windows ([0,32768) and [17408,50176)); each
    dst's edges are split between windows inside its class budgets SA/SB.
  - The table is laid out in 4 exchange chunks ([17,15,13,4] blocks/core,
    region-aligned) so each AllGather output is a contiguous table range; a
    single DRAM->DRAM DMA restrides [rows,16] into the 256B-row table.  The
    first 3 chunk exchanges overlap the current pass's remaining gathers; only
    the small 4-block tail exchange sits on the pass boundary.
"""

import dataclasses
import numpy as np

import concourse.bass as bass
import concourse.bacc as bacc
import concourse.tile as tile
from concourse import mybir
from concourse.bass_utils import run_bass_kernel_spmd
from concourse.masks import make_identity

F32 = mybir.dt.float32
I16 = mybir.dt.int16
AF = mybir.ActivationFunctionType


@dataclasses.dataclass(frozen=True)
class Cfg:
    N: int = 50000          # real nodes
    F: int = 64             # features
    NCORES: int = 8
    P: int = 128
    NBLK: int = 49          # blocks (classes) per core
    NQ: int = 4             # SWDGE queues
    GT_BUFS: int = 4        # gather tile pool depth
    GRP_TARGET: float = 2.0  # classes per gather group (approx)

    # exchange chunks: (region, blocks-per-core) in PROCESSING order; regions
    # are the int16 window areas R0=[0,17408) R1=[17408,32768) R2=[32768,50176).
    # The overlap region (1) is processed/exchanged first since both gather
    # windows need it; the 4-block tail is the only boundary-critical piece.
    CHUNKS = ((1, 15), (0, 17), (2, 13), (2, 4))
    LAG: int = 5            # A-call emission lead over B-call+reduce
    CC_DELAY: int = 3       # groups between ccin DMA and collective trigger

    @property
    def PER(self):
        return self.NBLK * self.P

    @property
    def NPAD(self):
        return self.NCORES * self.PER

    @property
    def WA_LEN(self):
        return 32768

    @property
    def WB_OFF(self):
        return self.NPAD - 32768


CFG = Cfg()

LAST_RESULTS = None


# --------------------------------------------------------------------------
# host preprocessing
# --------------------------------------------------------------------------

def _host_prep(inputs, cfg: Cfg):
    x = np.asarray(inputs["x"], np.float32)
    ei = np.asarray(inputs["edge_index"]).astype(np.int64)
    W = [np.asarray(inputs[f"W{i}"], np.float64) for i in range(4)]
    b = [np.asarray(inputs[f"b{i}"], np.float64) for i in range(4)]
    conv_w = np.asarray(inputs["conv_w"], np.float64)
    conv_b = np.asarray(inputs["conv_b"], np.float64)

    n = x.shape[0]
    assert n == cfg.N and x.shape[1] == cfg.F
    P, PER, NPAD, NBLK, NC = cfg.P, cfg.PER, cfg.NPAD, cfg.NBLK, cfg.NCORES

    src0, dst0 = ei[0], ei[1]           # real edges only; self-loops in epilogue
    E0 = len(src0)
    deg = np.bincount(dst0, minlength=n).astype(np.float64) + 1.0  # incl self
    dinv = 1.0 / np.sqrt(deg)

    # ---- weight-derived small matrices ----
    Cw = [conv_w[:, 0:64], conv_w[:, 64:128], conv_w[:, 128:192], conv_w[:, 192:193]]
    M1 = W[0] @ Cw[0].T
    M2 = W[0] @ W[1] @ Cw[1].T
    M3 = W[0] @ W[1] @ W[2] @ Cw[2].T
    M4 = W[0] @ W[1] @ W[2] @ W[3] @ Cw[3].T
    c0 = b[0] @ Cw[0].T + b[1] @ Cw[1].T + b[2] @ Cw[2].T + b[3] @ Cw[3].T + conv_b
    c1 = (b[0] @ W[1]) @ Cw[1].T + (b[1] @ W[2]) @ Cw[2].T + (b[2] @ W[3]) @ Cw[3].T
    c2 = (b[0] @ W[1] @ W[2]) @ Cw[2].T + (b[1] @ W[2] @ W[3]) @ Cw[3].T
    c3 = (b[0] @ W[1] @ W[2] @ W[3]) @ Cw[3].T

    def aggv(v):
        o = np.zeros(n)
        np.add.at(o, dst0, (v * dinv)[src0])
        o += v * dinv
        return o * dinv

    v1 = aggv(np.ones(n))
    v2 = aggv(v1)
    v3 = aggv(v2)
    bias = (np.outer(np.ones(n), c0) + np.outer(v1, c1)
            + np.outer(v2, c2) + np.outer(v3, c3))  # [n, 16]

    # ---- region assignment + signatures ----
    # region position ranges (block aligned): R0 [0,17408) R1 [17408,32768)
    # R2 [32768,50176); pads: 2 at end of R0, 2 at end of R1, 172 end of R2
    RSTART = np.array([0, 17408, 32768, NPAD])
    RCAP = np.array([17408 - 2, 15360 - 2, 17408 - 172])
    assert RCAP.sum() == n
    region_of_node = np.repeat(np.arange(3), RCAP)  # node id order
    sreg = region_of_node[src0]
    gdeg = np.bincount(dst0, minlength=n).astype(np.int64)
    nA_n = np.bincount(dst0, weights=(sreg == 0), minlength=n).astype(np.int64)
    nB_n = np.bincount(dst0, weights=(sreg == 2), minlength=n).astype(np.int64)

    # chunk layout: per core, chunks of blocks; classes indexed 0..NBLK-1
    chunks = cfg.CHUNKS
    nb_of_chunk = [c[1] for c in chunks]
    assert sum(nb_of_chunk) == NBLK
    rcursor = {0: 0, 1: 17408, 2: 32768}
    chunk_start_pos = []
    for r, nb in chunks:
        chunk_start_pos.append(rcursor[r])
        rcursor[r] += NC * nb * P
    chunk_start_pos = np.array(chunk_start_pos + [NPAD])  # last entry unused
    chunk_cls0 = np.concatenate([[0], np.cumsum(nb_of_chunk)])

    # per region: sort real nodes by signature, form blocks, rank into classes
    pos_of_node = np.full(n, -1, np.int64)
    cls_cost = np.zeros(NBLK, np.int64)
    for r in range(3):
        nodes = np.nonzero(region_of_node == r)[0]
        k = np.lexsort((nB_n[nodes], nA_n[nodes], gdeg[nodes] // 4))
        nodes = nodes[k]
        nblocks_r = (RSTART[r + 1] - RSTART[r]) // P
        ncls_r = nblocks_r // NC
        # blocks of 128 consecutive sorted nodes (pads fill the tail)
        nfull = len(nodes)
        # block cost = max(gdeg, nA+nB) over its nodes (pads contribute 0)
        bcost = np.zeros(nblocks_r, np.int64)
        bi = np.arange(nfull) // P
        np.maximum.at(bcost, bi, np.maximum(gdeg[nodes], nA_n[nodes] + nB_n[nodes]))
        # class i = NC CONSECUTIVE blocks in signature order (keeps budgets
        # tight); rank classes by cost only to route small ones to the tail
        # chunk; per-core load is Sum(class budgets) regardless (1 blk/core).
        ccost = bcost.reshape(ncls_r, NC).max(axis=1)
        cls_rank = np.argsort(-ccost, kind="stable")  # region-class in cost order
        rchunks = [ci for ci, c in enumerate(chunks) if c[0] == r]
        cls_slots = []  # (chunk_idx, slot_in_chunk); tail chunk listed last
        for ci in rchunks:
            cls_slots += [(ci, s) for s in range(nb_of_chunk[ci])]
        assert len(cls_slots) == ncls_r
        for i in range(ncls_r):
            ci, slot = cls_slots[i]
            cls_id = chunk_cls0[ci] + slot
            rc = cls_rank[i]                      # region-class index
            for kc in range(NC):
                blk = rc * NC + kc
                base = (chunk_start_pos[ci] + kc * nb_of_chunk[ci] * P + slot * P)
                lo, hi = blk * P, min((blk + 1) * P, nfull)
                if hi > lo:
                    pos_of_node[nodes[lo:hi]] = base + np.arange(hi - lo)
                cls_cost[cls_id] = max(cls_cost[cls_id], bcost[blk])

    assert (pos_of_node >= 0).all()
    pos2old = np.full(NPAD, -1, np.int64)
    pos2old[pos_of_node] = np.arange(n)

    # position -> (core, local block b, j)
    pos = np.arange(NPAD)
    core_of_pos = np.zeros(NPAD, np.int64)
    cls_of_pos = np.zeros(NPAD, np.int64)
    for ci, (r, nb) in enumerate(chunks):
        s = chunk_start_pos[ci]
        e = s + NC * nb_of_chunk[ci] * P
        rel = pos[s:e] - s
        core_of_pos[s:e] = rel // (nb * P)
        cls_of_pos[s:e] = chunk_cls0[ci] + (rel % (nb * P)) // P
    j_of_pos = pos % P

    # dummy zero rows: a pad position in window A and one in window B
    pad_pos = np.nonzero(pos2old < 0)[0]
    zA = int(pad_pos[pad_pos < cfg.WA_LEN][-1])
    zB = int(pad_pos[pad_pos >= cfg.WB_OFF][-1])
    assert zA != zB

    # ---- per-edge window split with per-class budgets ----
    s_pos = pos_of_node[src0]
    d_pos = pos_of_node[dst0]
    eo = np.argsort(d_pos, kind="stable")
    s_s = s_pos[eo]
    d_s = d_pos[eo]
    starts = np.searchsorted(d_s, np.arange(NPAD + 1))

    isA = s_s < cfg.WB_OFF
    isB = s_s >= cfg.WA_LEN
    isF = ~(isA | isB)
    nAo = np.bincount(d_s, weights=isA, minlength=NPAD).astype(np.int64)
    nBo = np.bincount(d_s, weights=isB, minlength=NPAD).astype(np.int64)
    nf = np.bincount(d_s, weights=isF, minlength=NPAD).astype(np.int64)
    tot = nAo + nBo + nf

    cp = cls_of_pos
    mA = np.zeros(NBLK, np.int64); np.maximum.at(mA, cp, nAo)
    mB = np.zeros(NBLK, np.int64); np.maximum.at(mB, cp, nBo)
    mT = np.zeros(NBLK, np.int64); np.maximum.at(mT, cp, tot)
    M = np.maximum(mT, mA + mB)
    SA = np.clip((M + 1) // 2, mA, M - mB)
    SB = M - SA
    SAp = SA[cp]
    SBp = SB[cp]
    nA = np.clip(tot - SBp, nAo, np.minimum(nAo + nf, SAp))

    cFex = np.concatenate([[0], np.cumsum(isF)])
    frank = cFex[:-1] - cFex[starts[d_s]]
    goA = isA | (isF & (frank < (nA - nAo)[d_s]))
    goB = ~goA
    cAex = np.concatenate([[0], np.cumsum(goA)])
    slotA = cAex[:-1] - cAex[starts[d_s]]
    cBex = np.concatenate([[0], np.cumsum(goB)])
    slotB = cBex[:-1] - cBex[starts[d_s]]
    nB_ = tot - nA
    assert (nA <= SAp).all() and (nB_ <= SBp).all()
    assert (SA + SB > 0).all()

    # ---- groups: classes within each chunk, balanced by slots ----
    groups = []          # list of list of class ids
    grp_chunk = []       # chunk index of each group
    for ci, (r, nb) in enumerate(chunks):
        cls_list = list(range(chunk_cls0[ci], chunk_cls0[ci] + nb))
        ng = int(np.ceil(nb / cfg.GRP_TARGET))
        # greedy balance by SA+SB
        order_d = sorted(cls_list, key=lambda c: -(SA[c] + SB[c]))
        gsets = [[] for _ in range(ng)]
        gsum = [0] * ng
        for c in order_d:
            q = min(range(ng), key=lambda i: (gsum[i], i))
            gsets[q].append(c)
            gsum[q] += SA[c] + SB[c]
        for g in gsets:
            groups.append(sorted(g))
            grp_chunk.append(ci)
    NGRP = len(groups)

    # slot offsets per class within its group's A/B gathers
    oa = np.zeros(NBLK, np.int64)
    ob = np.zeros(NBLK, np.int64)
    grp_of = np.zeros(NBLK, np.int64)
    SAg = np.zeros(NGRP, np.int64)
    SBg = np.zeros(NGRP, np.int64)
    for q, bl in enumerate(groups):
        offa = 0
        for bq in bl:
            oa[bq] = offa
            offa += SA[bq]
            grp_of[bq] = q
        offb = 0
        for bq in bl:
            ob[bq] = offb
            offb += SB[bq]
        SAg[q] = offa
        SBg[q] = offb

    colA0 = np.zeros(NGRP, np.int64)
    colB0 = np.zeros(NGRP, np.int64)
    cur = 0
    for q in range(NGRP):
        colA0[q] = cur
        cur += int(SAg[q]) * P // 16
        colB0[q] = cur
        cur += int(SBg[q]) * P // 16
    idxcols = int(cur)

    # ---- build per-core idx tensors ----
    zA_rel = np.int16(zA)
    zB_rel = np.int16(zB - cfg.WB_OFF)
    idx_np = np.empty((NC, 128, idxcols), np.int16)
    for q in range(NGRP):
        idx_np[:, :, colA0[q]:colA0[q] + int(SAg[q]) * P // 16] = zA_rel
        idx_np[:, :, colB0[q]:colB0[q] + int(SBg[q]) * P // 16] = zB_rel

    e_core = core_of_pos[d_s]
    e_cls = cls_of_pos[d_s]
    e_j = j_of_pos[d_s]
    e_q = grp_of[e_cls]
    posA = (oa[e_cls] + slotA) * P + e_j
    colA = colA0[e_q] + posA // 16
    rowA = posA % 16
    posB = (ob[e_cls] + slotB) * P + e_j
    colB = colB0[e_q] + posB // 16
    rowB = posB % 16
    valA = s_s.astype(np.int16)
    valB = (s_s - cfg.WB_OFF).astype(np.int16)
    for k in range(NC):
        mk = e_core == k
        mAk = mk & goA
        mBk = mk & goB
        for r in range(8):
            idx_np[k, rowA[mAk] + 16 * r, colA[mAk]] = valA[mAk]
            idx_np[k, rowB[mBk] + 16 * r, colB[mBk]] = valB[mBk]

    # ---- dense per-core arrays ----
    rmask = pos2old >= 0
    dinv_pos = np.ones(NPAD, np.float32)
    dinv_pos[rmask] = dinv[pos2old[rmask]].astype(np.float32)
    x_pos = np.zeros((NPAD, cfg.F), np.float32)
    x_pos[rmask] = x[pos2old[rmask]]
    bias_pos = np.zeros((NPAD, 16), np.float32)
    bias_pos[rmask] = bias[pos2old[rmask]].astype(np.float32)

    xraw_t = x_pos * dinv_pos[:, None]            # pass-4 table: dinv*x
    # per-core [j, b] layouts: position of (core k, cls b, j)
    pos_kbj = np.zeros((NC, NBLK, P), np.int64)
    for ci, (r, nb) in enumerate(chunks):
        for kc in range(NC):
            for s in range(nb):
                base = chunk_start_pos[ci] + kc * nb * P + s * P
                pos_kbj[kc, chunk_cls0[ci] + s] = base + np.arange(P)

    in_maps = []
    mmats = np.ascontiguousarray(np.concatenate([M3, M2, M1], axis=1).astype(np.float32))
    m4 = np.ascontiguousarray(M4.astype(np.float32))
    for k in range(NC):
        pk = pos_kbj[k]                            # [NBLK, P] positions
        db = dinv_pos[pk].T.astype(np.float32)     # [P, NBLK]
        d2 = (db * db).astype(np.float32)
        xTloc = np.ascontiguousarray(
            x_pos[pk.reshape(-1)].T)               # [F, PER] raw x
        xd2 = np.ascontiguousarray(x_pos[pk].transpose(1, 0, 2)
                                   * (dinv_pos[pk] ** 2).T[:, :, None]).astype(np.float32)
        bias_blk = np.ascontiguousarray(bias_pos[pk].transpose(1, 0, 2)).astype(np.float32)
        in_maps.append(dict(
            xraw=xraw_t,
            idx=np.ascontiguousarray(idx_np[k]),
            xT=xTloc,
            db=np.ascontiguousarray(db),
            d2=np.ascontiguousarray(d2),
            xd2=xd2,
            bias_blk=bias_blk,
            mmats=mmats,
            m4=m4,
        ))

    layout = dict(SA=SA, SB=SB, groups=groups, grp_chunk=grp_chunk, oa=oa, ob=ob,
                  SAg=SAg, SBg=SBg, colA0=colA0, colB0=colB0, idxcols=idxcols,
                  chunks=chunks, chunk_start_pos=chunk_start_pos,
                  chunk_cls0=chunk_cls0)
    return in_maps, layout, pos_kbj, pos2old


# --------------------------------------------------------------------------
# device module
# --------------------------------------------------------------------------

def _build_module(cfg: Cfg, layout):
    P, PER, NPAD, NBLK, NC = cfg.P, cfg.PER, cfg.NPAD, cfg.NBLK, cfg.NCORES
    SA, SB = layout["SA"], layout["SB"]
    groups, grp_chunk = layout["groups"], layout["grp_chunk"]
    oa, ob = layout["oa"], layout["ob"]
    SAg, SBg = layout["SAg"], layout["SBg"]
    colA0, colB0 = layout["colA0"], layout["colB0"]
    idxcols = layout["idxcols"]
    chunks = layout["chunks"]
    chunk_start_pos = layout["chunk_start_pos"]
    chunk_cls0 = layout["chunk_cls0"]
    NGRP = len(groups)
    NCH = len(chunks)

    nc = bacc.Bacc("TRN2", target_bir_lowering=False, debug=False, num_devices=NC,
                   num_swdge_queues=cfg.NQ, dynamic_dma_scratch_size=16384)

    xraw = nc.dram_tensor("xraw", [NPAD, cfg.F], F32, kind="ExternalInput").ap()
    idx = nc.dram_tensor("idx", [128, idxcols], I16, kind="ExternalInput").ap()
    xT = nc.dram_tensor("xT", [cfg.F, PER], F32, kind="ExternalInput").ap()
    db_in = nc.dram_tensor("db", [P, NBLK], F32, kind="ExternalInput").ap()
    d2_in = nc.dram_tensor("d2", [P, NBLK], F32, kind="ExternalInput").ap()
    xd2_in = nc.dram_tensor("xd2", [P, NBLK, cfg.F], F32, kind="ExternalInput").ap()
    bias_in = nc.dram_tensor("bias_blk", [P, NBLK, 16], F32, kind="ExternalInput").ap()
    mmats = nc.dram_tensor("mmats", [cfg.F, 48], F32, kind="ExternalInput").ap()
    m4 = nc.dram_tensor("m4", [cfg.F, 16], F32, kind="ExternalInput").ap()
    out = nc.dram_tensor("out", [P, NBLK, 16], F32, kind="ExternalOutput").ap()

    with tile.TileContext(nc) as tc:
        with (
            tc.tile_pool(name="const", bufs=1) as cp,
            tc.tile_pool(name="dram", bufs=1, space="DRAM") as dp,
        ):
            idx_sb = cp.tile([128, idxcols], I16)
            nc.sync.dma_start(idx_sb[:], idx)
            xT_sb = cp.tile([cfg.F, PER], F32)
            nc.sync.dma_start(xT_sb[:], xT)
            mm_sb = cp.tile([cfg.F, 48], F32)
            nc.sync.dma_start(mm_sb[:], mmats)
            m4_sb = cp.tile([cfg.F, 16], F32)
            nc.sync.dma_start(m4_sb[:], m4)
            db_sb = cp.tile([P, NBLK], F32)
            nc.sync.dma_start(db_sb[:], db_in)
            d2_sb = cp.tile([P, NBLK], F32)
            nc.sync.dma_start(d2_sb[:], d2_in)
            xd2_sb = cp.tile([P, NBLK, cfg.F], F32)
            nc.sync.dma_start(xd2_sb[:], xd2_in)
            bias_sb = cp.tile([P, NBLK, 16], F32)
            nc.sync.dma_start(bias_sb[:], bias_in)
            ident = cp.tile([P, P], F32)
            make_identity(nc, ident[:])

            # each generation is a (w1, w2) pair: w1 = table rows [0,32768),
            # w2 = rows [17408, 50176); chunk exchanges write into one or both
            tw = [(dp.tile([cfg.WA_LEN, cfg.F], F32, name=f"tw{i}_1"),
                   dp.tile([cfg.WA_LEN, cfg.F], F32, name=f"tw{i}_2"))
                  for i in range(2)]
            ccin = [dp.tile([nb * P, 16], F32, name=f"ccin{c}")
                    for c, (r, nb) in enumerate(chunks)]
            ccout = [[dp.tile([NC * nb * P, 16], F32, addr_space="Shared",
                              name=f"ccout{p}_{c}")
                      for c, (r, nb) in enumerate(chunks)] for p in range(3)]

            with (
                tc.tile_pool(name="gatha", bufs=cfg.LAG + 3) as gpa,
                tc.tile_pool(name="gathb", bufs=7) as gpb,
                tc.tile_pool(name="work", bufs=4) as wp,
                tc.tile_pool(name="stage", bufs=2) as sp,
                tc.tile_pool(name="psum", bufs=4, space="PSUM") as psp,
            ):
                # greedy queue schedule: call (in EMISSION order) -> least-loaded
                emit_order = []
                for step in range(NGRP + cfg.LAG):
                    if step < NGRP:
                        emit_order.append((step, "A", int(SAg[step])))
                    h = step - cfg.LAG
                    if h >= 0:
                        emit_order.append((h, "B", int(SBg[h])))
                # STRICT round-robin in emission order: each queue runs one
                # descgen at a time and the gpsimd engine dispatches in order,
                # so consecutive same-queue calls would head-of-line block.
                qsched = {}
                rr = [0]
                for q, part, sz in emit_order:
                    if sz:
                        qsched[(q, part)] = rr[0] % cfg.NQ
                        rr[0] += 1

                def make_stages(tag):
                    return [sp.tile([P, nb, 16], F32, tag=f"{tag}{c}",
                                    name=f"st_{tag}{c}")
                            for c, (r, nb) in enumerate(chunks)]

                def st_slot(sts, bq):
                    for c in range(NCH):
                        if bq < chunk_cls0[c + 1]:
                            return sts[c][:, bq - chunk_cls0[c], :]
                    raise AssertionError

                def exchange_start(c, st_tile):
                    nc.sync.dma_start(
                        ccin[c][:].rearrange("(b p) f -> p b f", p=P), st_tile[:])

                def exchange_finish(p, c, target):
                    # deferred so the gpsimd-resident collective trigger never
                    # blocks gather dispatch waiting on the ccin DMA
                    r, nb = chunks[c]
                    w1, w2 = target
                    nc.gpsimd.collective_compute(
                        "AllGather", mybir.AluOpType.bypass,
                        replica_groups=[list(range(NC))],
                        ins=[ccin[c][:]], outs=[ccout[p][c][:]],
                    )
                    s = int(chunk_start_pos[c])
                    rows = NC * nb * P
                    if s < cfg.WA_LEN:                    # overlaps window 1
                        hi = min(s + rows, cfg.WA_LEN)
                        nc.scalar.dma_start(
                            w1[s:hi, 0:16], ccout[p][c][0:hi - s, :])
                    if s + rows > cfg.WB_OFF:             # overlaps window 2
                        lo = max(s, cfg.WB_OFF)
                        nc.scalar.dma_start(
                            w2[lo - cfg.WB_OFF:s + rows - cfg.WB_OFF, 0:16],
                            ccout[p][c][lo - s:rows, :])

                last_of_chunk = {}
                for q in range(NGRP):
                    last_of_chunk[grp_chunk[q]] = q

                def run_pass(winA, winB, width, epi, chunk_hook=None):
                    # A-gathers issue LAG groups ahead of B-gathers+reduces so
                    # queue FIFOs stay busy across the pass boundary (A only
                    # depends on the first two chunk exchanges of the prior
                    # pass, B on all four).
                    gtA = {}
                    gtB = {}

                    def emit_A(q):
                        sag = int(SAg[q])
                        if not sag:
                            return
                        t = gpa.tile([P, sag, cfg.F], F32, tag="gtA", name="gtA")
                        gtA[q] = t
                        nc.gpsimd.dma_gather(
                            out_ap=t[:],
                            in_ap=winA,
                            idxs_ap=idx_sb[:, int(colA0[q]):int(colA0[q]) + sag * P // 16],
                            num_idxs=sag * P,
                            num_idxs_reg=sag * P,
                            elem_size=cfg.F,
                            single_packet=False,
                            queue_num=qsched[(q, "A")],
                        )

                    def emit_B(q):
                        sbg = int(SBg[q])
                        if not sbg:
                            return
                        t = gpb.tile([P, sbg, cfg.F], F32, tag="gtB", name="gtB")
                        gtB[q] = t
                        nc.gpsimd.dma_gather(
                            out_ap=t[:],
                            in_ap=winB,
                            idxs_ap=idx_sb[:, int(colB0[q]):int(colB0[q]) + sbg * P // 16],
                            num_idxs=sbg * P,
                            num_idxs_reg=sbg * P,
                            elem_size=cfg.F,
                            single_packet=False,
                            queue_num=qsched[(q, "B")],
                        )

                    def emit_reduces(q):
                        for bq in groups[q]:
                            acc = wp.tile([P, cfg.F], F32, tag="acc")
                            wrote = False
                            if SA[bq]:
                                a0, a1 = int(oa[bq]), int(oa[bq] + SA[bq])
                                nc.vector.reduce_sum(
                                    out=acc[:, 0:width],
                                    in_=gtA[q][:, a0:a1, 0:width].rearrange("p s f -> p f s"),
                                    axis=mybir.AxisListType.X,
                                )
                                wrote = True
                            if SB[bq]:
                                b0_, b1_ = int(ob[bq]), int(ob[bq] + SB[bq])
                                if wrote:
                                    acc2 = wp.tile([P, cfg.F], F32, tag="acc2")
                                    nc.vector.reduce_sum(
                                        out=acc2[:, 0:width],
                                        in_=gtB[q][:, b0_:b1_, 0:width].rearrange("p s f -> p f s"),
                                        axis=mybir.AxisListType.X,
                                    )
                                    nc.vector.tensor_add(
                                        out=acc[:, 0:width], in0=acc[:, 0:width],
                                        in1=acc2[:, 0:width])
                                else:
                                    nc.vector.reduce_sum(
                                        out=acc[:, 0:width],
                                        in_=gtB[q][:, b0_:b1_, 0:width].rearrange("p s f -> p f s"),
                                        axis=mybir.AxisListType.X,
                                    )
                            epi(bq, acc)

                    pending_finish = []
                    for step in range(NGRP + cfg.LAG + cfg.CC_DELAY):
                        if step < NGRP:
                            emit_A(step)
                        h = step - cfg.LAG
                        if chunk_hook is not None:
                            for dc, due in list(pending_finish):
                                if step - due >= cfg.CC_DELAY or h >= NGRP - 1:
                                    chunk_hook[1](dc)
                                    pending_finish.remove((dc, due))
                        if 0 <= h < NGRP:
                            emit_B(h)
                            emit_reduces(h)
                            if chunk_hook is not None and h == last_of_chunk[grp_chunk[h]]:
                                chunk_hook[0](grp_chunk[h])
                                pending_finish.append((grp_chunk[h], step))

                # ---- pass 4: gather dinv*x (64-wide), project via M4 ----
                st4 = make_stages("s")

                def epi4(bq, R):
                    rs = wp.tile([P, cfg.F], F32, tag="rs")
                    # rs = db*R + d2*x_own   (u such that st = db*(x@M3) + db*u@M4)
                    nc.scalar.activation(rs[:], R[:], AF.Copy, scale=db_sb[:, bq:bq + 1])
                    nc.vector.tensor_add(out=rs[:], in0=rs[:], in1=xd2_sb[:, bq, :])
                    pT = psp.tile([cfg.F, P], F32, tag="pT")
                    nc.tensor.transpose(pT[:], rs[:], ident[:])
                    rsT = wp.tile([cfg.F, P], F32, tag="rsT")
                    nc.vector.tensor_copy(rsT[:], pT[:])
                    ps = psp.tile([P, 16], F32, tag="ps")
                    nc.tensor.matmul(out=ps[:], lhsT=xT_sb[:, bq * P:(bq + 1) * P],
                                     rhs=mm_sb[:, 0:16], start=True, stop=False)
                    nc.tensor.matmul(out=ps[:], lhsT=rsT[:], rhs=m4_sb[:],
                                     start=False, stop=True)
                    nc.scalar.activation(st_slot(st4, bq), ps[:], AF.Copy,
                                         scale=db_sb[:, bq:bq + 1])

                run_pass(xraw[0:cfg.WA_LEN, :], xraw[cfg.WB_OFF:NPAD, :], cfg.F,
                         epi4, chunk_hook=(lambda c: exchange_start(c, st4[c]),
                                           lambda c: exchange_finish(0, c, tw[0])))

                # ---- passes 3 and 2 ----
                def mk_epi(mcol, sts_prev, sts_new):
                    def epi(bq, R):
                        ps = psp.tile([P, 16], F32, tag="ps")
                        nc.tensor.matmul(out=ps[:],
                                         lhsT=xT_sb[:, bq * P:(bq + 1) * P],
                                         rhs=mm_sb[:, mcol:mcol + 16],
                                         start=True, stop=True)
                        # acc_full = R + prev_stage (self-loop)
                        accf = wp.tile([P, 16], F32, tag="accf")
                        nc.vector.tensor_add(out=accf[:], in0=R[:, 0:16],
                                             in1=st_slot(sts_prev, bq))
                        ta = wp.tile([P, 16], F32, tag="ta")
                        nc.scalar.activation(ta[:], ps[:], AF.Copy,
                                             scale=db_sb[:, bq:bq + 1])
                        tb = wp.tile([P, 16], F32, tag="tb")
                        nc.scalar.activation(tb[:], accf[:], AF.Copy,
                                             scale=d2_sb[:, bq:bq + 1])
                        nc.vector.tensor_add(out=st_slot(sts_new, bq),
                                             in0=ta[:], in1=tb[:])
                    return epi

                st3 = make_stages("s")
                run_pass(tw[0][0][:], tw[0][1][:], 16, mk_epi(16, st4, st3),
                         chunk_hook=(lambda c: exchange_start(c, st3[c]),
                                     lambda c: exchange_finish(1, c, tw[1])))

                st2 = make_stages("s")
                run_pass(tw[1][0][:], tw[1][1][:], 16, mk_epi(32, st3, st2),
                         chunk_hook=(lambda c: exchange_start(c, st2[c]),
                                     lambda c: exchange_finish(2, c, tw[0])))

                # ---- pass 1: final output ----
                st1 = make_stages("s")

                def epi1(bq, R):
                    accf = wp.tile([P, 16], F32, tag="accf")
                    nc.vector.tensor_add(out=accf[:], in0=R[:, 0:16],
                                         in1=st_slot(st2, bq))
                    t1 = wp.tile([P, 16], F32, tag="ta")
                    nc.scalar.activation(t1[:], accf[:], AF.Copy,
                                         scale=db_sb[:, bq:bq + 1])
                    nc.vector.tensor_add(out=st_slot(st1, bq), in0=t1[:],
                                         in1=bias_sb[:, bq, :])

                run_pass(tw[0][0][:], tw[0][1][:], 16, epi1)
                for c in range(NCH):
                    lo, hi = int(chunk_cls0[c]), int(chunk_cls0[c + 1])
                    nc.sync.dma_start(out[:, lo:hi, :], st1[c][:])

    return nc


# --------------------------------------------------------------------------
# entry point
# --------------------------------------------------------------------------

def _run(inputs, cfg: Cfg, runner=None, **run_kwargs):
    global LAST_RESULTS
    in_maps, layout, pos_kbj, pos2old = _host_prep(inputs, cfg)
    nc = _build_module(cfg, layout)
    nc.compile()
    if runner is None:
        res = run_bass_kernel_spmd(nc, in_maps, core_ids=list(range(cfg.NCORES)),
                                   **run_kwargs)
        LAST_RESULTS = res
        outs = res.results
    else:
        outs = runner(nc, in_maps)
    full = np.empty((cfg.NPAD, 16), np.float32)
    for k in range(cfg.NCORES):
        o = np.asarray(outs[k]["out"])  # [P, NBLK, 16]
        full[pos_kbj[k].reshape(-1)] = o.transpose(1, 0, 2).reshape(cfg.PER, 16)
    old2new = np.empty(cfg.N, np.int64)
    rmask = pos2old >= 0
    old2new[pos2old[rmask]] = np.nonzero(rmask)[0]
    return full[old2new]


def kernel(**inputs) -> np.ndarray:
    return _run(inputs, CFG)


# revision 18
# speedup vs baseline: 1.6585x; 1.0057x over previous
"""Trainium2 Bass kernel: DGCNN forward (4-layer GCN + Conv1d readout) on 8 NeuronCores.

Math restructuring (validated vs reference to 2e-7):
  With A = D^-1/2 (Adj + I) D^-1/2 and Mk / ck derived from the (tiny) weights,
    out = A(x M1 + A(x M2 + A(x M3 + A(x M4)))) + 1 c0 + v1 c1 + v2 c2 + v3 c3
  Passes aggregate tables T; self-loop contributions are added in the epilogue
  from SBUF-resident data (previous pass's stage), so gathers cover only real
  edges.  Pass 4 gathers the 64-wide dinv*x table (host pre-scaled); its
  epilogue projects through M4.  Passes 3/2/1 gather 16-wide tables.

Device strategy (graph-parallel over 8 cores):
  - dma_gather (SWDGE) is descriptor-rate-bound (~8.1ns/desc per queue, 4
    queues scale linearly), so the kernel minimizes descriptors and keeps all
    4 queues fed:
    * nodes are placed into 128-row blocks clustered by (degree, #window-A
      sources, #window-B sources) signature, cutting ELL padding to ~5%
    * blocks are dealt into 8-wide "classes" (one block per core) so the SPMD
      module has uniform shapes; class slot budgets are cross-core maxes
    * gather tile pool is 6 deep and ~36 gather calls/pass rotate over the 4
      SWDGE queues so descriptor generation runs ~4-way concurrent
  - int16 gather indices limit a window to 32768 rows; the 50176-row table is
    covered by two overlapping windows ([0,32768) and [17408,50176)); each
    dst's edges are split between windows inside its class budgets SA/SB.
  - The table is laid out in 4 exchange chunks ([17,15,13,4] blocks/core,
    region-aligned) so each AllGather output is a contiguous table range; a
    single DRAM->DRAM DMA restrides [rows,16] into the 256B-row table.  The
    first 3 chunk exchanges overlap the current pass's remaining gathers; only
    the small 4-block tail exchange sits on the pass boundary.
"""

import dataclasses
import numpy as np

import concourse.bass as bass
import concourse.bacc as bacc
import concourse.tile as tile
from concourse import mybir
from concourse.bass_utils import run_bass_kernel_spmd
from concourse.masks import make_identity

F32 = mybir.dt.float32
I16 = mybir.dt.int16
AF = mybir.ActivationFunctionType


@dataclasses.dataclass(frozen=True)
class Cfg:
    N: int = 50000          # real nodes
    F: int = 64             # features
    NCORES: int = 8
    P: int = 128
    NBLK: int = 49          # blocks (classes) per core
    NQ: int = 4             # SWDGE queues
    GT_BUFS: int = 4        # gather tile pool depth
    GRP_TARGET: float = 2.0  # classes per gather group (approx)

    # exchange chunks: (region, blocks-per-core) in PROCESSING order; regions
    # are the int16 window areas R0=[0,17408) R1=[17408,32768) R2=[32768,50176).
    # The overlap region (1) is processed/exchanged first since both gather
    # windows need it; the 4-block tail is the only boundary-critical piece.
    CHUNKS = ((1, 15), (0, 17), (2, 13), (2, 4))
    LAG: int = 5            # A-call emission lead over B-call+reduce
    CC_DELAY: int = 3       # groups between ccin DMA and collective trigger

    @property
    def PER(self):
        return self.NBLK * self.P

    @property
    def NPAD(self):
        return self.NCORES * self.PER

    @property
    def WA_LEN(self):
        return 32768

    @property
    def WB_OFF(self):
        return self.NPAD - 32768


CFG = Cfg()

LAST_RESULTS = None


# --------------------------------------------------------------------------
# host preprocessing
# --------------------------------------------------------------------------

def _host_prep(inputs, cfg: Cfg):
    x = np.asarray(inputs["x"], np.float32)
    ei = np.asarray(inputs["edge_index"]).astype(np.int64)
    W = [np.asarray(inputs[f"W{i}"], np.float64) for i in range(4)]
    b = [np.asarray(inputs[f"b{i}"], np.float64) for i in range(4)]
    conv_w = np.asarray(inputs["conv_w"], np.float64)
    conv_b = np.asarray(inputs["conv_b"], np.float64)

    n = x.shape[0]
    assert n == cfg.N and x.shape[1] == cfg.F
    P, PER, NPAD, NBLK, NC = cfg.P, cfg.PER, cfg.NPAD, cfg.NBLK, cfg.NCORES

    src0, dst0 = ei[0], ei[1]           # real edges only; self-loops in epilogue
    E0 = len(src0)
    deg = np.bincount(dst0, minlength=n).astype(np.float64) + 1.0  # incl self
    dinv = 1.0 / np.sqrt(deg)

    # ---- weight-derived small matrices ----
    Cw = [conv_w[:, 0:64], conv_w[:, 64:128], conv_w[:, 128:192], conv_w[:, 192:193]]
    M1 = W[0] @ Cw[0].T
    M2 = W[0] @ W[1] @ Cw[1].T
    M3 = W[0] @ W[1] @ W[2] @ Cw[2].T
    M4 = W[0] @ W[1] @ W[2] @ W[3] @ Cw[3].T
    c0 = b[0] @ Cw[0].T + b[1] @ Cw[1].T + b[2] @ Cw[2].T + b[3] @ Cw[3].T + conv_b
    c1 = (b[0] @ W[1]) @ Cw[1].T + (b[1] @ W[2]) @ Cw[2].T + (b[2] @ W[3]) @ Cw[3].T
    c2 = (b[0] @ W[1] @ W[2]) @ Cw[2].T + (b[1] @ W[2] @ W[3]) @ Cw[3].T
    c3 = (b[0] @ W[1] @ W[2] @ W[3]) @ Cw[3].T

    def aggv(v):
        o = np.zeros(n)
        np.add.at(o, dst0, (v * dinv)[src0])
        o += v * dinv
        return o * dinv

    v1 = aggv(np.ones(n))
    v2 = aggv(v1)
    v3 = aggv(v2)
    bias = (np.outer(np.ones(n), c0) + np.outer(v1, c1)
            + np.outer(v2, c2) + np.outer(v3, c3))  # [n, 16]

    # ---- region assignment + signatures ----
    # region position ranges (block aligned): R0 [0,17408) R1 [17408,32768)
    # R2 [32768,50176); pads: 2 at end of R0, 2 at end of R1, 172 end of R2
    RSTART = np.array([0, 17408, 32768, NPAD])
    RCAP = np.array([17408 - 2, 15360 - 2, 17408 - 172])
    assert RCAP.sum() == n
    region_of_node = np.repeat(np.arange(3), RCAP)  # node id order
    sreg = region_of_node[src0]
    gdeg = np.bincount(dst0, minlength=n).astype(np.int64)
    nA_n = np.bincount(dst0, weights=(sreg == 0), minlength=n).astype(np.int64)
    nB_n = np.bincount(dst0, weights=(sreg == 2), minlength=n).astype(np.int64)

    # chunk layout: per core, chunks of blocks; classes indexed 0..NBLK-1
    chunks = cfg.CHUNKS
    nb_of_chunk = [c[1] for c in chunks]
    assert sum(nb_of_chunk) == NBLK
    rcursor = {0: 0, 1: 17408, 2: 32768}
    chunk_start_pos = []
    for r, nb in chunks:
        chunk_start_pos.append(rcursor[r])
        rcursor[r] += NC * nb * P
    chunk_start_pos = np.array(chunk_start_pos + [NPAD])  # last entry unused
    chunk_cls0 = np.concatenate([[0], np.cumsum(nb_of_chunk)])

    # per region: sort real nodes by signature, form blocks, rank into classes
    pos_of_node = np.full(n, -1, np.int64)
    cls_cost = np.zeros(NBLK, np.int64)
    for r in range(3):
        nodes = np.nonzero(region_of_node == r)[0]
        k = np.lexsort((nB_n[nodes], nA_n[nodes], gdeg[nodes] // 4))
        nodes = nodes[k]
        nblocks_r = (RSTART[r + 1] - RSTART[r]) // P
        ncls_r = nblocks_r // NC
        # blocks of 128 consecutive sorted nodes (pads fill the tail)
        nfull = len(nodes)
        # block cost = max(gdeg, nA+nB) over its nodes (pads contribute 0)
        bcost = np.zeros(nblocks_r, np.int64)
        bi = np.arange(nfull) // P
        np.maximum.at(bcost, bi, np.maximum(gdeg[nodes], nA_n[nodes] + nB_n[nodes]))
        # class i = NC CONSECUTIVE blocks in signature order (keeps budgets
        # tight); rank classes by cost only to route small ones to the tail
        # chunk; per-core load is Sum(class budgets) regardless (1 blk/core).
        ccost = bcost.reshape(ncls_r, NC).max(axis=1)
        cls_rank = np.argsort(-ccost, kind="stable")  # region-class in cost order
        rchunks = [ci for ci, c in enumerate(chunks) if c[0] == r]
        cls_slots = []  # (chunk_idx, slot_in_chunk); tail chunk listed last
        for ci in rchunks:
            cls_slots += [(ci, s) for s in range(nb_of_chunk[ci])]
        assert len(cls_slots) == ncls_r
        for i in range(ncls_r):
            ci, slot = cls_slots[i]
            cls_id = chunk_cls0[ci] + slot
            rc = cls_rank[i]                      # region-class index
            for kc in range(NC):
                blk = rc * NC + kc
                base = (chunk_start_pos[ci] + kc * nb_of_chunk[ci] * P + slot * P)
                lo, hi = blk * P, min((blk + 1) * P, nfull)
                if hi > lo:
                    pos_of_node[nodes[lo:hi]] = base + np.arange(hi - lo)
                cls_cost[cls_id] = max(cls_cost[cls_id], bcost[blk])

    assert (pos_of_node >= 0).all()
    pos2old = np.full(NPAD, -1, np.int64)
    pos2old[pos_of_node] = np.arange(n)

    # position -> (core, local block b, j)
    pos = np.arange(NPAD)
    core_of_pos = np.zeros(NPAD, np.int64)
    cls_of_pos = np.zeros(NPAD, np.int64)
    for ci, (r, nb) in enumerate(chunks):
        s = chunk_start_pos[ci]
        e = s + NC * nb_of_chunk[ci] * P
        rel = pos[s:e] - s
        core_of_pos[s:e] = rel // (nb * P)
        cls_of_pos[s:e] = chunk_cls0[ci] + (rel % (nb * P)) // P
    j_of_pos = pos % P

    # dummy zero rows: a pad position in window A and one in window B
    pad_pos = np.nonzero(pos2old < 0)[0]
    zA = int(pad_pos[pad_pos < cfg.WA_LEN][-1])
    zB = int(pad_pos[pad_pos >= cfg.WB_OFF][-1])
    assert zA != zB

    # ---- per-edge window split with per-class budgets ----
    s_pos = pos_of_node[src0]
    d_pos = pos_of_node[dst0]
    eo = np.argsort(d_pos, kind="stable")
    s_s = s_pos[eo]
    d_s = d_pos[eo]
    starts = np.searchsorted(d_s, np.arange(NPAD + 1))

    isA = s_s < cfg.WB_OFF
    isB = s_s >= cfg.WA_LEN
    isF = ~(isA | isB)
    nAo = np.bincount(d_s, weights=isA, minlength=NPAD).astype(np.int64)
    nBo = np.bincount(d_s, weights=isB, minlength=NPAD).astype(np.int64)
    nf = np.bincount(d_s, weights=isF, minlength=NPAD).astype(np.int64)
    tot = nAo + nBo + nf

    cp = cls_of_pos
    mA = np.zeros(NBLK, np.int64); np.maximum.at(mA, cp, nAo)
    mB = np.zeros(NBLK, np.int64); np.maximum.at(mB, cp, nBo)
    mT = np.zeros(NBLK, np.int64); np.maximum.at(mT, cp, tot)
    M = np.maximum(mT, mA + mB)
    SA = np.clip((M + 1) // 2, mA, M - mB)
    SB = M - SA
    SAp = SA[cp]
    SBp = SB[cp]
    nA = np.clip(tot - SBp, nAo, np.minimum(nAo + nf, SAp))

    cFex = np.concatenate([[0], np.cumsum(isF)])
    frank = cFex[:-1] - cFex[starts[d_s]]
    goA = isA | (isF & (frank < (nA - nAo)[d_s]))
    goB = ~goA
    cAex = np.concatenate([[0], np.cumsum(goA)])
    slotA = cAex[:-1] - cAex[starts[d_s]]
    cBex = np.concatenate([[0], np.cumsum(goB)])
    slotB = cBex[:-1] - cBex[starts[d_s]]
    nB_ = tot - nA
    assert (nA <= SAp).all() and (nB_ <= SBp).all()
    assert (SA + SB > 0).all()

    # ---- groups: classes within each chunk, balanced by slots ----
    groups = []          # list of list of class ids
    grp_chunk = []       # chunk index of each group
    for ci, (r, nb) in enumerate(chunks):
        cls_list = list(range(chunk_cls0[ci], chunk_cls0[ci] + nb))
        ng = int(np.ceil(nb / cfg.GRP_TARGET))
        # greedy balance by SA+SB
        order_d = sorted(cls_list, key=lambda c: -(SA[c] + SB[c]))
        gsets = [[] for _ in range(ng)]
        gsum = [0] * ng
        for c in order_d:
            q = min(range(ng), key=lambda i: (gsum[i], i))
            gsets[q].append(c)
            gsum[q] += SA[c] + SB[c]
        for g in gsets:
            groups.append(sorted(g))
            grp_chunk.append(ci)
    NGRP = len(groups)

    # slot offsets per class within its group's A/B gathers
    oa = np.zeros(NBLK, np.int64)
    ob = np.zeros(NBLK, np.int64)
    grp_of = np.zeros(NBLK, np.int64)
    SAg = np.zeros(NGRP, np.int64)
    SBg = np.zeros(NGRP, np.int64)
    for q, bl in enumerate(groups):
        offa = 0
        for bq in bl:
            oa[bq] = offa
            offa += SA[bq]
            grp_of[bq] = q
        offb = 0
        for bq in bl:
            ob[bq] = offb
            offb += SB[bq]
        SAg[q] = offa
        SBg[q] = offb

    colA0 = np.zeros(NGRP, np.int64)
    colB0 = np.zeros(NGRP, np.int64)
    cur = 0
    for q in range(NGRP):
        colA0[q] = cur
        cur += int(SAg[q]) * P // 16
        colB0[q] = cur
        cur += int(SBg[q]) * P // 16
    idxcols = int(cur)

    # ---- build per-core idx tensors ----
    zA_rel = np.int16(zA)
    zB_rel = np.int16(zB - cfg.WB_OFF)
    idx_np = np.empty((NC, 128, idxcols), np.int16)
    for q in range(NGRP):
        idx_np[:, :, colA0[q]:colA0[q] + int(SAg[q]) * P // 16] = zA_rel
        idx_np[:, :, colB0[q]:colB0[q] + int(SBg[q]) * P // 16] = zB_rel

    e_core = core_of_pos[d_s]
    e_cls = cls_of_pos[d_s]
    e_j = j_of_pos[d_s]
    e_q = grp_of[e_cls]
    posA = (oa[e_cls] + slotA) * P + e_j
    colA = colA0[e_q] + posA // 16
    rowA = posA % 16
    posB = (ob[e_cls] + slotB) * P + e_j
    colB = colB0[e_q] + posB // 16
    rowB = posB % 16
    valA = s_s.astype(np.int16)
    valB = (s_s - cfg.WB_OFF).astype(np.int16)
    for k in range(NC):
        mk = e_core == k
        mAk = mk & goA
        mBk = mk & goB
        for r in range(8):
            idx_np[k, rowA[mAk] + 16 * r, colA[mAk]] = valA[mAk]
            idx_np[k, rowB[mBk] + 16 * r, colB[mBk]] = valB[mBk]

    # ---- dense per-core arrays ----
    rmask = pos2old >= 0
    dinv_pos = np.ones(NPAD, np.float32)
    dinv_pos[rmask] = dinv[pos2old[rmask]].astype(np.float32)
    x_pos = np.zeros((NPAD, cfg.F), np.float32)
    x_pos[rmask] = x[pos2old[rmask]]
    bias_pos = np.zeros((NPAD, 16), np.float32)
    bias_pos[rmask] = bias[pos2old[rmask]].astype(np.float32)

    xraw_t = x_pos * dinv_pos[:, None]            # pass-4 table: dinv*x
    # per-core [j, b] layouts: position of (core k, cls b, j)
    pos_kbj = np.zeros((NC, NBLK, P), np.int64)
    for ci, (r, nb) in enumerate(chunks):
        for kc in range(NC):
            for s in range(nb):
                base = chunk_start_pos[ci] + kc * nb * P + s * P
                pos_kbj[kc, chunk_cls0[ci] + s] = base + np.arange(P)

    in_maps = []
    mmats = np.ascontiguousarray(np.concatenate([M3, M2, M1], axis=1).astype(np.float32))
    m4 = np.ascontiguousarray(M4.astype(np.float32))
    for k in range(NC):
        pk = pos_kbj[k]                            # [NBLK, P] positions
        db = dinv_pos[pk].T.astype(np.float32)     # [P, NBLK]
        d2 = (db * db).astype(np.float32)
        xTloc = np.ascontiguousarray(
            x_pos[pk.reshape(-1)].T)               # [F, PER] raw x
        xd2 = np.ascontiguousarray(x_pos[pk].transpose(1, 0, 2)
                                   * (dinv_pos[pk] ** 2).T[:, :, None]).astype(np.float32)
        bias_blk = np.ascontiguousarray(bias_pos[pk].transpose(1, 0, 2)).astype(np.float32)
        in_maps.append(dict(
            xraw=xraw_t,
            idx=np.ascontiguousarray(idx_np[k]),
            xT=xTloc,
            db=np.ascontiguousarray(db),
            d2=np.ascontiguousarray(d2),
            xd2=xd2,
            bias_blk=bias_blk,
            mmats=mmats,
            m4=m4,
        ))

    layout = dict(SA=SA, SB=SB, groups=groups, grp_chunk=grp_chunk, oa=oa, ob=ob,
                  SAg=SAg, SBg=SBg, colA0=colA0, colB0=colB0, idxcols=idxcols,
                  chunks=chunks, chunk_start_pos=chunk_start_pos,
                  chunk_cls0=chunk_cls0)
    return in_maps, layout, pos_kbj, pos2old


# --------------------------------------------------------------------------
# device module
# --------------------------------------------------------------------------

def _build_module(cfg: Cfg, layout):
    P, PER, NPAD, NBLK, NC = cfg.P, cfg.PER, cfg.NPAD, cfg.NBLK, cfg.NCORES
    SA, SB = layout["SA"], layout["SB"]
    groups, grp_chunk = layout["groups"], layout["grp_chunk"]
    oa, ob = layout["oa"], layout["ob"]
    SAg, SBg = layout["SAg"], layout["SBg"]
    colA0, colB0 = layout["colA0"], layout["colB0"]
    idxcols = layout["idxcols"]
    chunks = layout["chunks"]
    chunk_start_pos = layout["chunk_start_pos"]
    chunk_cls0 = layout["chunk_cls0"]
    NGRP = len(groups)
    NCH = len(chunks)

    nc = bacc.Bacc("TRN2", target_bir_lowering=False, debug=False, num_devices=NC,
                   num_swdge_queues=cfg.NQ, dynamic_dma_scratch_size=16384)

    xraw = nc.dram_tensor("xraw", [NPAD, cfg.F], F32, kind="ExternalInput").ap()
    idx = nc.dram_tensor("idx", [128, idxcols], I16, kind="ExternalInput").ap()
    xT = nc.dram_tensor("xT", [cfg.F, PER], F32, kind="ExternalInput").ap()
    db_in = nc.dram_tensor("db", [P, NBLK], F32, kind="ExternalInput").ap()
    d2_in = nc.dram_tensor("d2", [P, NBLK], F32, kind="ExternalInput").ap()
    xd2_in = nc.dram_tensor("xd2", [P, NBLK, cfg.F], F32, kind="ExternalInput").ap()
    bias_in = nc.dram_tensor("bias_blk", [P, NBLK, 16], F32, kind="ExternalInput").ap()
    mmats = nc.dram_tensor("mmats", [cfg.F, 48], F32, kind="ExternalInput").ap()
    m4 = nc.dram_tensor("m4", [cfg.F, 16], F32, kind="ExternalInput").ap()
    out = nc.dram_tensor("out", [P, NBLK, 16], F32, kind="ExternalOutput").ap()

    with tile.TileContext(nc) as tc:
        with (
            tc.tile_pool(name="const", bufs=1) as cp,
            tc.tile_pool(name="dram", bufs=1, space="DRAM") as dp,
        ):
            idx_sb = cp.tile([128, idxcols], I16)
            nc.sync.dma_start(idx_sb[:], idx)
            xT_sb = cp.tile([cfg.F, PER], F32)
            nc.sync.dma_start(xT_sb[:], xT)
            mm_sb = cp.tile([cfg.F, 48], F32)
            nc.sync.dma_start(mm_sb[:], mmats)
            m4_sb = cp.tile([cfg.F, 16], F32)
            nc.sync.dma_start(m4_sb[:], m4)
            db_sb = cp.tile([P, NBLK], F32)
            nc.sync.dma_start(db_sb[:], db_in)
            d2_sb = cp.tile([P, NBLK], F32)
            nc.sync.dma_start(d2_sb[:], d2_in)
            xd2_sb = cp.tile([P, NBLK, cfg.F], F32)
            nc.sync.dma_start(xd2_sb[:], xd2_in)
            bias_sb = cp.tile([P, NBLK, 16], F32)
            nc.sync.dma_start(bias_sb[:], bias_in)
            ident = cp.tile([P, P], F32)
            make_identity(nc, ident[:])

            # each generation is a (w1, w2) pair: w1 = table rows [0,32768),
            # w2 = rows [17408, 50176); chunk exchanges write into one or both
            tw = [(dp.tile([cfg.WA_LEN, cfg.F], F32, name=f"tw{i}_1"),
                   dp.tile([cfg.WA_LEN, cfg.F], F32, name=f"tw{i}_2"))
                  for i in range(2)]
            # two combined AllGathers per pass: AG0 = chunks {0,1}, AG1 = {2,3}
            AGS = ((0, 1), (2, 3))
            ag_rows = [sum(chunks[c][1] for c in grp) * P for grp in AGS]
            ccin = [dp.tile([ag_rows[a], 16], F32, name=f"ccin{a}")
                    for a in range(2)]
            ccout = [[dp.tile([NC * ag_rows[a], 16], F32, addr_space="Shared",
                              name=f"ccout{p}_{a}") for a in range(2)]
                     for p in range(3)]
            # chunk -> (ag index, row offset inside the core's ccin block)
            ag_of_chunk = {}
            for a, grp in enumerate(AGS):
                off = 0
                for c in grp:
                    ag_of_chunk[c] = (a, off)
                    off += chunks[c][1] * P

            with (
                tc.tile_pool(name="gatha", bufs=cfg.LAG + 3) as gpa,
                tc.tile_pool(name="gathb", bufs=7) as gpb,
                tc.tile_pool(name="work", bufs=4) as wp,
                tc.tile_pool(name="stage", bufs=2) as sp,
                tc.tile_pool(name="psum", bufs=4, space="PSUM") as psp,
            ):
                # greedy queue schedule: call (in EMISSION order) -> least-loaded
                emit_order = []
                for step in range(NGRP + cfg.LAG):
                    if step < NGRP:
                        emit_order.append((step, "A", int(SAg[step])))
                    h = step - cfg.LAG
                    if h >= 0:
                        emit_order.append((h, "B", int(SBg[h])))
                # STRICT round-robin in emission order: each queue runs one
                # descgen at a time and the gpsimd engine dispatches in order,
                # so consecutive same-queue calls would head-of-line block.
                qsched = {}
                rr = [0]
                for q, part, sz in emit_order:
                    if sz:
                        qsched[(q, part)] = rr[0] % cfg.NQ
                        rr[0] += 1

                def make_stages(tag):
                    return [sp.tile([P, nb, 16], F32, tag=f"{tag}{c}",
                                    name=f"st_{tag}{c}")
                            for c, (r, nb) in enumerate(chunks)]

                def st_slot(sts, bq):
                    for c in range(NCH):
                        if bq < chunk_cls0[c + 1]:
                            return sts[c][:, bq - chunk_cls0[c], :]
                    raise AssertionError

                def exchange_start(c, st_tile):
                    a, off = ag_of_chunk[c]
                    nb = chunks[c][1]
                    nc.sync.dma_start(
                        ccin[a][off:off + nb * P, :]
                        .rearrange("(b p) f -> p b f", p=P), st_tile[:])

                def exchange_finish(p, a, target):
                    # deferred so the gpsimd-resident collective trigger never
                    # blocks gather dispatch waiting on the ccin DMAs
                    w1, w2 = target
                    nc.gpsimd.collective_compute(
                        "AllGather", mybir.AluOpType.bypass,
                        replica_groups=[list(range(NC))],
                        ins=[ccin[a][:]], outs=[ccout[p][a][:]],
                    )
                    blk = ag_rows[a]
                    for k in range(NC):
                        for c in AGS[a]:
                            _, off = ag_of_chunk[c]
                            nb = chunks[c][1]
                            src0_ = k * blk + off
                            s = int(chunk_start_pos[c]) + k * nb * P
                            rows = nb * P
                            if s < cfg.WA_LEN:
                                hi = min(s + rows, cfg.WA_LEN)
                                nc.scalar.dma_start(
                                    w1[s:hi, 0:16],
                                    ccout[p][a][src0_:src0_ + hi - s, :])
                            if s + rows > cfg.WB_OFF:
                                lo = max(s, cfg.WB_OFF)
                                nc.scalar.dma_start(
                                    w2[lo - cfg.WB_OFF:s + rows - cfg.WB_OFF, 0:16],
                                    ccout[p][a][src0_ + lo - s:src0_ + rows, :])

                last_of_chunk = {}
                for q in range(NGRP):
                    last_of_chunk[grp_chunk[q]] = q

                def run_pass(winA, winB, width, epi, chunk_hook=None):
                    # A-gathers issue LAG groups ahead of B-gathers+reduces so
                    # queue FIFOs stay busy across the pass boundary (A only
                    # depends on the first two chunk exchanges of the prior
                    # pass, B on all four).
                    gtA = {}
                    gtB = {}

                    def emit_A(q):
                        sag = int(SAg[q])
                        if not sag:
                            return
                        t = gpa.tile([P, sag, cfg.F], F32, tag="gtA", name="gtA")
                        gtA[q] = t
                        nc.gpsimd.dma_gather(
                            out_ap=t[:],
                            in_ap=winA,
                            idxs_ap=idx_sb[:, int(colA0[q]):int(colA0[q]) + sag * P // 16],
                            num_idxs=sag * P,
                            num_idxs_reg=sag * P,
                            elem_size=cfg.F,
                            single_packet=False,
                            queue_num=qsched[(q, "A")],
                        )

                    def emit_B(q):
                        sbg = int(SBg[q])
                        if not sbg:
                            return
                        t = gpb.tile([P, sbg, cfg.F], F32, tag="gtB", name="gtB")
                        gtB[q] = t
                        nc.gpsimd.dma_gather(
                            out_ap=t[:],
                            in_ap=winB,
                            idxs_ap=idx_sb[:, int(colB0[q]):int(colB0[q]) + sbg * P // 16],
                            num_idxs=sbg * P,
                            num_idxs_reg=sbg * P,
                            elem_size=cfg.F,
                            single_packet=False,
                            queue_num=qsched[(q, "B")],
                        )

                    def emit_reduces(q):
                        for bq in groups[q]:
                            acc = wp.tile([P, cfg.F], F32, tag="acc")
                            wrote = False
                            if SA[bq]:
                                a0, a1 = int(oa[bq]), int(oa[bq] + SA[bq])
                                nc.vector.reduce_sum(
                                    out=acc[:, 0:width],
                                    in_=gtA[q][:, a0:a1, 0:width].rearrange("p s f -> p f s"),
                                    axis=mybir.AxisListType.X,
                                )
                                wrote = True
                            if SB[bq]:
                                b0_, b1_ = int(ob[bq]), int(ob[bq] + SB[bq])
                                if wrote:
                                    acc2 = wp.tile([P, cfg.F], F32, tag="acc2")
                                    nc.vector.reduce_sum(
                                        out=acc2[:, 0:width],
                                        in_=gtB[q][:, b0_:b1_, 0:width].rearrange("p s f -> p f s"),
                                        axis=mybir.AxisListType.X,
                                    )
                                    nc.vector.tensor_add(
                                        out=acc[:, 0:width], in0=acc[:, 0:width],
                                        in1=acc2[:, 0:width])
                                else:
                                    nc.vector.reduce_sum(
                                        out=acc[:, 0:width],
                                        in_=gtB[q][:, b0_:b1_, 0:width].rearrange("p s f -> p f s"),
                                        axis=mybir.AxisListType.X,
                                    )
                            epi(bq, acc)

                    pending_finish = []
                    for step in range(NGRP + cfg.LAG + cfg.CC_DELAY):
                        if step < NGRP:
                            emit_A(step)
                        h = step - cfg.LAG
                        if chunk_hook is not None:
                            for da, due in list(pending_finish):
                                if step - due >= cfg.CC_DELAY or h >= NGRP - 1:
                                    chunk_hook[1](da)
                                    pending_finish.remove((da, due))
                        if 0 <= h < NGRP:
                            emit_B(h)
                            emit_reduces(h)
                            if chunk_hook is not None and h == last_of_chunk[grp_chunk[h]]:
                                c = grp_chunk[h]
                                chunk_hook[0](c)
                                if c == AGS[ag_of_chunk[c][0]][-1]:
                                    pending_finish.append((ag_of_chunk[c][0], step))

                # ---- pass 4: gather dinv*x (64-wide), project via M4 ----
                st4 = make_stages("s")

                def epi4(bq, R):
                    rs = wp.tile([P, cfg.F], F32, tag="rs")
                    # rs = db*R + d2*x_own   (u such that st = db*(x@M3) + db*u@M4)
                    nc.scalar.activation(rs[:], R[:], AF.Copy, scale=db_sb[:, bq:bq + 1])
                    nc.vector.tensor_add(out=rs[:], in0=rs[:], in1=xd2_sb[:, bq, :])
                    pT = psp.tile([cfg.F, P], F32, tag="pT")
                    nc.tensor.transpose(pT[:], rs[:], ident[:])
                    rsT = wp.tile([cfg.F, P], F32, tag="rsT")
                    nc.vector.tensor_copy(rsT[:], pT[:])
                    ps = psp.tile([P, 16], F32, tag="ps")
                    nc.tensor.matmul(out=ps[:], lhsT=xT_sb[:, bq * P:(bq + 1) * P],
                                     rhs=mm_sb[:, 0:16], start=True, stop=False)
                    nc.tensor.matmul(out=ps[:], lhsT=rsT[:], rhs=m4_sb[:],
                                     start=False, stop=True)
                    nc.scalar.activation(st_slot(st4, bq), ps[:], AF.Copy,
                                         scale=db_sb[:, bq:bq + 1])

                run_pass(xraw[0:cfg.WA_LEN, :], xraw[cfg.WB_OFF:NPAD, :], cfg.F,
                         epi4, chunk_hook=(lambda c: exchange_start(c, st4[c]),
                                           lambda a: exchange_finish(0, a, tw[0])))

                # ---- passes 3 and 2 ----
                def mk_epi(mcol, sts_prev, sts_new):
                    def epi(bq, R):
                        ps = psp.tile([P, 16], F32, tag="ps")
                        nc.tensor.matmul(out=ps[:],
                                         lhsT=xT_sb[:, bq * P:(bq + 1) * P],
                                         rhs=mm_sb[:, mcol:mcol + 16],
                                         start=True, stop=True)
                        # acc_full = R + prev_stage (self-loop)
                        accf = wp.tile([P, 16], F32, tag="accf")
                        nc.vector.tensor_add(out=accf[:], in0=R[:, 0:16],
                                             in1=st_slot(sts_prev, bq))
                        ta = wp.tile([P, 16], F32, tag="ta")
                        nc.scalar.activation(ta[:], ps[:], AF.Copy,
                                             scale=db_sb[:, bq:bq + 1])
                        tb = wp.tile([P, 16], F32, tag="tb")
                        nc.scalar.activation(tb[:], accf[:], AF.Copy,
                                             scale=d2_sb[:, bq:bq + 1])
                        nc.vector.tensor_add(out=st_slot(sts_new, bq),
                                             in0=ta[:], in1=tb[:])
                    return epi

                st3 = make_stages("s")
                run_pass(tw[0][0][:], tw[0][1][:], 16, mk_epi(16, st4, st3),
                         chunk_hook=(lambda c: exchange_start(c, st3[c]),
                                     lambda a: exchange_finish(1, a, tw[1])))

                st2 = make_stages("s")
                run_pass(tw[1][0][:], tw[1][1][:], 16, mk_epi(32, st3, st2),
                         chunk_hook=(lambda c: exchange_start(c, st2[c]),
                                     lambda a: exchange_finish(2, a, tw[0])))

                # ---- pass 1: final output ----
                st1 = make_stages("s")

                def epi1(bq, R):
                    accf = wp.tile([P, 16], F32, tag="accf")
                    nc.vector.tensor_add(out=accf[:], in0=R[:, 0:16],
                                         in1=st_slot(st2, bq))
                    t1 = wp.tile([P, 16], F32, tag="ta")
                    nc.scalar.activation(t1[:], accf[:], AF.Copy,
                                         scale=db_sb[:, bq:bq + 1])
                    nc.vector.tensor_add(out=st_slot(st1, bq), in0=t1[:],
                                         in1=bias_sb[:, bq, :])

                run_pass(tw[0][0][:], tw[0][1][:], 16, epi1)
                for c in range(NCH):
                    lo, hi = int(chunk_cls0[c]), int(chunk_cls0[c + 1])
                    nc.sync.dma_start(out[:, lo:hi, :], st1[c][:])

    return nc


# --------------------------------------------------------------------------
# entry point
# --------------------------------------------------------------------------

def _run(inputs, cfg: Cfg, runner=None, **run_kwargs):
    global LAST_RESULTS
    in_maps, layout, pos_kbj, pos2old = _host_prep(inputs, cfg)
    nc = _build_module(cfg, layout)
    nc.compile()
    if runner is None:
        res = run_bass_kernel_spmd(nc, in_maps, core_ids=list(range(cfg.NCORES)),
                                   **run_kwargs)
        LAST_RESULTS = res
        outs = res.results
    else:
        outs = runner(nc, in_maps)
    full = np.empty((cfg.NPAD, 16), np.float32)
    for k in range(cfg.NCORES):
        o = np.asarray(outs[k]["out"])  # [P, NBLK, 16]
        full[pos_kbj[k].reshape(-1)] = o.transpose(1, 0, 2).reshape(cfg.PER, 16)
    old2new = np.empty(cfg.N, np.int64)
    rmask = pos2old >= 0
    old2new[pos2old[rmask]] = np.nonzero(rmask)[0]
    return full[old2new]


def kernel(**inputs) -> np.ndarray:
    return _run(inputs, CFG)
